# revision 23
# baseline (speedup 1.0000x reference)
"""Tree-GRU (arity-8, depth-5) over embedded leaves on 8 TRN2 NeuronCores.

Sharding: data-parallel over subtrees. Each core takes 4096 contiguous leaves
and runs levels 5..2 of the tree locally (512 -> 64 -> 8 -> 1 parents). The
root (level 1, 8 children = the 8 cores' level-2 outputs) is a trivial
16-matvec GRU done on host after gathering the per-core [384] outputs.

Device layout is feature-transposed: tensors live as [128 part, 3 ktile, ...]
with feature f = 128*k + p, so the GRU matmuls contract the partition dim.
Node storage is flat leaf-order (child-fastest), so all elementwise state
updates and the level-boundary x_next writes are contiguous; only the matmul
rhs / gi reads use stride-8 child slices. Weights are host-pre-transposed
into lhsT tiles; matmul dtype bf16 with fp32 PSUM accumulation.

Leaf level: embedding gather (indirect DMA, bf16 table, 4 SWDGE queues)
feeds PE transposes; step 0 (h=0, gi-only) runs in 4 sub-chunks of 128
parents that track gather completion; steps 1-7 ping-pong 2 chunks of 256.

Small levels (64/8/1 parents): the input transform gi for the whole level
(all 8 children x all parents) is precomputed in one batched matmul pass
(biases folded in via a K=1 ones-column matmul) and stored in SBUF, so the
sequential per-step work is only the recurrent matmul + gate chain. Per step
the r/z gi rows are injected into PSUM via an identity matmul (one start=True
covering MM per bank), the hn bias via a K=3 one-hot matmul, and the h update
uses h' = zc*n + z*h with zc = sigmoid(-pre_z) so z*h is computed off-path
(gpsimd) while tanh runs. The 1/8 output-mean scale is folded into a
pre-scaled copy of W_ih used by the gi passes; per-step output sums are
accumulated (hacc) and the final step writes the next level's input directly
as a fused raw-sum add.
"""

import numpy as np
import ml_dtypes

ARITY = 8
DIM = 384
VOCAB = 32000
NCORES = 8
P = 128
J = 3  # DIM // 128 feature tiles
N_LEAVES = 32768
LEAVES_CORE = N_LEAVES // NCORES  # 4096

BF16 = ml_dtypes.bfloat16

_PROG_CACHE = {}


def _levels_for(n_leaves_core):
    levels = []
    p = n_leaves_core // ARITY
    while p >= 1:
        levels.append(p)
        p //= ARITY
    assert levels[-1] == 1
    return levels


def _emit(tc, nc, aps, n_leaves_core):
    import concourse.mybir as mybir
    import concourse.bass as bass

    f32 = mybir.dt.float32
    bf16 = mybir.dt.bfloat16
    Sig = mybir.ActivationFunctionType.Sigmoid
    Tanh = mybir.ActivationFunctionType.Tanh
    Add = mybir.AluOpType.add
    Sub = mybir.AluOpType.subtract
    Mult = mybir.AluOpType.mult

    (tokens, embed, wih_t, wih_s, whh_t, biases, biases_mm, bias1, onehot3,
     ones, identity, out_xh) = aps
    levels = _levels_for(n_leaves_core)
    P5 = levels[0]
    n_gtiles = n_leaves_core // P  # 32

    from contextlib import ExitStack

    with ExitStack() as ctx:
        const = ctx.enter_context(tc.tile_pool(name="const", bufs=1))
        xpool = ctx.enter_context(tc.tile_pool(name="xpool", bufs=1))
        state = ctx.enter_context(tc.tile_pool(name="state", bufs=1))
        gates = ctx.enter_context(tc.tile_pool(name="gates", bufs=6))
        gpool = ctx.enter_context(tc.tile_pool(name="gpool", bufs=1))
        pspool = ctx.enter_context(tc.tile_pool(name="pspool", bufs=8, space="PSUM"))

        # ---- tokens first, then kick off all gathers (2 SWDGE queues).
        # The gathers are paced by gpsimd descriptor generation (~1.1us per
        # 128 rows), so nothing else may sit ahead of them in the gpsimd
        # FIFO -- the identity build comes after.
        tok_sb = const.tile([P, n_gtiles], mybir.dt.int32)
        nc.sync.dma_start(tok_sb[:], tokens.rearrange("(g p) -> p g", p=P))

        xgs = []
        for g in range(n_gtiles):
            xg = gpool.tile([P, DIM], bf16, name="xg", tag="xg", bufs=n_gtiles)
            gi_inst = nc.gpsimd.indirect_dma_start(
                out=xg[:],
                out_offset=None,
                in_=embed[:],
                in_offset=bass.IndirectOffsetOnAxis(ap=tok_sb[:, g : g + 1], axis=0),
            )
            if g % 2 == 1:
                gi_inst.ins.queue = "qPoolDynamic1"
            xgs.append(xg)

        # identity comes in via DMA so no gpsimd work sits ahead of the
        # gather descriptor generation
        ident = const.tile([P, P], bf16)
        nc.sync.dma_start(ident[:], identity[:])

        # ---- constants / weights ----
        wih_sb = const.tile([P, J, 9, P], bf16)
        wih_s_sb = const.tile([P, J, 9, P], bf16)
        whh_sb = const.tile([P, J, 9, P], bf16)
        bias_sb = const.tile([P, 12], f32)
        bias3_sb = const.tile([3, 4, P], bf16)
        bias1_sb = const.tile([1, 9, P], bf16)
        onehot3_sb = const.tile([3, 3, 512], bf16)
        ones_sb = const.tile([1, 512], bf16)
        nc.sync.dma_start(wih_sb[:], wih_t[:])
        nc.sync.dma_start(wih_s_sb[:], wih_s[:])
        nc.sync.dma_start(whh_sb[:], whh_t[:])
        nc.sync.dma_start(bias_sb[:], biases[:])
        nc.sync.dma_start(bias3_sb[:], biases_mm[:])
        nc.sync.dma_start(bias1_sb[:], bias1[:])
        nc.sync.dma_start(onehot3_sb[:], onehot3[:])
        nc.sync.dma_start(ones_sb[:], ones[:])

        # child-major x per level: [P, J, 8, Pl] (contiguous matmul rhs)
        x_in = {}
        for Pl in levels:
            x_in[Pl] = xpool.tile([P, J, ARITY, Pl], bf16, name=f"x{Pl}", tag=f"x{Pl}")

        def psum_tile():
            return pspool.tile([P, 512], f32, name="ps", tag="ps")

        def warm(n):
            # PE warm-keeper: HAM re-throttles the PE clock to 1.2 GHz after
            # ~3.4us of idle; during the latency-bound small levels, issue
            # dependency-free matmuls so the gate-chain windows don't cool
            # the PE and the next real matmul burst runs at 2.4 GHz.
            for _ in range(n):
                wp = pspool.tile([P, 512], f32, name="warm", tag="ps")
                nc.tensor.matmul(
                    wp[:, :512], ident[:], wih_sb[:, 0, 0:4, :], start=True, stop=True
                )

        def new_state(name, dtype, Pl):
            return state.tile([P, J, Pl], dtype, name=f"{name}{Pl}", tag=f"{name}{Pl}")

        # =====================  LEAF LEVEL (Pl = P5)  =====================
        x5 = x_in[P5]
        h = new_state("h", bf16, P5)
        # permuted (child-major) accumulator so the level-end x_next add is a
        # contiguous write into the next level's child-major x
        hacc = state.tile([P, J, ARITY, P5 // ARITY], f32, name="hacc5", tag="hacc5")

        def emit_transposes(g0, g1):
            for g in range(g0, g1):
                for j in range(J):
                    tp = pspool.tile([P, 512], bf16, name="tp", tag="ps")
                    nc.tensor.transpose(
                        tp[:, :P], xgs[g][:, j * P : (j + 1) * P], ident[:]
                    )
                    nc.vector.tensor_copy(
                        out=x5[:, j, :, 16 * g : 16 * (g + 1)],
                        in_=tp[:, :P].rearrange("p (par c) -> p c par", c=ARITY),
                    )

        level_csum = [None]

        with nc.named_scope("leaf_t0"):
            # step 0: h=0, gi only; 4 sub-chunks of 128 parents (8 gtiles each)
            NSC = P5 // 4  # 128 parents per sub-chunk
            gsc = n_gtiles // 4
            c0 = ARITY - 1  # first GRU input is the last child
            for sc in range(4):
                emit_transposes(sc * gsc, (sc + 1) * gsc)
                sl = slice(sc * NSC, (sc + 1) * NSC)
                ps_r = [psum_tile()[:, :NSC] for _ in range(3)]
                ps_z = [psum_tile()[:, :NSC] for _ in range(3)]
                ps_in = [psum_tile()[:, :NSC] for _ in range(3)]
                for ps, moff in ((ps_r, 0), (ps_z, 3), (ps_in, 6)):
                    for m in range(3):
                        for k in range(J):
                            nc.tensor.matmul(
                                ps[m],
                                wih_sb[:, k, moff + m, :],
                                x5[:, k, c0, sl],
                                start=(k == 0),
                                stop=(k == 2),
                            )
                r_sb = gates.tile([P, J, NSC], bf16, name="r0", tag="r0")
                z_sb = gates.tile([P, J, NSC], bf16, name="z0", tag="z0")
                n_sb = gates.tile([P, J, NSC], bf16, name="n0", tag="n0")
                rhn = gates.tile([P, J, NSC], f32, name="rhn0", tag="rhn0")
                t1 = gates.tile([P, J, NSC], bf16, name="t10", tag="t10")
                for m in range(3):
                    nc.scalar.activation(
                        r_sb[:, m], ps_r[m], Sig, bias=bias_sb[:, m : m + 1]
                    )
                for m in range(3):
                    nc.scalar.activation(
                        z_sb[:, m], ps_z[m], Sig, bias=bias_sb[:, 3 + m : 4 + m]
                    )
                for m in range(3):
                    nc.vector.tensor_scalar_mul(
                        rhn[:, m], r_sb[:, m], bias_sb[:, 6 + m : 7 + m]
                    )
                for m in range(3):
                    nc.vector.tensor_tensor(
                        out=rhn[:, m], in0=rhn[:, m], in1=ps_in[m], op=Add
                    )
                for m in range(3):
                    nc.scalar.activation(
                        n_sb[:, m], rhn[:, m], Tanh, bias=bias_sb[:, 9 + m : 10 + m]
                    )
                # h0=0: h' = n - z*n
                hsl = h[:, :, sl]
                nc.vector.tensor_tensor(out=t1[:], in0=z_sb[:], in1=n_sb[:], op=Mult)
                nc.vector.tensor_tensor(out=hsl, in0=n_sb[:], in1=t1[:], op=Sub)
                nc.gpsimd.tensor_copy(
                    out=hacc[:, :, :, sc * (NSC // ARITY) : (sc + 1) * (NSC // ARITY)],
                    in_=hsl.rearrange("p j (q c) -> p j c q", c=ARITY),
                )

        NCH = 256
        nch = P5 // NCH
        for t in range(1, ARITY):
            c = ARITY - 1 - t
            with nc.named_scope(f"leaf_t{t}"):
                for ch in range(nch):
                    sl = slice(ch * NCH, (ch + 1) * NCH)
                    ps_r = [psum_tile()[:, :NCH] for _ in range(3)]
                    ps_z = [psum_tile()[:, :NCH] for _ in range(3)]
                    ps_in = [psum_tile()[:, :NCH] for _ in range(3)]
                    ps_hn = [psum_tile()[:, :NCH] for _ in range(3)]
                    for ps, moff in ((ps_r, 0), (ps_z, 3), (ps_in, 6)):
                        for m in range(3):
                            for k in range(J):
                                nc.tensor.matmul(
                                    ps[m],
                                    wih_sb[:, k, moff + m, :],
                                    x5[:, k, c, sl],
                                    start=(k == 0),
                                    stop=(k == 2 and moff == 6),
                                )
                    for ps, moff in ((ps_r, 0), (ps_z, 3), (ps_hn, 6)):
                        for m in range(3):
                            for k in range(J):
                                nc.tensor.matmul(
                                    ps[m],
                                    whh_sb[:, k, moff + m, :],
                                    h[:, k, sl],
                                    start=(k == 0 and moff == 6),
                                    stop=(k == 2),
                                )

                    r_sb = gates.tile([P, J, NCH], bf16, name="r_sb", tag="r_sb")
                    z_sb = gates.tile([P, J, NCH], bf16, name="z_sb", tag="z_sb")
                    n_sb = gates.tile([P, J, NCH], bf16, name="n_sb", tag="n_sb")
                    rhn = gates.tile([P, J, NCH], f32, name="rhn", tag="rhn")
                    t1 = gates.tile([P, J, NCH], bf16, name="t1", tag="t1")

                    for m in range(3):
                        nc.scalar.activation(
                            r_sb[:, m], ps_r[m], Sig, bias=bias_sb[:, m : m + 1]
                        )
                    for m in range(3):
                        nc.scalar.activation(
                            z_sb[:, m], ps_z[m], Sig, bias=bias_sb[:, 3 + m : 4 + m]
                        )
                    for m in range(3):
                        nc.vector.scalar_tensor_tensor(
                            out=rhn[:, m],
                            in0=ps_hn[m],
                            scalar=bias_sb[:, 6 + m : 7 + m],
                            in1=r_sb[:, m],
                            op0=Add,
                            op1=Mult,
                        )
                    for m in range(3):
                        nc.vector.tensor_tensor(
                            out=rhn[:, m], in0=rhn[:, m], in1=ps_in[m], op=Add
                        )
                    for m in range(3):
                        nc.scalar.activation(
                            n_sb[:, m], rhn[:, m], Tanh, bias=bias_sb[:, 9 + m : 10 + m]
                        )

                    # h' = n + z*(h - n)
                    hsl = h[:, :, sl]
                    nc.vector.tensor_tensor(out=t1[:], in0=hsl, in1=n_sb[:], op=Sub)
                    nc.vector.tensor_tensor(out=t1[:], in0=z_sb[:], in1=t1[:], op=Mult)
                    nc.vector.tensor_tensor(out=hsl, in0=n_sb[:], in1=t1[:], op=Add)
                    hperm = hsl.rearrange("p j (q c) -> p j c q", c=ARITY)
                    qsl = slice(ch * NCH // ARITY, (ch + 1) * NCH // ARITY)
                    if t == ARITY - 1:
                        if ch == 0:
                            csum = state.tile(
                                [P, J, P5 // ARITY], f32, name="csum5", tag="csum5"
                            )
                            level_csum[0] = csum
                        nc.vector.tensor_reduce(
                            out=level_csum[0][:, :, qsl],
                            in_=hsl.rearrange("p j (q c) -> p j q c", c=ARITY),
                            axis=mybir.AxisListType.X,
                            op=Add,
                        )
                        xn = x_in[P5 // ARITY]
                        for j in range(J):
                            eng = nc.gpsimd if j == 2 else nc.vector
                            eng.tensor_tensor(
                                out=xn[:, j, :, qsl],
                                in0=hacc[:, j, :, qsl],
                                in1=hperm[:, j],
                                op=Add,
                            )
                        # bridge the level-end tail so the PE stays warm into
                        # the gi_64 pass
                        warm(8)
                    else:
                        nc.gpsimd.tensor_tensor(
                            out=hacc[:, :, :, qsl],
                            in0=hacc[:, :, :, qsl],
                            in1=hperm,
                            op=Add,
                        )

        # =====================  SMALL LEVELS (64, 8, 1)  ==================
        for Pl in levels[1:]:
            NC8 = ARITY * Pl  # children count = gi batch size
            with nc.named_scope(f"gi_{Pl}"):
                # gi stored child-major [P, 9, 8, Pl]: the gi pass rhs is the
                # child-major x (contiguous), so PSUM comes out (c, q)-ordered
                gi_sb = xpool.tile([P, 9, ARITY, Pl], bf16, name=f"gi{Pl}",
                                   tag=f"gi{Pl}")
                # m-order: r (0,1,2) first so step 0's r-inject unblocks early,
                # then z (3,4,5), then n (6,7,8)
                for mi, m in enumerate((0, 1, 2, 3, 4, 5, 6, 7, 8)):
                    ps = psum_tile()[:, :NC8]
                    nc.tensor.matmul(
                        ps, bias1_sb[:, m, :], ones_sb[:, :NC8],
                        start=True, stop=False,
                    )
                    for k in range(J):
                        nc.tensor.matmul(
                            ps,
                            wih_s_sb[:, k, m, :],
                            x_in[Pl][:, k, :, :],
                            start=False,
                            stop=(k == 2),
                        )
                    # alternate copy engine so the PSUM->SBUF drain keeps up
                    # with the matmul waves
                    if mi % 2 == 0:
                        nc.vector.tensor_copy(
                            out=gi_sb[:, m].rearrange("p c q -> p (c q)"), in_=ps
                        )
                    else:
                        nc.scalar.copy(
                            out=gi_sb[:, m].rearrange("p c q -> p (c q)"), in_=ps
                        )
                    if Pl == 64:
                        warm(1)

            csum = level_csum[0]
            h = new_state("h", bf16, Pl)
            hacc = new_state("hacc", f32, Pl)
            nc.scalar.mul(h[:], csum[:], 1.0 / ARITY)

            for t in range(ARITY):
                c = ARITY - 1 - t
                with nc.named_scope(f"lv{Pl}_t{t}"):
                    N3 = 3 * Pl
                    ps_z, ps_r, ps_hn = psum_tile(), psum_tile(), psum_tile()

                    def view3(pst):
                        return pst[:, :N3].rearrange("p (j n) -> p j n", j=3)

                    def msl(pst, m):
                        return pst[:, m * Pl : (m + 1) * Pl]

                    # r first: sigma(r) heads the serial chain, so its PSUM
                    # group must close first and nothing may sit ahead of
                    # sigma(r) in the scalar FIFO
                    nc.tensor.matmul(
                        ps_r[:, :N3], ident[:], gi_sb[:, 0:3, c, :],
                        start=True, stop=False,
                    )
                    for m in range(3):
                        for k in range(J):
                            nc.tensor.matmul(
                                msl(ps_r, m), whh_sb[:, k, m, :], h[:, k, :],
                                start=False, stop=(m == 2 and k == 2),
                            )
                    # z
                    nc.tensor.matmul(
                        ps_z[:, :N3], ident[:], gi_sb[:, 3:6, c, :],
                        start=True, stop=False,
                    )
                    for m in range(3):
                        for k in range(J):
                            nc.tensor.matmul(
                                msl(ps_z, m), whh_sb[:, k, 3 + m, :], h[:, k, :],
                                start=False, stop=(m == 2 and k == 2),
                            )
                    # hn: bias via one-hot, then hh
                    nc.tensor.matmul(
                        ps_hn[:, :N3], bias3_sb[:, 2, :], onehot3_sb[:, :, :Pl],
                        start=True, stop=False,
                    )
                    for m in range(3):
                        for k in range(J):
                            nc.tensor.matmul(
                                msl(ps_hn, m), whh_sb[:, k, 6 + m, :], h[:, k, :],
                                start=False, stop=(m == 2 and k == 2),
                            )
                    warm(6 if Pl == 64 else 4)

                    z_sb = gates.tile([P, J, Pl], bf16, name="z_sb", tag="z_sb")
                    zc_sb = gates.tile([P, J, Pl], bf16, name="zc_sb", tag="zc_sb")
                    r_sb = gates.tile([P, J, Pl], bf16, name="r_sb", tag="r_sb")
                    n_sb = gates.tile([P, J, Pl], bf16, name="n_sb", tag="n_sb")
                    rhn = gates.tile([P, J, Pl], f32, name="rhn", tag="rhn")
                    t1 = gates.tile([P, J, Pl], f32, name="t1", tag="t1")
                    t2 = gates.tile([P, J, Pl], f32, name="t2", tag="t2")

                    nc.scalar.activation(r_sb[:], view3(ps_r), Sig)
                    nc.scalar.activation(z_sb[:], view3(ps_z), Sig)
                    nc.scalar.activation(zc_sb[:], view3(ps_z), Sig, scale=-1.0)
                    # t2 = z*h off-path while r/n compute
                    nc.gpsimd.tensor_tensor(out=t2[:], in0=z_sb[:], in1=h[:], op=Mult)
                    nc.vector.tensor_tensor(
                        out=rhn[:], in0=view3(ps_hn), in1=r_sb[:], op=Mult
                    )
                    nc.vector.tensor_tensor(
                        out=rhn[:], in0=rhn[:], in1=gi_sb[:, 6:9, c, :], op=Add
                    )
                    nc.scalar.activation(n_sb[:], rhn[:], Tanh)
                    # h' = zc*n + z*h
                    nc.vector.tensor_tensor(out=t1[:], in0=zc_sb[:], in1=n_sb[:], op=Mult)
                    nc.vector.tensor_tensor(out=h[:], in0=t1[:], in1=t2[:], op=Add)

                    if t == 0:
                        nc.gpsimd.tensor_copy(out=hacc[:], in_=h[:])
                    elif t == ARITY - 1 and Pl > 1:
                        csum = state.tile(
                            [P, J, Pl // ARITY], f32, name=f"csum{Pl}", tag=f"csum{Pl}"
                        )
                        level_csum[0] = csum
                        nc.vector.tensor_reduce(
                            out=csum[:],
                            in_=h[:].rearrange("p j (q c) -> p j q c", c=ARITY),
                            axis=mybir.AxisListType.X,
                            op=Add,
                        )
                        xn = x_in[Pl // ARITY]
                        nc.vector.tensor_tensor(
                            out=xn[:],
                            in0=hacc[:].rearrange("p j (q c) -> p j c q", c=ARITY),
                            in1=h[:].rearrange("p j (q c) -> p j c q", c=ARITY),
                            op=Add,
                        )
                    else:
                        nc.gpsimd.tensor_tensor(
                            out=hacc[:], in0=hacc[:], in1=h[:], op=Add
                        )

        # ---- outputs: [P, 2, J] = (x_root, h_root) ----
        out_t = state.tile([P, 2, J], f32, name="out_t", tag="out_t")
        nc.scalar.mul(out_t[:, 0], hacc[:, :, 0], 1.0 / ARITY)
        nc.vector.tensor_copy(out=out_t[:, 1], in_=h[:, :, 0])
        nc.sync.dma_start(out_xh[:], out_t[:])


def _build_program(n_leaves_core):
    if n_leaves_core in _PROG_CACHE:
        return _PROG_CACHE[n_leaves_core]
    import concourse.bacc as bacc
    import concourse.mybir as mybir
    import concourse.tile as tile

    f32 = mybir.dt.float32
    bf16 = mybir.dt.bfloat16

    nc = bacc.Bacc(
        "TRN2",
        target_bir_lowering=False,
        debug=False,
        enable_asserts=False,
        num_devices=NCORES,
        num_swdge_queues=4,
    )
    tokens = nc.dram_tensor("tokens", [n_leaves_core], mybir.dt.int32, kind="ExternalInput").ap()
    embed = nc.dram_tensor("embed", [VOCAB, DIM], bf16, kind="ExternalInput").ap()
    wih_t = nc.dram_tensor("wih_t", [P, J, 9, P], bf16, kind="ExternalInput").ap()
    wih_s = nc.dram_tensor("wih_s", [P, J, 9, P], bf16, kind="ExternalInput").ap()
    whh_t = nc.dram_tensor("whh_t", [P, J, 9, P], bf16, kind="ExternalInput").ap()
    biases = nc.dram_tensor("biases", [P, 12], f32, kind="ExternalInput").ap()
    biases_mm = nc.dram_tensor("biases_mm", [3, 4, P], bf16, kind="ExternalInput").ap()
    bias1 = nc.dram_tensor("bias1", [1, 9, P], bf16, kind="ExternalInput").ap()
    onehot3 = nc.dram_tensor("onehot3", [3, 3, 512], bf16, kind="ExternalInput").ap()
    ones = nc.dram_tensor("ones", [1, 512], bf16, kind="ExternalInput").ap()
    identity = nc.dram_tensor("identity", [P, P], bf16, kind="ExternalInput").ap()
    out_xh = nc.dram_tensor("out_xh", [P, 2, J], f32, kind="ExternalOutput").ap()

    with tile.TileContext(nc) as tc:
        _emit(
            tc,
            nc,
            (tokens, embed, wih_t, wih_s, whh_t, biases, biases_mm, bias1,
             onehot3, ones, identity, out_xh),
            n_leaves_core,
        )
    nc.compile()
    _PROG_CACHE[n_leaves_core] = nc
    return nc


def _retile_weights(w):
    # w: [1152, 384] -> lhsT tiles [128(k_part), 3(k), 9(m), 128(m_col)] bf16
    wt = np.ascontiguousarray(w.T)  # [384, 1152]
    wt = wt.reshape(J, P, 9, P).transpose(1, 0, 2, 3)
    return np.ascontiguousarray(wt).astype(BF16)


def _prep_bias(b_ih, b_hh):
    biases = np.zeros((P, 12), np.float32)
    comb = (b_ih + b_hh).reshape(9, P)
    biases[:, 0:6] = comb[0:6].T
    biases[:, 6:9] = b_hh.reshape(9, P)[6:9].T
    biases[:, 9:12] = b_ih.reshape(9, P)[6:9].T
    return biases


def _prep_bias_mm(b_ih, b_hh):
    # lhsT[k, ro, q] = bias[q, 3*ro + k]: the K=3 bias matmul against the
    # one-hot rhs yields out[q, (j, n)] = bias[q, 3*ro + j].
    b = _prep_bias(b_ih, b_hh)  # [128, 12] cols: r0..2 z0..2 hn0..2 in0..2
    out = b.T.reshape(4, 3, P).transpose(1, 0, 2)
    return np.ascontiguousarray(out).astype(BF16)


def _prep_bias1(b_ih, b_hh):
    # K=1 lhsT for the gi-precompute bias: out[col, :] += bias1[0, m, col].
    # r/z rows carry the combined input+hidden bias; n rows carry b_in only.
    out = np.zeros((1, 9, P), np.float32)
    comb = (b_ih + b_hh).reshape(9, P)
    out[0, 0:6] = comb[0:6]
    out[0, 6:9] = b_ih.reshape(9, P)[6:9]
    return out.astype(BF16)


def _prep_onehot3():
    out = np.zeros((3, 3, 512), np.float32)
    for k in range(3):
        out[k, k, :] = 1.0
    return out.astype(BF16)


def _gru_gates(x_t, h, w_ih, w_hh, b_ih, b_hh):
    gi = x_t @ w_ih.T + b_ih
    gh = h @ w_hh.T + b_hh
    i_r, i_z, i_n = np.split(gi, 3, axis=-1)
    h_r, h_z, h_n = np.split(gh, 3, axis=-1)
    r = 1.0 / (1.0 + np.exp(-(i_r + h_r)))
    z = 1.0 / (1.0 + np.exp(-(i_z + h_z)))
    n = np.tanh(i_n + r * h_n)
    return (1.0 - z) * n + z * h


def _root_gru(x_children, h0, w_ih, w_hh, b_ih, b_hh):
    h = h0.astype(np.float64)
    acc = np.zeros_like(h)
    for t in range(ARITY):
        x_t = x_children[ARITY - 1 - t].astype(np.float64)
        h = _gru_gates(x_t, h, w_ih.astype(np.float64), w_hh.astype(np.float64),
                       b_ih.astype(np.float64), b_hh.astype(np.float64))
        acc += h
    return (acc / ARITY).astype(np.float32)


def kernel(leaf_tokens, embed_table, w_ih, w_hh, b_ih, b_hh):
    from concourse.bass_utils import run_bass_kernel_spmd

    leaf_tokens = np.asarray(leaf_tokens, np.int32)
    embed_table = np.asarray(embed_table, np.float32)
    w_ih = np.asarray(w_ih, np.float32)
    w_hh = np.asarray(w_hh, np.float32)
    b_ih = np.asarray(b_ih, np.float32)
    b_hh = np.asarray(b_hh, np.float32)

    nc = _build_program(LEAVES_CORE)

    embed_bf = embed_table.astype(BF16)
    wih_t = _retile_weights(w_ih)
    wih_s = _retile_weights(w_ih / ARITY)
    whh_t = _retile_weights(w_hh)
    biases = _prep_bias(b_ih, b_hh)
    biases_mm = _prep_bias_mm(b_ih, b_hh)
    bias1 = _prep_bias1(b_ih, b_hh)
    ones = np.ones((1, 512), np.float32).astype(BF16)
    in_maps = []
    for core in range(NCORES):
        in_maps.append(
            {
                "tokens": np.ascontiguousarray(
                    leaf_tokens[core * LEAVES_CORE : (core + 1) * LEAVES_CORE]
                ),
                "embed": embed_bf,
                "wih_t": wih_t,
                "wih_s": wih_s,
                "whh_t": whh_t,
                "biases": biases,
                "biases_mm": biases_mm,
                "bias1": bias1,
                "onehot3": _prep_onehot3(),
                "ones": ones,
                "identity": np.eye(P, dtype=np.float32).astype(BF16),
            }
        )
    res = run_bass_kernel_spmd(nc, in_maps, core_ids=list(range(NCORES)))

    xs = np.zeros((NCORES, DIM), np.float32)
    h8 = np.zeros((NCORES, DIM), np.float32)
    for core in range(NCORES):
        out = res.results[core]["out_xh"]  # [P, 2, J]
        xs[core] = out[:, 0].T.reshape(-1)
        h8[core] = out[:, 1].T.reshape(-1)

    h0 = h8.mean(axis=0)
    out = _root_gru(xs, h0, w_ih, w_hh, b_ih, b_hh)
    return out.reshape(1, 1, DIM)


# revision 25
# speedup vs baseline: 1.1025x; 1.1025x over previous
"""Tree-GRU (arity-8, depth-5) over embedded leaves on 8 TRN2 NeuronCores.

Sharding: data-parallel over subtrees. Each core takes 4096 contiguous leaves
and runs levels 5..2 of the tree locally (512 -> 64 -> 8 -> 1 parents). The
root (level 1, 8 children = the 8 cores' level-2 outputs) is a trivial
16-matvec GRU done on host after gathering the per-core [384] outputs.

Device layout is feature-transposed: tensors live as [128 part, 3 ktile, ...]
with feature f = 128*k + p, so the GRU matmuls contract the partition dim.
Node storage is flat leaf-order (child-fastest), so all elementwise state
updates and the level-boundary x_next writes are contiguous; only the matmul
rhs / gi reads use stride-8 child slices. Weights are host-pre-transposed
into lhsT tiles; matmul dtype bf16 with fp32 PSUM accumulation.

Leaf level: embedding gather (indirect DMA, bf16 table, 4 SWDGE queues)
feeds PE transposes; step 0 (h=0, gi-only) runs in 4 sub-chunks of 128
parents that track gather completion; steps 1-7 ping-pong 2 chunks of 256.

Small levels (64/8/1 parents): the input transform gi for the whole level
(all 8 children x all parents) is precomputed in one batched matmul pass
(biases folded in via a K=1 ones-column matmul) and stored in SBUF, so the
sequential per-step work is only the recurrent matmul + gate chain. Per step
the r/z gi rows are injected into PSUM via an identity matmul (one start=True
covering MM per bank), the hn bias via a K=3 one-hot matmul, and the h update
uses h' = zc*n + z*h with zc = sigmoid(-pre_z) so z*h is computed off-path
(gpsimd) while tanh runs. The 1/8 output-mean scale is folded into a
pre-scaled copy of W_ih used by the gi passes; per-step output sums are
accumulated (hacc) and the final step writes the next level's input directly
as a fused raw-sum add.
"""

import numpy as np
import ml_dtypes

ARITY = 8
DIM = 384
VOCAB = 32000
NCORES = 8
P = 128
J = 3  # DIM // 128 feature tiles
N_LEAVES = 32768
LEAVES_CORE = N_LEAVES // NCORES  # 4096

BF16 = ml_dtypes.bfloat16

_PROG_CACHE = {}


def _levels_for(n_leaves_core):
    levels = []
    p = n_leaves_core // ARITY
    while p >= 1:
        levels.append(p)
        p //= ARITY
    assert levels[-1] == 1
    return levels


def _emit(tc, nc, aps, n_leaves_core):
    import concourse.mybir as mybir
    import concourse.bass as bass

    f32 = mybir.dt.float32
    bf16 = mybir.dt.bfloat16
    Sig = mybir.ActivationFunctionType.Sigmoid
    Tanh = mybir.ActivationFunctionType.Tanh
    Add = mybir.AluOpType.add
    Sub = mybir.AluOpType.subtract
    Mult = mybir.AluOpType.mult

    (tokens, embed, wih_t, wih_s, whh_t, biases, biases_mm, bias1, onehot3,
     ones, identity, out_xh) = aps
    levels = _levels_for(n_leaves_core)
    P5 = levels[0]
    n_gtiles = n_leaves_core // P  # 32

    from contextlib import ExitStack

    with ExitStack() as ctx:
        const = ctx.enter_context(tc.tile_pool(name="const", bufs=1))
        xpool = ctx.enter_context(tc.tile_pool(name="xpool", bufs=1))
        state = ctx.enter_context(tc.tile_pool(name="state", bufs=1))
        gates = ctx.enter_context(tc.tile_pool(name="gates", bufs=6))
        gpool = ctx.enter_context(tc.tile_pool(name="gpool", bufs=1))
        pspool = ctx.enter_context(tc.tile_pool(name="pspool", bufs=8, space="PSUM"))

        # ---- tokens first, then kick off all gathers (2 SWDGE queues).
        # The gathers are paced by gpsimd descriptor generation (~1.1us per
        # 128 rows), so nothing else may sit ahead of them in the gpsimd
        # FIFO -- the identity build comes after.
        tok_sb = const.tile([P, n_gtiles], mybir.dt.int32)
        nc.sync.dma_start(tok_sb[:], tokens.rearrange("(g p) -> p g", p=P))

        xgs = []
        for g in range(n_gtiles):
            xg = gpool.tile([P, DIM], bf16, name="xg", tag="xg", bufs=n_gtiles)
            gi_inst = nc.gpsimd.indirect_dma_start(
                out=xg[:],
                out_offset=None,
                in_=embed[:],
                in_offset=bass.IndirectOffsetOnAxis(ap=tok_sb[:, g : g + 1], axis=0),
            )
            if g % 2 == 1:
                gi_inst.ins.queue = "qPoolDynamic1"
            xgs.append(xg)

        # identity comes in via DMA so no gpsimd work sits ahead of the
        # gather descriptor generation
        ident = const.tile([P, P], bf16)
        nc.sync.dma_start(ident[:], identity[:])

        # ---- constants / weights ----
        wih_sb = const.tile([P, J, 9, P], bf16)
        wih_s_sb = const.tile([P, J, 9, P], bf16)
        whh_sb = const.tile([P, J, 9, P], bf16)
        bias_sb = const.tile([P, 12], f32)
        bias3_sb = const.tile([3, 4, P], bf16)
        bias1_sb = const.tile([1, 9, P], bf16)
        onehot3_sb = const.tile([3, 3, 512], bf16)
        ones_sb = const.tile([1, 512], bf16)
        nc.sync.dma_start(wih_sb[:], wih_t[:])
        nc.sync.dma_start(wih_s_sb[:], wih_s[:])
        nc.sync.dma_start(whh_sb[:], whh_t[:])
        nc.sync.dma_start(bias_sb[:], biases[:])
        nc.sync.dma_start(bias3_sb[:], biases_mm[:])
        nc.sync.dma_start(bias1_sb[:], bias1[:])
        nc.sync.dma_start(onehot3_sb[:], onehot3[:])
        nc.sync.dma_start(ones_sb[:], ones[:])

        # child-major x per level: [P, J, 8, Pl] (contiguous matmul rhs)
        x_in = {}
        for Pl in levels:
            x_in[Pl] = xpool.tile([P, J, ARITY, Pl], bf16, name=f"x{Pl}", tag=f"x{Pl}")

        def psum_tile():
            return pspool.tile([P, 512], f32, name="ps", tag="ps")

        def warm(n):
            # PE warm-keeper: HAM re-throttles the PE clock to 1.2 GHz after
            # ~3.4us of idle; during the latency-bound small levels, issue
            # dependency-free matmuls so the gate-chain windows don't cool
            # the PE and the next real matmul burst runs at 2.4 GHz.
            for _ in range(n):
                wp = pspool.tile([P, 512], f32, name="warm", tag="ps")
                nc.tensor.matmul(
                    wp[:, :512], ident[:], wih_sb[:, 0, 0:4, :], start=True, stop=True
                )

        def new_state(name, dtype, Pl):
            return state.tile([P, J, Pl], dtype, name=f"{name}{Pl}", tag=f"{name}{Pl}")

        # =====================  LEAF LEVEL (Pl = P5)  =====================
        x5 = x_in[P5]
        h = new_state("h", bf16, P5)
        # permuted (child-major) accumulator so the level-end x_next add is a
        # contiguous write into the next level's child-major x
        hacc = state.tile([P, J, ARITY, P5 // ARITY], f32, name="hacc5", tag="hacc5")

        def emit_transposes(g0, g1):
            for g in range(g0, g1):
                for j in range(J):
                    tp = pspool.tile([P, 512], bf16, name="tp", tag="ps")
                    nc.tensor.transpose(
                        tp[:, :P], xgs[g][:, j * P : (j + 1) * P], ident[:]
                    )
                    nc.vector.tensor_copy(
                        out=x5[:, j, :, 16 * g : 16 * (g + 1)],
                        in_=tp[:, :P].rearrange("p (par c) -> p c par", c=ARITY),
                    )

        level_csum = [None]

        with nc.named_scope("leaf_t0"):
            # step 0: h=0, gi only; 4 sub-chunks of 128 parents (8 gtiles
            # each) that track gather completion. Biases are injected into
            # PSUM via the K=3 one-hot matmul (the single start=True per
            # bank), so the activations span all 3 m-tiles in one op.
            NSC = P5 // 4  # 128 parents per sub-chunk
            gsc = n_gtiles // 4
            c0 = ARITY - 1  # first GRU input is the last child
            for sc in range(4):
                emit_transposes(sc * gsc, (sc + 1) * gsc)
                sl = slice(sc * NSC, (sc + 1) * NSC)
                N3 = 3 * NSC
                ps_r, ps_z, ps_in = psum_tile(), psum_tile(), psum_tile()

                def view3s(pst):
                    return pst[:, :N3].rearrange("p (j n) -> p j n", j=3)

                for pst, ro, moff in ((ps_r, 0, 0), (ps_z, 1, 3), (ps_in, 3, 6)):
                    nc.tensor.matmul(
                        pst[:, :N3], bias3_sb[:, ro, :], onehot3_sb[:, :, :NSC],
                        start=True, stop=False,
                    )
                    for m in range(3):
                        for k in range(J):
                            nc.tensor.matmul(
                                pst[:, (m * NSC) : (m + 1) * NSC],
                                wih_sb[:, k, moff + m, :],
                                x5[:, k, c0, sl],
                                start=False,
                                stop=(m == 2 and k == 2),
                            )
                r_sb = gates.tile([P, J, NSC], bf16, name="r0", tag="r0")
                z_sb = gates.tile([P, J, NSC], bf16, name="z0", tag="z0")
                n_sb = gates.tile([P, J, NSC], bf16, name="n0", tag="n0")
                rhn = gates.tile([P, J, NSC], f32, name="rhn0", tag="rhn0")
                t1 = gates.tile([P, J, NSC], bf16, name="t10", tag="t10")
                nc.scalar.activation(r_sb[:], view3s(ps_r), Sig)
                nc.scalar.activation(z_sb[:], view3s(ps_z), Sig)
                # n = tanh(i_n + b_in + r*b_hn): gh_n of the zero state is
                # just b_hn, folded in per m via the scalar port
                for m in range(3):
                    nc.vector.scalar_tensor_tensor(
                        out=rhn[:, m],
                        in0=r_sb[:, m],
                        scalar=bias_sb[:, 6 + m : 7 + m],
                        in1=view3s(ps_in)[:, m],
                        op0=Mult,
                        op1=Add,
                    )
                nc.scalar.activation(n_sb[:], rhn[:], Tanh)
                # h0=0: h' = n - z*n
                hsl = h[:, :, sl]
                nc.vector.tensor_tensor(out=t1[:], in0=z_sb[:], in1=n_sb[:], op=Mult)
                nc.vector.tensor_tensor(out=hsl, in0=n_sb[:], in1=t1[:], op=Sub)
                nc.gpsimd.tensor_copy(
                    out=hacc[:, :, :, sc * (NSC // ARITY) : (sc + 1) * (NSC // ARITY)],
                    in_=hsl.rearrange("p j (q c) -> p j c q", c=ARITY),
                )

        NCH = 256
        nch = P5 // NCH
        # Skewed emission: chunk A's steps run while chunk B's gathers and
        # step-0 sub-chunks are still completing (the engine FIFOs are
        # in-order, so chunk B work must not be queued until its data is
        # nearly ready). B1 sits after A5.
        SKEW = 5
        step_seq = []
        for t in range(1, ARITY):
            step_seq.append((t, 0))
            if t >= SKEW + 1:
                step_seq.append((t - SKEW, 1))
        for t in range(ARITY - SKEW, ARITY):
            step_seq.append((t, 1))
        for t, ch in step_seq:
            c = ARITY - 1 - t
            with nc.named_scope(f"leaf_t{t}c{ch}"):
                if True:
                    sl = slice(ch * NCH, (ch + 1) * NCH)
                    ps_r = [psum_tile()[:, :NCH] for _ in range(3)]
                    ps_z = [psum_tile()[:, :NCH] for _ in range(3)]
                    ps_in = [psum_tile()[:, :NCH] for _ in range(3)]
                    ps_hn = [psum_tile()[:, :NCH] for _ in range(3)]
                    for ps, moff in ((ps_r, 0), (ps_z, 3), (ps_in, 6)):
                        for m in range(3):
                            for k in range(J):
                                nc.tensor.matmul(
                                    ps[m],
                                    wih_sb[:, k, moff + m, :],
                                    x5[:, k, c, sl],
                                    start=(k == 0),
                                    stop=(k == 2 and moff == 6),
                                )
                    for ps, moff in ((ps_r, 0), (ps_z, 3), (ps_hn, 6)):
                        for m in range(3):
                            for k in range(J):
                                nc.tensor.matmul(
                                    ps[m],
                                    whh_sb[:, k, moff + m, :],
                                    h[:, k, sl],
                                    start=(k == 0 and moff == 6),
                                    stop=(k == 2),
                                )

                    r_sb = gates.tile([P, J, NCH], bf16, name="r_sb", tag="r_sb")
                    z_sb = gates.tile([P, J, NCH], bf16, name="z_sb", tag="z_sb")
                    n_sb = gates.tile([P, J, NCH], bf16, name="n_sb", tag="n_sb")
                    rhn = gates.tile([P, J, NCH], f32, name="rhn", tag="rhn")
                    t1 = gates.tile([P, J, NCH], bf16, name="t1", tag="t1")

                    for m in range(3):
                        nc.scalar.activation(
                            r_sb[:, m], ps_r[m], Sig, bias=bias_sb[:, m : m + 1]
                        )
                    for m in range(3):
                        nc.scalar.activation(
                            z_sb[:, m], ps_z[m], Sig, bias=bias_sb[:, 3 + m : 4 + m]
                        )
                    for m in range(3):
                        nc.vector.scalar_tensor_tensor(
                            out=rhn[:, m],
                            in0=ps_hn[m],
                            scalar=bias_sb[:, 6 + m : 7 + m],
                            in1=r_sb[:, m],
                            op0=Add,
                            op1=Mult,
                        )
                    for m in range(3):
                        nc.vector.tensor_tensor(
                            out=rhn[:, m], in0=rhn[:, m], in1=ps_in[m], op=Add
                        )
                    for m in range(3):
                        nc.scalar.activation(
                            n_sb[:, m], rhn[:, m], Tanh, bias=bias_sb[:, 9 + m : 10 + m]
                        )

                    # h' = n + z*(h - n)
                    hsl = h[:, :, sl]
                    nc.vector.tensor_tensor(out=t1[:], in0=hsl, in1=n_sb[:], op=Sub)
                    nc.vector.tensor_tensor(out=t1[:], in0=z_sb[:], in1=t1[:], op=Mult)
                    nc.vector.tensor_tensor(out=hsl, in0=n_sb[:], in1=t1[:], op=Add)
                    hperm = hsl.rearrange("p j (q c) -> p j c q", c=ARITY)
                    qsl = slice(ch * NCH // ARITY, (ch + 1) * NCH // ARITY)
                    if t == ARITY - 1:
                        if ch == 0:
                            csum = state.tile(
                                [P, J, P5 // ARITY], f32, name="csum5", tag="csum5"
                            )
                            level_csum[0] = csum
                        nc.vector.tensor_reduce(
                            out=level_csum[0][:, :, qsl],
                            in_=hsl.rearrange("p j (q c) -> p j q c", c=ARITY),
                            axis=mybir.AxisListType.X,
                            op=Add,
                        )
                        xn = x_in[P5 // ARITY]
                        for j in range(J):
                            eng = nc.gpsimd if j == 2 else nc.vector
                            eng.tensor_tensor(
                                out=xn[:, j, :, qsl],
                                in0=hacc[:, j, :, qsl],
                                in1=hperm[:, j],
                                op=Add,
                            )
                        # bridge the level-end tail so the PE stays warm into
                        # the gi_64 pass
                        warm(8)
                    else:
                        nc.gpsimd.tensor_tensor(
                            out=hacc[:, :, :, qsl],
                            in0=hacc[:, :, :, qsl],
                            in1=hperm,
                            op=Add,
                        )

        # =====================  SMALL LEVELS (64, 8, 1)  ==================
        for Pl in levels[1:]:
            NC8 = ARITY * Pl  # children count = gi batch size
            with nc.named_scope(f"gi_{Pl}"):
                # gi stored child-major [P, 9, 8, Pl]: the gi pass rhs is the
                # child-major x (contiguous), so PSUM comes out (c, q)-ordered
                gi_sb = xpool.tile([P, 9, ARITY, Pl], bf16, name=f"gi{Pl}",
                                   tag=f"gi{Pl}")
                # m-order: r (0,1,2) first so step 0's r-inject unblocks early,
                # then z (3,4,5), then n (6,7,8)
                for mi, m in enumerate((0, 1, 2, 3, 4, 5, 6, 7, 8)):
                    ps = psum_tile()[:, :NC8]
                    nc.tensor.matmul(
                        ps, bias1_sb[:, m, :], ones_sb[:, :NC8],
                        start=True, stop=False,
                    )
                    for k in range(J):
                        nc.tensor.matmul(
                            ps,
                            wih_s_sb[:, k, m, :],
                            x_in[Pl][:, k, :, :],
                            start=False,
                            stop=(k == 2),
                        )
                    # alternate copy engine so the PSUM->SBUF drain keeps up
                    # with the matmul waves
                    if mi % 2 == 0:
                        nc.vector.tensor_copy(
                            out=gi_sb[:, m].rearrange("p c q -> p (c q)"), in_=ps
                        )
                    else:
                        nc.scalar.copy(
                            out=gi_sb[:, m].rearrange("p c q -> p (c q)"), in_=ps
                        )
                    if Pl == 64:
                        warm(1)

            csum = level_csum[0]
            h = new_state("h", bf16, Pl)
            hacc = new_state("hacc", f32, Pl)
            nc.scalar.mul(h[:], csum[:], 1.0 / ARITY)

            for t in range(ARITY):
                c = ARITY - 1 - t
                with nc.named_scope(f"lv{Pl}_t{t}"):
                    N3 = 3 * Pl
                    ps_z, ps_r, ps_hn = psum_tile(), psum_tile(), psum_tile()

                    def view3(pst):
                        return pst[:, :N3].rearrange("p (j n) -> p j n", j=3)

                    def msl(pst, m):
                        return pst[:, m * Pl : (m + 1) * Pl]

                    # r first: sigma(r) heads the serial chain, so its PSUM
                    # group must close first and nothing may sit ahead of
                    # sigma(r) in the scalar FIFO
                    nc.tensor.matmul(
                        ps_r[:, :N3], ident[:], gi_sb[:, 0:3, c, :],
                        start=True, stop=False,
                    )
                    for m in range(3):
                        for k in range(J):
                            nc.tensor.matmul(
                                msl(ps_r, m), whh_sb[:, k, m, :], h[:, k, :],
                                start=False, stop=(m == 2 and k == 2),
                            )
                    # hn second so its accumulation closes before z's: the
                    # serial path is sigma(r) -> rhn = ps_hn*r -> tanh
                    nc.tensor.matmul(
                        ps_hn[:, :N3], bias3_sb[:, 2, :], onehot3_sb[:, :, :Pl],
                        start=True, stop=False,
                    )
                    for m in range(3):
                        for k in range(J):
                            nc.tensor.matmul(
                                msl(ps_hn, m), whh_sb[:, k, 6 + m, :], h[:, k, :],
                                start=False, stop=(m == 2 and k == 2),
                            )
                    # z last (sigma(z)/zc/t2 have slack until the h update)
                    nc.tensor.matmul(
                        ps_z[:, :N3], ident[:], gi_sb[:, 3:6, c, :],
                        start=True, stop=False,
                    )
                    for m in range(3):
                        for k in range(J):
                            nc.tensor.matmul(
                                msl(ps_z, m), whh_sb[:, k, 3 + m, :], h[:, k, :],
                                start=False, stop=(m == 2 and k == 2),
                            )
                    warm(6 if Pl == 64 else 4)

                    z_sb = gates.tile([P, J, Pl], bf16, name="z_sb", tag="z_sb")
                    zc_sb = gates.tile([P, J, Pl], bf16, name="zc_sb", tag="zc_sb")
                    r_sb = gates.tile([P, J, Pl], bf16, name="r_sb", tag="r_sb")
                    n_sb = gates.tile([P, J, Pl], bf16, name="n_sb", tag="n_sb")
                    rhn = gates.tile([P, J, Pl], f32, name="rhn", tag="rhn")
                    t1 = gates.tile([P, J, Pl], f32, name="t1", tag="t1")
                    t2 = gates.tile([P, J, Pl], f32, name="t2", tag="t2")

                    nc.scalar.activation(r_sb[:], view3(ps_r), Sig)
                    nc.scalar.activation(z_sb[:], view3(ps_z), Sig)
                    nc.scalar.activation(zc_sb[:], view3(ps_z), Sig, scale=-1.0)
                    # t2 = z*h off-path while r/n compute
                    nc.gpsimd.tensor_tensor(out=t2[:], in0=z_sb[:], in1=h[:], op=Mult)
                    nc.vector.tensor_tensor(
                        out=rhn[:], in0=view3(ps_hn), in1=r_sb[:], op=Mult
                    )
                    nc.vector.tensor_tensor(
                        out=rhn[:], in0=rhn[:], in1=gi_sb[:, 6:9, c, :], op=Add
                    )
                    nc.scalar.activation(n_sb[:], rhn[:], Tanh)
                    # h' = zc*n + z*h
                    nc.vector.tensor_tensor(out=t1[:], in0=zc_sb[:], in1=n_sb[:], op=Mult)
                    nc.vector.tensor_tensor(out=h[:], in0=t1[:], in1=t2[:], op=Add)

                    if t == 0:
                        nc.gpsimd.tensor_copy(out=hacc[:], in_=h[:])
                    elif t == ARITY - 1 and Pl > 1:
                        csum = state.tile(
                            [P, J, Pl // ARITY], f32, name=f"csum{Pl}", tag=f"csum{Pl}"
                        )
                        level_csum[0] = csum
                        nc.vector.tensor_reduce(
                            out=csum[:],
                            in_=h[:].rearrange("p j (q c) -> p j q c", c=ARITY),
                            axis=mybir.AxisListType.X,
                            op=Add,
                        )
                        xn = x_in[Pl // ARITY]
                        nc.vector.tensor_tensor(
                            out=xn[:],
                            in0=hacc[:].rearrange("p j (q c) -> p j c q", c=ARITY),
                            in1=h[:].rearrange("p j (q c) -> p j c q", c=ARITY),
                            op=Add,
                        )
                    else:
                        nc.gpsimd.tensor_tensor(
                            out=hacc[:], in0=hacc[:], in1=h[:], op=Add
                        )

        # ---- outputs: [P, 2, J] = (x_root, h_root) ----
        out_t = state.tile([P, 2, J], f32, name="out_t", tag="out_t")
        nc.scalar.mul(out_t[:, 0], hacc[:, :, 0], 1.0 / ARITY)
        nc.vector.tensor_copy(out=out_t[:, 1], in_=h[:, :, 0])
        nc.sync.dma_start(out_xh[:], out_t[:])


def _build_program(n_leaves_core):
    if n_leaves_core in _PROG_CACHE:
        return _PROG_CACHE[n_leaves_core]
    import concourse.bacc as bacc
    import concourse.mybir as mybir
    import concourse.tile as tile

    f32 = mybir.dt.float32
    bf16 = mybir.dt.bfloat16

    nc = bacc.Bacc(
        "TRN2",
        target_bir_lowering=False,
        debug=False,
        enable_asserts=False,
        num_devices=NCORES,
        num_swdge_queues=4,
    )
    tokens = nc.dram_tensor("tokens", [n_leaves_core], mybir.dt.int32, kind="ExternalInput").ap()
    embed = nc.dram_tensor("embed", [VOCAB, DIM], bf16, kind="ExternalInput").ap()
    wih_t = nc.dram_tensor("wih_t", [P, J, 9, P], bf16, kind="ExternalInput").ap()
    wih_s = nc.dram_tensor("wih_s", [P, J, 9, P], bf16, kind="ExternalInput").ap()
    whh_t = nc.dram_tensor("whh_t", [P, J, 9, P], bf16, kind="ExternalInput").ap()
    biases = nc.dram_tensor("biases", [P, 12], f32, kind="ExternalInput").ap()
    biases_mm = nc.dram_tensor("biases_mm", [3, 4, P], bf16, kind="ExternalInput").ap()
    bias1 = nc.dram_tensor("bias1", [1, 9, P], bf16, kind="ExternalInput").ap()
    onehot3 = nc.dram_tensor("onehot3", [3, 3, 512], bf16, kind="ExternalInput").ap()
    ones = nc.dram_tensor("ones", [1, 512], bf16, kind="ExternalInput").ap()
    identity = nc.dram_tensor("identity", [P, P], bf16, kind="ExternalInput").ap()
    out_xh = nc.dram_tensor("out_xh", [P, 2, J], f32, kind="ExternalOutput").ap()

    with tile.TileContext(nc) as tc:
        _emit(
            tc,
            nc,
            (tokens, embed, wih_t, wih_s, whh_t, biases, biases_mm, bias1,
             onehot3, ones, identity, out_xh),
            n_leaves_core,
        )
    nc.compile()
    _PROG_CACHE[n_leaves_core] = nc
    return nc


def _retile_weights(w):
    # w: [1152, 384] -> lhsT tiles [128(k_part), 3(k), 9(m), 128(m_col)] bf16
    wt = np.ascontiguousarray(w.T)  # [384, 1152]
    wt = wt.reshape(J, P, 9, P).transpose(1, 0, 2, 3)
    return np.ascontiguousarray(wt).astype(BF16)


def _prep_bias(b_ih, b_hh):
    biases = np.zeros((P, 12), np.float32)
    comb = (b_ih + b_hh).reshape(9, P)
    biases[:, 0:6] = comb[0:6].T
    biases[:, 6:9] = b_hh.reshape(9, P)[6:9].T
    biases[:, 9:12] = b_ih.reshape(9, P)[6:9].T
    return biases


def _prep_bias_mm(b_ih, b_hh):
    # lhsT[k, ro, q] = bias[q, 3*ro + k]: the K=3 bias matmul against the
    # one-hot rhs yields out[q, (j, n)] = bias[q, 3*ro + j].
    b = _prep_bias(b_ih, b_hh)  # [128, 12] cols: r0..2 z0..2 hn0..2 in0..2
    out = b.T.reshape(4, 3, P).transpose(1, 0, 2)
    return np.ascontiguousarray(out).astype(BF16)


def _prep_bias1(b_ih, b_hh):
    # K=1 lhsT for the gi-precompute bias: out[col, :] += bias1[0, m, col].
    # r/z rows carry the combined input+hidden bias; n rows carry b_in only.
    out = np.zeros((1, 9, P), np.float32)
    comb = (b_ih + b_hh).reshape(9, P)
    out[0, 0:6] = comb[0:6]
    out[0, 6:9] = b_ih.reshape(9, P)[6:9]
    return out.astype(BF16)


def _prep_onehot3():
    out = np.zeros((3, 3, 512), np.float32)
    for k in range(3):
        out[k, k, :] = 1.0
    return out.astype(BF16)


def _gru_gates(x_t, h, w_ih, w_hh, b_ih, b_hh):
    gi = x_t @ w_ih.T + b_ih
    gh = h @ w_hh.T + b_hh
    i_r, i_z, i_n = np.split(gi, 3, axis=-1)
    h_r, h_z, h_n = np.split(gh, 3, axis=-1)
    r = 1.0 / (1.0 + np.exp(-(i_r + h_r)))
    z = 1.0 / (1.0 + np.exp(-(i_z + h_z)))
    n = np.tanh(i_n + r * h_n)
    return (1.0 - z) * n + z * h


def _root_gru(x_children, h0, w_ih, w_hh, b_ih, b_hh):
    h = h0.astype(np.float64)
    acc = np.zeros_like(h)
    for t in range(ARITY):
        x_t = x_children[ARITY - 1 - t].astype(np.float64)
        h = _gru_gates(x_t, h, w_ih.astype(np.float64), w_hh.astype(np.float64),
                       b_ih.astype(np.float64), b_hh.astype(np.float64))
        acc += h
    return (acc / ARITY).astype(np.float32)


def kernel(leaf_tokens, embed_table, w_ih, w_hh, b_ih, b_hh):
    from concourse.bass_utils import run_bass_kernel_spmd

    leaf_tokens = np.asarray(leaf_tokens, np.int32)
    embed_table = np.asarray(embed_table, np.float32)
    w_ih = np.asarray(w_ih, np.float32)
    w_hh = np.asarray(w_hh, np.float32)
    b_ih = np.asarray(b_ih, np.float32)
    b_hh = np.asarray(b_hh, np.float32)

    nc = _build_program(LEAVES_CORE)

    embed_bf = embed_table.astype(BF16)
    wih_t = _retile_weights(w_ih)
    wih_s = _retile_weights(w_ih / ARITY)
    whh_t = _retile_weights(w_hh)
    biases = _prep_bias(b_ih, b_hh)
    biases_mm = _prep_bias_mm(b_ih, b_hh)
    bias1 = _prep_bias1(b_ih, b_hh)
    ones = np.ones((1, 512), np.float32).astype(BF16)
    in_maps = []
    for core in range(NCORES):
        in_maps.append(
            {
                "tokens": np.ascontiguousarray(
                    leaf_tokens[core * LEAVES_CORE : (core + 1) * LEAVES_CORE]
                ),
                "embed": embed_bf,
                "wih_t": wih_t,
                "wih_s": wih_s,
                "whh_t": whh_t,
                "biases": biases,
                "biases_mm": biases_mm,
                "bias1": bias1,
                "onehot3": _prep_onehot3(),
                "ones": ones,
                "identity": np.eye(P, dtype=np.float32).astype(BF16),
            }
        )
    res = run_bass_kernel_spmd(nc, in_maps, core_ids=list(range(NCORES)))

    xs = np.zeros((NCORES, DIM), np.float32)
    h8 = np.zeros((NCORES, DIM), np.float32)
    for core in range(NCORES):
        out = res.results[core]["out_xh"]  # [P, 2, J]
        xs[core] = out[:, 0].T.reshape(-1)
        h8[core] = out[:, 1].T.reshape(-1)

    h0 = h8.mean(axis=0)
    out = _root_gru(xs, h0, w_ih, w_hh, b_ih, b_hh)
    return out.reshape(1, 1, DIM)


# revision 30
# speedup vs baseline: 1.1053x; 1.0025x over previous
"""Tree-GRU (arity-8, depth-5) over embedded leaves on 8 TRN2 NeuronCores.

Sharding: data-parallel over subtrees. Each core takes 4096 contiguous leaves
and runs levels 5..2 of the tree locally (512 -> 64 -> 8 -> 1 parents). The
root (level 1, 8 children = the 8 cores' level-2 outputs) is a trivial
16-matvec GRU done on host after gathering the per-core [384] outputs.

Device layout is feature-transposed: tensors live as [128 part, 3 ktile, ...]
with feature f = 128*k + p, so the GRU matmuls contract the partition dim.
Node storage is flat leaf-order (child-fastest), so all elementwise state
updates and the level-boundary x_next writes are contiguous; only the matmul
rhs / gi reads use stride-8 child slices. Weights are host-pre-transposed
into lhsT tiles; matmul dtype bf16 with fp32 PSUM accumulation.

Leaf level: embedding gather (indirect DMA, bf16 table, 4 SWDGE queues)
feeds PE transposes; step 0 (h=0, gi-only) runs in 4 sub-chunks of 128
parents that track gather completion; steps 1-7 ping-pong 2 chunks of 256.

Small levels (64/8/1 parents): the input transform gi for the whole level
(all 8 children x all parents) is precomputed in one batched matmul pass
(biases folded in via a K=1 ones-column matmul) and stored in SBUF, so the
sequential per-step work is only the recurrent matmul + gate chain. Per step
the r/z gi rows are injected into PSUM via an identity matmul (one start=True
covering MM per bank), the hn bias via a K=3 one-hot matmul, and the h update
uses h' = zc*n + z*h with zc = sigmoid(-pre_z) so z*h is computed off-path
(gpsimd) while tanh runs. The 1/8 output-mean scale is folded into a
pre-scaled copy of W_ih used by the gi passes; per-step output sums are
accumulated (hacc) and the final step writes the next level's input directly
as a fused raw-sum add.
"""

import numpy as np
import ml_dtypes

ARITY = 8
DIM = 384
VOCAB = 32000
NCORES = 8
P = 128
J = 3  # DIM // 128 feature tiles
N_LEAVES = 32768
LEAVES_CORE = N_LEAVES // NCORES  # 4096

BF16 = ml_dtypes.bfloat16

_PROG_CACHE = {}


def _levels_for(n_leaves_core):
    levels = []
    p = n_leaves_core // ARITY
    while p >= 1:
        levels.append(p)
        p //= ARITY
    assert levels[-1] == 1
    return levels


def _emit(tc, nc, aps, n_leaves_core):
    import concourse.mybir as mybir
    import concourse.bass as bass

    f32 = mybir.dt.float32
    bf16 = mybir.dt.bfloat16
    Sig = mybir.ActivationFunctionType.Sigmoid
    Tanh = mybir.ActivationFunctionType.Tanh
    Add = mybir.AluOpType.add
    Sub = mybir.AluOpType.subtract
    Mult = mybir.AluOpType.mult

    (tokens, embed, wih_t, wih_s, whh_t, biases, biases_mm, bias1, onehot3,
     ones, identity, out_xh) = aps
    levels = _levels_for(n_leaves_core)
    P5 = levels[0]
    n_gtiles = n_leaves_core // P  # 32

    from contextlib import ExitStack

    with ExitStack() as ctx:
        const = ctx.enter_context(tc.tile_pool(name="const", bufs=1))
        xpool = ctx.enter_context(tc.tile_pool(name="xpool", bufs=1))
        state = ctx.enter_context(tc.tile_pool(name="state", bufs=1))
        gates = ctx.enter_context(tc.tile_pool(name="gates", bufs=6))
        gpool = ctx.enter_context(tc.tile_pool(name="gpool", bufs=1))
        pspool = ctx.enter_context(tc.tile_pool(name="pspool", bufs=8, space="PSUM"))

        # ---- tokens first, then kick off all gathers (dma_gather ucode,
        # int16 idx lists, 512 rows per call; 2 SWDGE queues). The gathers
        # are paced by gpsimd descriptor generation, so nothing else may sit
        # ahead of them in the gpsimd FIFO -- the identity build comes after.
        from concourse.library_config import mlp as _mlp_lib

        GB = 512  # rows per dma_gather call
        n_gcalls = n_leaves_core // GB  # 8
        tok_sb = const.tile([P, n_gcalls * (GB // 16)], mybir.dt.int16)
        nc.sync.dma_start(tok_sb[:], tokens[:])
        nc.gpsimd.load_library(_mlp_lib)

        xgcalls = []
        for b in range(n_gcalls):
            xg4 = gpool.tile([P, GB // P, DIM], bf16, name="xg4", tag="xg4",
                             bufs=n_gcalls)
            nc.gpsimd.dma_gather(
                xg4[:],
                embed[:],
                tok_sb[:, b * (GB // 16) : (b + 1) * (GB // 16)],
                GB,
                GB,
                DIM,
                queue_num=(b % 2),
            )
            xgcalls.append(xg4)

        def xg_slice(g, j):
            # g-tile g (128 leaves) feature block j as [P, 128]
            return xgcalls[g // 4][:, g % 4, j * P : (j + 1) * P]

        # identity comes in via DMA so no gpsimd work sits ahead of the
        # gather descriptor generation
        ident = const.tile([P, P], bf16)
        nc.sync.dma_start(ident[:], identity[:])

        # ---- constants / weights ----
        wih_sb = const.tile([P, J, 9, P], bf16)
        wih_s_sb = const.tile([P, J, 9, P], bf16)
        whh_sb = const.tile([P, J, 9, P], bf16)
        bias_sb = const.tile([P, 12], f32)
        bias3_sb = const.tile([3, 4, P], bf16)
        bias1_sb = const.tile([1, 9, P], bf16)
        onehot3_sb = const.tile([3, 3, 512], bf16)
        ones_sb = const.tile([1, 512], bf16)
        nc.sync.dma_start(wih_sb[:], wih_t[:])
        nc.sync.dma_start(wih_s_sb[:], wih_s[:])
        nc.sync.dma_start(whh_sb[:], whh_t[:])
        nc.sync.dma_start(bias_sb[:], biases[:])
        nc.sync.dma_start(bias3_sb[:], biases_mm[:])
        nc.sync.dma_start(bias1_sb[:], bias1[:])
        nc.sync.dma_start(onehot3_sb[:], onehot3[:])
        nc.sync.dma_start(ones_sb[:], ones[:])

        # child-major x per level: [P, J, 8, Pl] (contiguous matmul rhs)
        x_in = {}
        for Pl in levels:
            x_in[Pl] = xpool.tile([P, J, ARITY, Pl], bf16, name=f"x{Pl}", tag=f"x{Pl}")

        def psum_tile():
            return pspool.tile([P, 512], f32, name="ps", tag="ps")

        def warm(n):
            # PE warm-keeper: HAM re-throttles the PE clock to 1.2 GHz after
            # ~3.4us of idle; during the latency-bound small levels, issue
            # dependency-free matmuls so the gate-chain windows don't cool
            # the PE and the next real matmul burst runs at 2.4 GHz.
            for _ in range(n):
                wp = pspool.tile([P, 512], f32, name="warm", tag="ps")
                nc.tensor.matmul(
                    wp[:, :512], ident[:], wih_sb[:, 0, 0:4, :], start=True, stop=True
                )

        def new_state(name, dtype, Pl):
            return state.tile([P, J, Pl], dtype, name=f"{name}{Pl}", tag=f"{name}{Pl}")

        # =====================  LEAF LEVEL (Pl = P5)  =====================
        x5 = x_in[P5]
        h = new_state("h", bf16, P5)
        # permuted (child-major) accumulator so the level-end x_next add is a
        # contiguous write into the next level's child-major x
        hacc = state.tile([P, J, ARITY, P5 // ARITY], f32, name="hacc5", tag="hacc5")

        def emit_transposes(g0, g1):
            for g in range(g0, g1):
                for j in range(J):
                    tp = pspool.tile([P, 512], bf16, name="tp", tag="ps")
                    nc.tensor.transpose(
                        tp[:, :P], xg_slice(g, j), ident[:]
                    )
                    nc.vector.tensor_copy(
                        out=x5[:, j, :, 16 * g : 16 * (g + 1)],
                        in_=tp[:, :P].rearrange("p (par c) -> p c par", c=ARITY),
                    )

        level_csum = [None]

        with nc.named_scope("leaf_t0"):
            # step 0: h=0, gi only; 4 sub-chunks of 128 parents (8 gtiles
            # each) that track gather completion. Biases are injected into
            # PSUM via the K=3 one-hot matmul (the single start=True per
            # bank), so the activations span all 3 m-tiles in one op.
            NSC = P5 // 4  # 128 parents per sub-chunk
            gsc = n_gtiles // 4
            c0 = ARITY - 1  # first GRU input is the last child
            for sc in range(4):
                emit_transposes(sc * gsc, (sc + 1) * gsc)
                sl = slice(sc * NSC, (sc + 1) * NSC)
                N3 = 3 * NSC
                ps_r, ps_z, ps_in = psum_tile(), psum_tile(), psum_tile()

                def view3s(pst):
                    return pst[:, :N3].rearrange("p (j n) -> p j n", j=3)

                for pst, ro, moff in ((ps_r, 0, 0), (ps_z, 1, 3), (ps_in, 3, 6)):
                    nc.tensor.matmul(
                        pst[:, :N3], bias3_sb[:, ro, :], onehot3_sb[:, :, :NSC],
                        start=True, stop=False,
                    )
                    for m in range(3):
                        for k in range(J):
                            nc.tensor.matmul(
                                pst[:, (m * NSC) : (m + 1) * NSC],
                                wih_sb[:, k, moff + m, :],
                                x5[:, k, c0, sl],
                                start=False,
                                stop=(m == 2 and k == 2),
                            )
                r_sb = gates.tile([P, J, NSC], bf16, name="r0", tag="r0")
                z_sb = gates.tile([P, J, NSC], bf16, name="z0", tag="z0")
                n_sb = gates.tile([P, J, NSC], bf16, name="n0", tag="n0")
                rhn = gates.tile([P, J, NSC], f32, name="rhn0", tag="rhn0")
                t1 = gates.tile([P, J, NSC], bf16, name="t10", tag="t10")
                nc.scalar.activation(r_sb[:], view3s(ps_r), Sig)
                nc.scalar.activation(z_sb[:], view3s(ps_z), Sig)
                # n = tanh(i_n + b_in + r*b_hn): gh_n of the zero state is
                # just b_hn, folded in per m via the scalar port
                for m in range(3):
                    nc.vector.scalar_tensor_tensor(
                        out=rhn[:, m],
                        in0=r_sb[:, m],
                        scalar=bias_sb[:, 6 + m : 7 + m],
                        in1=view3s(ps_in)[:, m],
                        op0=Mult,
                        op1=Add,
                    )
                nc.scalar.activation(n_sb[:], rhn[:], Tanh)
                # h0=0: h' = n - z*n
                hsl = h[:, :, sl]
                nc.vector.tensor_tensor(out=t1[:], in0=z_sb[:], in1=n_sb[:], op=Mult)
                nc.vector.tensor_tensor(out=hsl, in0=n_sb[:], in1=t1[:], op=Sub)
                nc.gpsimd.tensor_copy(
                    out=hacc[:, :, :, sc * (NSC // ARITY) : (sc + 1) * (NSC // ARITY)],
                    in_=hsl.rearrange("p j (q c) -> p j c q", c=ARITY),
                )

        NCH = 256
        nch = P5 // NCH
        # Skewed emission: chunk A's steps run while chunk B's gathers and
        # step-0 sub-chunks are still completing (the engine FIFOs are
        # in-order, so chunk B work must not be queued until its data is
        # nearly ready). B1 sits after A5.
        SKEW = 5
        step_seq = []
        for t in range(1, ARITY):
            step_seq.append((t, 0))
            if t >= SKEW + 1:
                step_seq.append((t - SKEW, 1))
        for t in range(ARITY - SKEW, ARITY):
            step_seq.append((t, 1))
        for t, ch in step_seq:
            c = ARITY - 1 - t
            with nc.named_scope(f"leaf_t{t}c{ch}"):
                if True:
                    sl = slice(ch * NCH, (ch + 1) * NCH)
                    ps_r = [psum_tile()[:, :NCH] for _ in range(3)]
                    ps_z = [psum_tile()[:, :NCH] for _ in range(3)]
                    ps_in = [psum_tile()[:, :NCH] for _ in range(3)]
                    ps_hn = [psum_tile()[:, :NCH] for _ in range(3)]
                    for ps, moff in ((ps_r, 0), (ps_z, 3), (ps_in, 6)):
                        for m in range(3):
                            for k in range(J):
                                nc.tensor.matmul(
                                    ps[m],
                                    wih_sb[:, k, moff + m, :],
                                    x5[:, k, c, sl],
                                    start=(k == 0),
                                    stop=(k == 2 and moff == 6),
                                )
                    for ps, moff in ((ps_r, 0), (ps_z, 3), (ps_hn, 6)):
                        for m in range(3):
                            for k in range(J):
                                nc.tensor.matmul(
                                    ps[m],
                                    whh_sb[:, k, moff + m, :],
                                    h[:, k, sl],
                                    start=(k == 0 and moff == 6),
                                    stop=(k == 2),
                                )

                    r_sb = gates.tile([P, J, NCH], bf16, name="r_sb", tag="r_sb")
                    z_sb = gates.tile([P, J, NCH], bf16, name="z_sb", tag="z_sb")
                    n_sb = gates.tile([P, J, NCH], bf16, name="n_sb", tag="n_sb")
                    rhn = gates.tile([P, J, NCH], f32, name="rhn", tag="rhn")
                    t1 = gates.tile([P, J, NCH], bf16, name="t1", tag="t1")

                    for m in range(3):
                        nc.scalar.activation(
                            r_sb[:, m], ps_r[m], Sig, bias=bias_sb[:, m : m + 1]
                        )
                    for m in range(3):
                        nc.scalar.activation(
                            z_sb[:, m], ps_z[m], Sig, bias=bias_sb[:, 3 + m : 4 + m]
                        )
                    for m in range(3):
                        nc.vector.scalar_tensor_tensor(
                            out=rhn[:, m],
                            in0=ps_hn[m],
                            scalar=bias_sb[:, 6 + m : 7 + m],
                            in1=r_sb[:, m],
                            op0=Add,
                            op1=Mult,
                        )
                    for m in range(3):
                        nc.vector.tensor_tensor(
                            out=rhn[:, m], in0=rhn[:, m], in1=ps_in[m], op=Add
                        )
                    for m in range(3):
                        nc.scalar.activation(
                            n_sb[:, m], rhn[:, m], Tanh, bias=bias_sb[:, 9 + m : 10 + m]
                        )

                    # h' = n + z*(h - n)
                    hsl = h[:, :, sl]
                    nc.vector.tensor_tensor(out=t1[:], in0=hsl, in1=n_sb[:], op=Sub)
                    nc.vector.tensor_tensor(out=t1[:], in0=z_sb[:], in1=t1[:], op=Mult)
                    nc.vector.tensor_tensor(out=hsl, in0=n_sb[:], in1=t1[:], op=Add)
                    hperm = hsl.rearrange("p j (q c) -> p j c q", c=ARITY)
                    qsl = slice(ch * NCH // ARITY, (ch + 1) * NCH // ARITY)
                    if t == ARITY - 1:
                        if ch == 0:
                            csum = state.tile(
                                [P, J, P5 // ARITY], f32, name="csum5", tag="csum5"
                            )
                            level_csum[0] = csum
                        nc.vector.tensor_reduce(
                            out=level_csum[0][:, :, qsl],
                            in_=hsl.rearrange("p j (q c) -> p j q c", c=ARITY),
                            axis=mybir.AxisListType.X,
                            op=Add,
                        )
                        xn = x_in[P5 // ARITY]
                        for j in range(J):
                            eng = nc.gpsimd if j == 2 else nc.vector
                            eng.tensor_tensor(
                                out=xn[:, j, :, qsl],
                                in0=hacc[:, j, :, qsl],
                                in1=hperm[:, j],
                                op=Add,
                            )
                        # bridge the level-end tail so the PE stays warm into
                        # the gi_64 pass
                        warm(8)
                    else:
                        nc.gpsimd.tensor_tensor(
                            out=hacc[:, :, :, qsl],
                            in0=hacc[:, :, :, qsl],
                            in1=hperm,
                            op=Add,
                        )

        # =====================  SMALL LEVELS (64, 8, 1)  ==================
        for Pl in levels[1:]:
            NC8 = ARITY * Pl  # children count = gi batch size
            with nc.named_scope(f"gi_{Pl}"):
                # gi stored child-major [P, 9, 8, Pl]: the gi pass rhs is the
                # child-major x (contiguous), so PSUM comes out (c, q)-ordered
                gi_sb = xpool.tile([P, 9, ARITY, Pl], bf16, name=f"gi{Pl}",
                                   tag=f"gi{Pl}")
                # m-order: r (0,1,2) first so step 0's r-inject unblocks early,
                # then z (3,4,5), then n (6,7,8)
                for mi, m in enumerate((0, 1, 2, 3, 4, 5, 6, 7, 8)):
                    ps = psum_tile()[:, :NC8]
                    nc.tensor.matmul(
                        ps, bias1_sb[:, m, :], ones_sb[:, :NC8],
                        start=True, stop=False,
                    )
                    for k in range(J):
                        nc.tensor.matmul(
                            ps,
                            wih_s_sb[:, k, m, :],
                            x_in[Pl][:, k, :, :],
                            start=False,
                            stop=(k == 2),
                        )
                    # alternate copy engine so the PSUM->SBUF drain keeps up
                    # with the matmul waves
                    if mi % 2 == 0:
                        nc.vector.tensor_copy(
                            out=gi_sb[:, m].rearrange("p c q -> p (c q)"), in_=ps
                        )
                    else:
                        nc.scalar.copy(
                            out=gi_sb[:, m].rearrange("p c q -> p (c q)"), in_=ps
                        )
                    if Pl == 64:
                        warm(1)

            csum = level_csum[0]
            h = new_state("h", bf16, Pl)
            hacc = new_state("hacc", f32, Pl)
            nc.scalar.mul(h[:], csum[:], 1.0 / ARITY)

            for t in range(ARITY):
                c = ARITY - 1 - t
                with nc.named_scope(f"lv{Pl}_t{t}"):
                    N3 = 3 * Pl
                    ps_z, ps_r, ps_hn = psum_tile(), psum_tile(), psum_tile()

                    def view3(pst):
                        return pst[:, :N3].rearrange("p (j n) -> p j n", j=3)

                    def msl(pst, m):
                        return pst[:, m * Pl : (m + 1) * Pl]

                    # r first: sigma(r) heads the serial chain, so its PSUM
                    # group must close first and nothing may sit ahead of
                    # sigma(r) in the scalar FIFO
                    nc.tensor.matmul(
                        ps_r[:, :N3], ident[:], gi_sb[:, 0:3, c, :],
                        start=True, stop=False,
                    )
                    for m in range(3):
                        for k in range(J):
                            nc.tensor.matmul(
                                msl(ps_r, m), whh_sb[:, k, m, :], h[:, k, :],
                                start=False, stop=(m == 2 and k == 2),
                            )
                    # hn second so its accumulation closes before z's: the
                    # serial path is sigma(r) -> rhn = ps_hn*r -> tanh
                    nc.tensor.matmul(
                        ps_hn[:, :N3], bias3_sb[:, 2, :], onehot3_sb[:, :, :Pl],
                        start=True, stop=False,
                    )
                    for m in range(3):
                        for k in range(J):
                            nc.tensor.matmul(
                                msl(ps_hn, m), whh_sb[:, k, 6 + m, :], h[:, k, :],
                                start=False, stop=(m == 2 and k == 2),
                            )
                    # z last (sigma(z)/zc/t2 have slack until the h update)
                    nc.tensor.matmul(
                        ps_z[:, :N3], ident[:], gi_sb[:, 3:6, c, :],
                        start=True, stop=False,
                    )
                    for m in range(3):
                        for k in range(J):
                            nc.tensor.matmul(
                                msl(ps_z, m), whh_sb[:, k, 3 + m, :], h[:, k, :],
                                start=False, stop=(m == 2 and k == 2),
                            )
                    warm(6 if Pl == 64 else 4)

                    z_sb = gates.tile([P, J, Pl], bf16, name="z_sb", tag="z_sb")
                    zc_sb = gates.tile([P, J, Pl], bf16, name="zc_sb", tag="zc_sb")
                    r_sb = gates.tile([P, J, Pl], bf16, name="r_sb", tag="r_sb")
                    n_sb = gates.tile([P, J, Pl], bf16, name="n_sb", tag="n_sb")
                    rhn = gates.tile([P, J, Pl], f32, name="rhn", tag="rhn")
                    t1 = gates.tile([P, J, Pl], f32, name="t1", tag="t1")
                    t2 = gates.tile([P, J, Pl], f32, name="t2", tag="t2")

                    nc.scalar.activation(r_sb[:], view3(ps_r), Sig)
                    nc.scalar.activation(z_sb[:], view3(ps_z), Sig)
                    nc.scalar.activation(zc_sb[:], view3(ps_z), Sig, scale=-1.0)
                    # t2 = z*h off-path while r/n compute
                    nc.gpsimd.tensor_tensor(out=t2[:], in0=z_sb[:], in1=h[:], op=Mult)
                    nc.vector.tensor_tensor(
                        out=rhn[:], in0=view3(ps_hn), in1=r_sb[:], op=Mult
                    )
                    nc.vector.tensor_tensor(
                        out=rhn[:], in0=rhn[:], in1=gi_sb[:, 6:9, c, :], op=Add
                    )
                    nc.scalar.activation(n_sb[:], rhn[:], Tanh)
                    # h' = zc*n + z*h
                    nc.vector.tensor_tensor(out=t1[:], in0=zc_sb[:], in1=n_sb[:], op=Mult)
                    nc.vector.tensor_tensor(out=h[:], in0=t1[:], in1=t2[:], op=Add)

                    if t == 0:
                        nc.gpsimd.tensor_copy(out=hacc[:], in_=h[:])
                    elif t == ARITY - 1 and Pl > 1:
                        csum = state.tile(
                            [P, J, Pl // ARITY], f32, name=f"csum{Pl}", tag=f"csum{Pl}"
                        )
                        level_csum[0] = csum
                        nc.vector.tensor_reduce(
                            out=csum[:],
                            in_=h[:].rearrange("p j (q c) -> p j q c", c=ARITY),
                            axis=mybir.AxisListType.X,
                            op=Add,
                        )
                        xn = x_in[Pl // ARITY]
                        nc.vector.tensor_tensor(
                            out=xn[:],
                            in0=hacc[:].rearrange("p j (q c) -> p j c q", c=ARITY),
                            in1=h[:].rearrange("p j (q c) -> p j c q", c=ARITY),
                            op=Add,
                        )
                    else:
                        nc.gpsimd.tensor_tensor(
                            out=hacc[:], in0=hacc[:], in1=h[:], op=Add
                        )

        # ---- outputs: [P, 2, J] = (x_root, h_root) ----
        out_t = state.tile([P, 2, J], f32, name="out_t", tag="out_t")
        nc.scalar.mul(out_t[:, 0], hacc[:, :, 0], 1.0 / ARITY)
        nc.vector.tensor_copy(out=out_t[:, 1], in_=h[:, :, 0])
        nc.sync.dma_start(out_xh[:], out_t[:])


def _build_program(n_leaves_core):
    if n_leaves_core in _PROG_CACHE:
        return _PROG_CACHE[n_leaves_core]
    import concourse.bacc as bacc
    import concourse.mybir as mybir
    import concourse.tile as tile

    f32 = mybir.dt.float32
    bf16 = mybir.dt.bfloat16

    nc = bacc.Bacc(
        "TRN2",
        target_bir_lowering=False,
        debug=False,
        enable_asserts=False,
        num_devices=NCORES,
        num_swdge_queues=4,
    )
    tokens = nc.dram_tensor(
        "tokens", [P, n_leaves_core // 16], mybir.dt.int16, kind="ExternalInput"
    ).ap()
    embed = nc.dram_tensor("embed", [VOCAB, DIM], bf16, kind="ExternalInput").ap()
    wih_t = nc.dram_tensor("wih_t", [P, J, 9, P], bf16, kind="ExternalInput").ap()
    wih_s = nc.dram_tensor("wih_s", [P, J, 9, P], bf16, kind="ExternalInput").ap()
    whh_t = nc.dram_tensor("whh_t", [P, J, 9, P], bf16, kind="ExternalInput").ap()
    biases = nc.dram_tensor("biases", [P, 12], f32, kind="ExternalInput").ap()
    biases_mm = nc.dram_tensor("biases_mm", [3, 4, P], bf16, kind="ExternalInput").ap()
    bias1 = nc.dram_tensor("bias1", [1, 9, P], bf16, kind="ExternalInput").ap()
    onehot3 = nc.dram_tensor("onehot3", [3, 3, 512], bf16, kind="ExternalInput").ap()
    ones = nc.dram_tensor("ones", [1, 512], bf16, kind="ExternalInput").ap()
    identity = nc.dram_tensor("identity", [P, P], bf16, kind="ExternalInput").ap()
    out_xh = nc.dram_tensor("out_xh", [P, 2, J], f32, kind="ExternalOutput").ap()

    with tile.TileContext(nc) as tc:
        _emit(
            tc,
            nc,
            (tokens, embed, wih_t, wih_s, whh_t, biases, biases_mm, bias1,
             onehot3, ones, identity, out_xh),
            n_leaves_core,
        )
    nc.compile()
    _PROG_CACHE[n_leaves_core] = nc
    return nc


def _retile_weights(w):
    # w: [1152, 384] -> lhsT tiles [128(k_part), 3(k), 9(m), 128(m_col)] bf16
    wt = np.ascontiguousarray(w.T)  # [384, 1152]
    wt = wt.reshape(J, P, 9, P).transpose(1, 0, 2, 3)
    return np.ascontiguousarray(wt).astype(BF16)


def _prep_bias(b_ih, b_hh):
    biases = np.zeros((P, 12), np.float32)
    comb = (b_ih + b_hh).reshape(9, P)
    biases[:, 0:6] = comb[0:6].T
    biases[:, 6:9] = b_hh.reshape(9, P)[6:9].T
    biases[:, 9:12] = b_ih.reshape(9, P)[6:9].T
    return biases


def _prep_bias_mm(b_ih, b_hh):
    # lhsT[k, ro, q] = bias[q, 3*ro + k]: the K=3 bias matmul against the
    # one-hot rhs yields out[q, (j, n)] = bias[q, 3*ro + j].
    b = _prep_bias(b_ih, b_hh)  # [128, 12] cols: r0..2 z0..2 hn0..2 in0..2
    out = b.T.reshape(4, 3, P).transpose(1, 0, 2)
    return np.ascontiguousarray(out).astype(BF16)


def _prep_bias1(b_ih, b_hh):
    # K=1 lhsT for the gi-precompute bias: out[col, :] += bias1[0, m, col].
    # r/z rows carry the combined input+hidden bias; n rows carry b_in only.
    out = np.zeros((1, 9, P), np.float32)
    comb = (b_ih + b_hh).reshape(9, P)
    out[0, 0:6] = comb[0:6]
    out[0, 6:9] = b_ih.reshape(9, P)[6:9]
    return out.astype(BF16)


def _wrap_tokens(tok):
    # dma_gather idx layout: idx i of a 512-row call lives at partition i%16,
    # column i//16, replicated across the 8 Q7 core partition-groups
    arr = np.zeros((P, len(tok) // 16), np.int16)
    for b in range(len(tok) // 512):
        blk = tok[b * 512 : (b + 1) * 512].astype(np.int16)
        m = blk.reshape(32, 16).T  # [16, 32]
        arr[:, b * 32 : (b + 1) * 32] = np.tile(m, (8, 1))
    return arr


def _prep_onehot3():
    out = np.zeros((3, 3, 512), np.float32)
    for k in range(3):
        out[k, k, :] = 1.0
    return out.astype(BF16)


def _gru_gates(x_t, h, w_ih, w_hh, b_ih, b_hh):
    gi = x_t @ w_ih.T + b_ih
    gh = h @ w_hh.T + b_hh
    i_r, i_z, i_n = np.split(gi, 3, axis=-1)
    h_r, h_z, h_n = np.split(gh, 3, axis=-1)
    r = 1.0 / (1.0 + np.exp(-(i_r + h_r)))
    z = 1.0 / (1.0 + np.exp(-(i_z + h_z)))
    n = np.tanh(i_n + r * h_n)
    return (1.0 - z) * n + z * h


def _root_gru(x_children, h0, w_ih, w_hh, b_ih, b_hh):
    h = h0.astype(np.float64)
    acc = np.zeros_like(h)
    for t in range(ARITY):
        x_t = x_children[ARITY - 1 - t].astype(np.float64)
        h = _gru_gates(x_t, h, w_ih.astype(np.float64), w_hh.astype(np.float64),
                       b_ih.astype(np.float64), b_hh.astype(np.float64))
        acc += h
    return (acc / ARITY).astype(np.float32)


def kernel(leaf_tokens, embed_table, w_ih, w_hh, b_ih, b_hh):
    from concourse.bass_utils import run_bass_kernel_spmd

    leaf_tokens = np.asarray(leaf_tokens, np.int32)
    embed_table = np.asarray(embed_table, np.float32)
    w_ih = np.asarray(w_ih, np.float32)
    w_hh = np.asarray(w_hh, np.float32)
    b_ih = np.asarray(b_ih, np.float32)
    b_hh = np.asarray(b_hh, np.float32)

    nc = _build_program(LEAVES_CORE)

    embed_bf = embed_table.astype(BF16)
    wih_t = _retile_weights(w_ih)
    wih_s = _retile_weights(w_ih / ARITY)
    whh_t = _retile_weights(w_hh)
    biases = _prep_bias(b_ih, b_hh)
    biases_mm = _prep_bias_mm(b_ih, b_hh)
    bias1 = _prep_bias1(b_ih, b_hh)
    ones = np.ones((1, 512), np.float32).astype(BF16)
    in_maps = []
    for core in range(NCORES):
        in_maps.append(
            {
                "tokens": _wrap_tokens(
                    leaf_tokens[core * LEAVES_CORE : (core + 1) * LEAVES_CORE]
                ),
                "embed": embed_bf,
                "wih_t": wih_t,
                "wih_s": wih_s,
                "whh_t": whh_t,
                "biases": biases,
                "biases_mm": biases_mm,
                "bias1": bias1,
                "onehot3": _prep_onehot3(),
                "ones": ones,
                "identity": np.eye(P, dtype=np.float32).astype(BF16),
            }
        )
    res = run_bass_kernel_spmd(nc, in_maps, core_ids=list(range(NCORES)))

    xs = np.zeros((NCORES, DIM), np.float32)
    h8 = np.zeros((NCORES, DIM), np.float32)
    for core in range(NCORES):
        out = res.results[core]["out_xh"]  # [P, 2, J]
        xs[core] = out[:, 0].T.reshape(-1)
        h8[core] = out[:, 1].T.reshape(-1)

    h0 = h8.mean(axis=0)
    out = _root_gru(xs, h0, w_ih, w_hh, b_ih, b_hh)
    return out.reshape(1, 1, DIM)


# revision 33
# speedup vs baseline: 1.1780x; 1.0658x over previous
"""Tree-GRU (arity-8, depth-5) over embedded leaves on 8 TRN2 NeuronCores.

Sharding: data-parallel over subtrees. Each core takes 4096 contiguous leaves
and runs levels 5..2 of the tree locally (512 -> 64 -> 8 -> 1 parents). The
root (level 1, 8 children = the 8 cores' level-2 outputs) is a trivial
16-matvec GRU done on host after gathering the per-core [384] outputs.

Device layout is feature-transposed: tensors live as [128 part, 3 ktile, ...]
with feature f = 128*k + p, so the GRU matmuls contract the partition dim.
Node storage is flat leaf-order (child-fastest), so all elementwise state
updates and the level-boundary x_next writes are contiguous; only the matmul
rhs / gi reads use stride-8 child slices. Weights are host-pre-transposed
into lhsT tiles; matmul dtype bf16 with fp32 PSUM accumulation.

Leaf level: embedding gather (indirect DMA, bf16 table, 4 SWDGE queues)
feeds PE transposes; step 0 (h=0, gi-only) runs in 4 sub-chunks of 128
parents that track gather completion; steps 1-7 ping-pong 2 chunks of 256.

Small levels (64/8/1 parents): the input transform gi for the whole level
(all 8 children x all parents) is precomputed in one batched matmul pass
(biases folded in via a K=1 ones-column matmul) and stored in SBUF, so the
sequential per-step work is only the recurrent matmul + gate chain. Per step
the r/z gi rows are injected into PSUM via an identity matmul (one start=True
covering MM per bank), the hn bias via a K=3 one-hot matmul, and the h update
uses h' = zc*n + z*h with zc = sigmoid(-pre_z) so z*h is computed off-path
(gpsimd) while tanh runs. The 1/8 output-mean scale is folded into a
pre-scaled copy of W_ih used by the gi passes; per-step output sums are
accumulated (hacc) and the final step writes the next level's input directly
as a fused raw-sum add.
"""

import numpy as np
import ml_dtypes

ARITY = 8
DIM = 384
VOCAB = 32000
NCORES = 8
P = 128
J = 3  # DIM // 128 feature tiles
N_LEAVES = 32768
LEAVES_CORE = N_LEAVES // NCORES  # 4096

BF16 = ml_dtypes.bfloat16

_PROG_CACHE = {}


def _levels_for(n_leaves_core):
    levels = []
    p = n_leaves_core // ARITY
    while p >= 1:
        levels.append(p)
        p //= ARITY
    assert levels[-1] == 1
    return levels


def _emit(tc, nc, aps, n_leaves_core):
    import concourse.mybir as mybir
    import concourse.bass as bass

    f32 = mybir.dt.float32
    bf16 = mybir.dt.bfloat16
    Sig = mybir.ActivationFunctionType.Sigmoid
    Tanh = mybir.ActivationFunctionType.Tanh
    Add = mybir.AluOpType.add
    Sub = mybir.AluOpType.subtract
    Mult = mybir.AluOpType.mult

    (tokens, embed, wih_t, wih_s, whh_t, biases, biases_mm, bias1, onehot3,
     ones, identity, out_xh) = aps
    levels = _levels_for(n_leaves_core)
    P5 = levels[0]
    n_gtiles = n_leaves_core // P  # 32

    from contextlib import ExitStack

    with ExitStack() as ctx:
        const = ctx.enter_context(tc.tile_pool(name="const", bufs=1))
        xpool = ctx.enter_context(tc.tile_pool(name="xpool", bufs=1))
        state = ctx.enter_context(tc.tile_pool(name="state", bufs=1))
        gates = ctx.enter_context(tc.tile_pool(name="gates", bufs=6))
        gpool = ctx.enter_context(tc.tile_pool(name="gpool", bufs=1))
        pspool = ctx.enter_context(tc.tile_pool(name="pspool", bufs=8, space="PSUM"))

        # ---- tokens first, then kick off all gathers (dma_gather ucode,
        # int16 idx lists, 512 rows per call; 2 SWDGE queues). The gathers
        # are paced by gpsimd descriptor generation, so nothing else may sit
        # ahead of them in the gpsimd FIFO -- the identity build comes after.
        from concourse.library_config import mlp as _mlp_lib

        GB = 512  # rows per dma_gather call (64 parents, child-major)
        n_gcalls = n_leaves_core // GB  # 8
        QB = GB // ARITY  # 64 parents per call tile
        tok_sb = const.tile([P, n_gcalls * (GB // 16)], mybir.dt.int16)
        nc.sync.dma_start(tok_sb[:], tokens[:])
        nc.gpsimd.load_library(_mlp_lib)

        # transpose=True lands each embedding row across partitions with
        # feature f = 128*j + p -- exactly the matmul lhsT layout -- and the
        # host orders each call's 512 indices child-major, so the tile is
        # directly the [P, J, 8, 64] x input. No PE transposes, no copies.
        xts = []
        for b in range(n_gcalls):
            xt = gpool.tile([P, J, GB], bf16, name="xt", tag="xt", bufs=n_gcalls)
            nc.gpsimd.dma_gather(
                xt[:],
                embed[:],
                tok_sb[:, b * (GB // 16) : (b + 1) * (GB // 16)],
                GB,
                GB,
                DIM,
                transpose=True,
                queue_num=(b % 2),
            )
            xts.append(xt)

        def xtv(b):
            # child-sliced view of call b: [P, J, 8, QB]
            return xts[b][:].rearrange("p j (c q) -> p j c q", q=QB)

        # identity comes in via DMA so no gpsimd work sits ahead of the
        # gather descriptor generation
        ident = const.tile([P, P], bf16)
        nc.sync.dma_start(ident[:], identity[:])

        # ---- constants / weights ----
        wih_sb = const.tile([P, J, 9, P], bf16)
        wih_s_sb = const.tile([P, J, 9, P], bf16)
        whh_sb = const.tile([P, J, 9, P], bf16)
        bias_sb = const.tile([P, 12], f32)
        bias3_sb = const.tile([3, 4, P], bf16)
        bias1_sb = const.tile([1, 9, P], bf16)
        onehot3_sb = const.tile([3, 3, 512], bf16)
        ones_sb = const.tile([1, 512], bf16)
        nc.sync.dma_start(wih_sb[:], wih_t[:])
        nc.sync.dma_start(wih_s_sb[:], wih_s[:])
        nc.sync.dma_start(whh_sb[:], whh_t[:])
        nc.sync.dma_start(bias_sb[:], biases[:])
        nc.sync.dma_start(bias3_sb[:], biases_mm[:])
        nc.sync.dma_start(bias1_sb[:], bias1[:])
        nc.sync.dma_start(onehot3_sb[:], onehot3[:])
        nc.sync.dma_start(ones_sb[:], ones[:])

        # child-major x per level: [P, J, 8, Pl] (contiguous matmul rhs)
        x_in = {}
        for Pl in levels[1:]:
            x_in[Pl] = xpool.tile([P, J, ARITY, Pl], bf16, name=f"x{Pl}", tag=f"x{Pl}")

        def psum_tile():
            return pspool.tile([P, 512], f32, name="ps", tag="ps")

        def warm(n):
            # PE warm-keeper: HAM re-throttles the PE clock to 1.2 GHz after
            # ~3.4us of idle; during the latency-bound small levels, issue
            # dependency-free matmuls so the gate-chain windows don't cool
            # the PE and the next real matmul burst runs at 2.4 GHz.
            for _ in range(n):
                wp = pspool.tile([P, 512], f32, name="warm", tag="ps")
                nc.tensor.matmul(
                    wp[:, :512], ident[:], wih_sb[:, 0, 0:4, :], start=True, stop=True
                )

        def new_state(name, dtype, Pl):
            return state.tile([P, J, Pl], dtype, name=f"{name}{Pl}", tag=f"{name}{Pl}")

        # =====================  LEAF LEVEL (Pl = P5)  =====================
        h = new_state("h", bf16, P5)
        # permuted (child-major) accumulator so the level-end x_next add is a
        # contiguous write into the next level's child-major x
        hacc = state.tile([P, J, ARITY, P5 // ARITY], f32, name="hacc5", tag="hacc5")

        level_csum = [None]

        with nc.named_scope("leaf_t0"):
            # step 0: h=0, gi only; 4 sub-chunks of 128 parents (8 gtiles
            # each) that track gather completion. Biases are injected into
            # PSUM via the K=3 one-hot matmul (the single start=True per
            # bank), so the activations span all 3 m-tiles in one op.
            NSC = P5 // 4  # 128 parents per sub-chunk

            c0 = ARITY - 1  # first GRU input is the last child
            for sc in range(4):
                sl = slice(sc * NSC, (sc + 1) * NSC)
                N3 = 3 * NSC
                ps_r, ps_z, ps_in = psum_tile(), psum_tile(), psum_tile()

                def view3s(pst):
                    return pst[:, :N3].rearrange("p (j n) -> p j n", j=3)

                for pst, ro, moff in ((ps_r, 0, 0), (ps_z, 1, 3), (ps_in, 3, 6)):
                    nc.tensor.matmul(
                        pst[:, :N3], bias3_sb[:, ro, :], onehot3_sb[:, :, :NSC],
                        start=True, stop=False,
                    )
                    for m in range(3):
                        for k in range(J):
                            for b2 in range(2):
                                nc.tensor.matmul(
                                    pst[:, m * NSC + b2 * QB : m * NSC + (b2 + 1) * QB],
                                    wih_sb[:, k, moff + m, :],
                                    xtv(2 * sc + b2)[:, k, c0, :],
                                    start=False,
                                    stop=(m == 2 and k == 2 and b2 == 1),
                                )
                r_sb = gates.tile([P, J, NSC], bf16, name="r0", tag="r0")
                z_sb = gates.tile([P, J, NSC], bf16, name="z0", tag="z0")
                n_sb = gates.tile([P, J, NSC], bf16, name="n0", tag="n0")
                rhn = gates.tile([P, J, NSC], f32, name="rhn0", tag="rhn0")
                t1 = gates.tile([P, J, NSC], bf16, name="t10", tag="t10")
                nc.scalar.activation(r_sb[:], view3s(ps_r), Sig)
                nc.scalar.activation(z_sb[:], view3s(ps_z), Sig)
                # n = tanh(i_n + b_in + r*b_hn): gh_n of the zero state is
                # just b_hn, folded in per m via the scalar port
                for m in range(3):
                    nc.vector.scalar_tensor_tensor(
                        out=rhn[:, m],
                        in0=r_sb[:, m],
                        scalar=bias_sb[:, 6 + m : 7 + m],
                        in1=view3s(ps_in)[:, m],
                        op0=Mult,
                        op1=Add,
                    )
                nc.scalar.activation(n_sb[:], rhn[:], Tanh)
                # h0=0: h' = n - z*n
                hsl = h[:, :, sl]
                nc.vector.tensor_tensor(out=t1[:], in0=z_sb[:], in1=n_sb[:], op=Mult)
                nc.vector.tensor_tensor(out=hsl, in0=n_sb[:], in1=t1[:], op=Sub)
                nc.gpsimd.tensor_copy(
                    out=hacc[:, :, :, sc * (NSC // ARITY) : (sc + 1) * (NSC // ARITY)],
                    in_=hsl.rearrange("p j (q c) -> p j c q", c=ARITY),
                )

        NCH = 256
        nch = P5 // NCH
        # Skewed emission: chunk A's steps run while chunk B's gathers and
        # step-0 sub-chunks are still completing (the engine FIFOs are
        # in-order, so chunk B work must not be queued until its data is
        # nearly ready). B1 sits after A5.
        SKEW = 2
        step_seq = []
        for t in range(1, ARITY):
            step_seq.append((t, 0))
            if t >= SKEW + 1:
                step_seq.append((t - SKEW, 1))
        for t in range(ARITY - SKEW, ARITY):
            step_seq.append((t, 1))
        for t, ch in step_seq:
            c = ARITY - 1 - t
            with nc.named_scope(f"leaf_t{t}c{ch}"):
                if True:
                    sl = slice(ch * NCH, (ch + 1) * NCH)
                    ps_r = [psum_tile()[:, :NCH] for _ in range(3)]
                    ps_z = [psum_tile()[:, :NCH] for _ in range(3)]
                    ps_in = [psum_tile()[:, :NCH] for _ in range(3)]
                    ps_hn = [psum_tile()[:, :NCH] for _ in range(3)]
                    for ps, moff in ((ps_r, 0), (ps_z, 3), (ps_in, 6)):
                        for m in range(3):
                            for k in range(J):
                                for b4 in range(4):
                                    nc.tensor.matmul(
                                        ps[m][:, b4 * QB : (b4 + 1) * QB],
                                        wih_sb[:, k, moff + m, :],
                                        xtv(4 * ch + b4)[:, k, c, :],
                                        start=(k == 0 and b4 == 0),
                                        stop=(k == 2 and moff == 6 and b4 == 3),
                                    )
                    for ps, moff in ((ps_r, 0), (ps_z, 3), (ps_hn, 6)):
                        for m in range(3):
                            for k in range(J):
                                nc.tensor.matmul(
                                    ps[m],
                                    whh_sb[:, k, moff + m, :],
                                    h[:, k, sl],
                                    start=(k == 0 and moff == 6),
                                    stop=(k == 2),
                                )

                    r_sb = gates.tile([P, J, NCH], bf16, name="r_sb", tag="r_sb")
                    z_sb = gates.tile([P, J, NCH], bf16, name="z_sb", tag="z_sb")
                    n_sb = gates.tile([P, J, NCH], bf16, name="n_sb", tag="n_sb")
                    rhn = gates.tile([P, J, NCH], f32, name="rhn", tag="rhn")
                    t1 = gates.tile([P, J, NCH], bf16, name="t1", tag="t1")

                    for m in range(3):
                        nc.scalar.activation(
                            r_sb[:, m], ps_r[m], Sig, bias=bias_sb[:, m : m + 1]
                        )
                    for m in range(3):
                        nc.scalar.activation(
                            z_sb[:, m], ps_z[m], Sig, bias=bias_sb[:, 3 + m : 4 + m]
                        )
                    for m in range(3):
                        nc.vector.scalar_tensor_tensor(
                            out=rhn[:, m],
                            in0=ps_hn[m],
                            scalar=bias_sb[:, 6 + m : 7 + m],
                            in1=r_sb[:, m],
                            op0=Add,
                            op1=Mult,
                        )
                    for m in range(3):
                        nc.vector.tensor_tensor(
                            out=rhn[:, m], in0=rhn[:, m], in1=ps_in[m], op=Add
                        )
                    for m in range(3):
                        nc.scalar.activation(
                            n_sb[:, m], rhn[:, m], Tanh, bias=bias_sb[:, 9 + m : 10 + m]
                        )

                    # h' = n + z*(h - n)
                    hsl = h[:, :, sl]
                    nc.vector.tensor_tensor(out=t1[:], in0=hsl, in1=n_sb[:], op=Sub)
                    nc.vector.tensor_tensor(out=t1[:], in0=z_sb[:], in1=t1[:], op=Mult)
                    nc.vector.tensor_tensor(out=hsl, in0=n_sb[:], in1=t1[:], op=Add)
                    hperm = hsl.rearrange("p j (q c) -> p j c q", c=ARITY)
                    qsl = slice(ch * NCH // ARITY, (ch + 1) * NCH // ARITY)
                    if t == ARITY - 1:
                        if ch == 0:
                            csum = state.tile(
                                [P, J, P5 // ARITY], f32, name="csum5", tag="csum5"
                            )
                            level_csum[0] = csum
                        nc.vector.tensor_reduce(
                            out=level_csum[0][:, :, qsl],
                            in_=hsl.rearrange("p j (q c) -> p j q c", c=ARITY),
                            axis=mybir.AxisListType.X,
                            op=Add,
                        )
                        xn = x_in[P5 // ARITY]
                        for j in range(J):
                            eng = nc.gpsimd if j == 2 else nc.vector
                            eng.tensor_tensor(
                                out=xn[:, j, :, qsl],
                                in0=hacc[:, j, :, qsl],
                                in1=hperm[:, j],
                                op=Add,
                            )
                        # bridge the level-end tail so the PE stays warm into
                        # the gi_64 pass
                        warm(8)
                    else:
                        nc.gpsimd.tensor_tensor(
                            out=hacc[:, :, :, qsl],
                            in0=hacc[:, :, :, qsl],
                            in1=hperm,
                            op=Add,
                        )

        # =====================  SMALL LEVELS (64, 8, 1)  ==================
        for Pl in levels[1:]:
            NC8 = ARITY * Pl  # children count = gi batch size
            with nc.named_scope(f"gi_{Pl}"):
                # gi stored child-major [P, 9, 8, Pl]: the gi pass rhs is the
                # child-major x (contiguous), so PSUM comes out (c, q)-ordered
                gi_sb = xpool.tile([P, 9, ARITY, Pl], bf16, name=f"gi{Pl}",
                                   tag=f"gi{Pl}")
                # m-order: r (0,1,2) first so step 0's r-inject unblocks early,
                # then z (3,4,5), then n (6,7,8)
                for mi, m in enumerate((0, 1, 2, 3, 4, 5, 6, 7, 8)):
                    ps = psum_tile()[:, :NC8]
                    nc.tensor.matmul(
                        ps, bias1_sb[:, m, :], ones_sb[:, :NC8],
                        start=True, stop=False,
                    )
                    for k in range(J):
                        nc.tensor.matmul(
                            ps,
                            wih_s_sb[:, k, m, :],
                            x_in[Pl][:, k, :, :],
                            start=False,
                            stop=(k == 2),
                        )
                    # alternate copy engine so the PSUM->SBUF drain keeps up
                    # with the matmul waves
                    if mi % 2 == 0:
                        nc.vector.tensor_copy(
                            out=gi_sb[:, m].rearrange("p c q -> p (c q)"), in_=ps
                        )
                    else:
                        nc.scalar.copy(
                            out=gi_sb[:, m].rearrange("p c q -> p (c q)"), in_=ps
                        )
                    if Pl == 64:
                        warm(1)

            csum = level_csum[0]
            h = new_state("h", bf16, Pl)
            hacc = new_state("hacc", f32, Pl)
            nc.scalar.mul(h[:], csum[:], 1.0 / ARITY)

            for t in range(ARITY):
                c = ARITY - 1 - t
                with nc.named_scope(f"lv{Pl}_t{t}"):
                    N3 = 3 * Pl
                    ps_z, ps_r, ps_hn = psum_tile(), psum_tile(), psum_tile()

                    def view3(pst):
                        return pst[:, :N3].rearrange("p (j n) -> p j n", j=3)

                    def msl(pst, m):
                        return pst[:, m * Pl : (m + 1) * Pl]

                    # r first: sigma(r) heads the serial chain, so its PSUM
                    # group must close first and nothing may sit ahead of
                    # sigma(r) in the scalar FIFO
                    nc.tensor.matmul(
                        ps_r[:, :N3], ident[:], gi_sb[:, 0:3, c, :],
                        start=True, stop=False,
                    )
                    for m in range(3):
                        for k in range(J):
                            nc.tensor.matmul(
                                msl(ps_r, m), whh_sb[:, k, m, :], h[:, k, :],
                                start=False, stop=(m == 2 and k == 2),
                            )
                    # hn second so its accumulation closes before z's: the
                    # serial path is sigma(r) -> rhn = ps_hn*r -> tanh
                    nc.tensor.matmul(
                        ps_hn[:, :N3], bias3_sb[:, 2, :], onehot3_sb[:, :, :Pl],
                        start=True, stop=False,
                    )
                    for m in range(3):
                        for k in range(J):
                            nc.tensor.matmul(
                                msl(ps_hn, m), whh_sb[:, k, 6 + m, :], h[:, k, :],
                                start=False, stop=(m == 2 and k == 2),
                            )
                    # z last (sigma(z)/zc/t2 have slack until the h update)
                    nc.tensor.matmul(
                        ps_z[:, :N3], ident[:], gi_sb[:, 3:6, c, :],
                        start=True, stop=False,
                    )
                    for m in range(3):
                        for k in range(J):
                            nc.tensor.matmul(
                                msl(ps_z, m), whh_sb[:, k, 3 + m, :], h[:, k, :],
                                start=False, stop=(m == 2 and k == 2),
                            )
                    warm(6 if Pl == 64 else 4)

                    z_sb = gates.tile([P, J, Pl], bf16, name="z_sb", tag="z_sb")
                    zc_sb = gates.tile([P, J, Pl], bf16, name="zc_sb", tag="zc_sb")
                    r_sb = gates.tile([P, J, Pl], bf16, name="r_sb", tag="r_sb")
                    n_sb = gates.tile([P, J, Pl], bf16, name="n_sb", tag="n_sb")
                    rhn = gates.tile([P, J, Pl], f32, name="rhn", tag="rhn")
                    t1 = gates.tile([P, J, Pl], f32, name="t1", tag="t1")
                    t2 = gates.tile([P, J, Pl], f32, name="t2", tag="t2")

                    nc.scalar.activation(r_sb[:], view3(ps_r), Sig)
                    nc.scalar.activation(z_sb[:], view3(ps_z), Sig)
                    nc.scalar.activation(zc_sb[:], view3(ps_z), Sig, scale=-1.0)
                    # t2 = z*h off-path while r/n compute
                    nc.gpsimd.tensor_tensor(out=t2[:], in0=z_sb[:], in1=h[:], op=Mult)
                    nc.vector.tensor_tensor(
                        out=rhn[:], in0=view3(ps_hn), in1=r_sb[:], op=Mult
                    )
                    nc.vector.tensor_tensor(
                        out=rhn[:], in0=rhn[:], in1=gi_sb[:, 6:9, c, :], op=Add
                    )
                    nc.scalar.activation(n_sb[:], rhn[:], Tanh)
                    # h' = zc*n + z*h
                    nc.vector.tensor_tensor(out=t1[:], in0=zc_sb[:], in1=n_sb[:], op=Mult)
                    nc.vector.tensor_tensor(out=h[:], in0=t1[:], in1=t2[:], op=Add)

                    if t == 0:
                        nc.gpsimd.tensor_copy(out=hacc[:], in_=h[:])
                    elif t == ARITY - 1 and Pl > 1:
                        csum = state.tile(
                            [P, J, Pl // ARITY], f32, name=f"csum{Pl}", tag=f"csum{Pl}"
                        )
                        level_csum[0] = csum
                        nc.vector.tensor_reduce(
                            out=csum[:],
                            in_=h[:].rearrange("p j (q c) -> p j q c", c=ARITY),
                            axis=mybir.AxisListType.X,
                            op=Add,
                        )
                        xn = x_in[Pl // ARITY]
                        nc.vector.tensor_tensor(
                            out=xn[:],
                            in0=hacc[:].rearrange("p j (q c) -> p j c q", c=ARITY),
                            in1=h[:].rearrange("p j (q c) -> p j c q", c=ARITY),
                            op=Add,
                        )
                    else:
                        nc.gpsimd.tensor_tensor(
                            out=hacc[:], in0=hacc[:], in1=h[:], op=Add
                        )

        # ---- outputs: [P, 2, J] = (x_root, h_root) ----
        out_t = state.tile([P, 2, J], f32, name="out_t", tag="out_t")
        nc.scalar.mul(out_t[:, 0], hacc[:, :, 0], 1.0 / ARITY)
        nc.vector.tensor_copy(out=out_t[:, 1], in_=h[:, :, 0])
        nc.sync.dma_start(out_xh[:], out_t[:])


def _build_program(n_leaves_core):
    if n_leaves_core in _PROG_CACHE:
        return _PROG_CACHE[n_leaves_core]
    import concourse.bacc as bacc
    import concourse.mybir as mybir
    import concourse.tile as tile

    f32 = mybir.dt.float32
    bf16 = mybir.dt.bfloat16

    nc = bacc.Bacc(
        "TRN2",
        target_bir_lowering=False,
        debug=False,
        enable_asserts=False,
        num_devices=NCORES,
        num_swdge_queues=4,
    )
    tokens = nc.dram_tensor(
        "tokens", [P, n_leaves_core // 16], mybir.dt.int16, kind="ExternalInput"
    ).ap()
    embed = nc.dram_tensor("embed", [VOCAB, DIM], bf16, kind="ExternalInput").ap()
    wih_t = nc.dram_tensor("wih_t", [P, J, 9, P], bf16, kind="ExternalInput").ap()
    wih_s = nc.dram_tensor("wih_s", [P, J, 9, P], bf16, kind="ExternalInput").ap()
    whh_t = nc.dram_tensor("whh_t", [P, J, 9, P], bf16, kind="ExternalInput").ap()
    biases = nc.dram_tensor("biases", [P, 12], f32, kind="ExternalInput").ap()
    biases_mm = nc.dram_tensor("biases_mm", [3, 4, P], bf16, kind="ExternalInput").ap()
    bias1 = nc.dram_tensor("bias1", [1, 9, P], bf16, kind="ExternalInput").ap()
    onehot3 = nc.dram_tensor("onehot3", [3, 3, 512], bf16, kind="ExternalInput").ap()
    ones = nc.dram_tensor("ones", [1, 512], bf16, kind="ExternalInput").ap()
    identity = nc.dram_tensor("identity", [P, P], bf16, kind="ExternalInput").ap()
    out_xh = nc.dram_tensor("out_xh", [P, 2, J], f32, kind="ExternalOutput").ap()

    with tile.TileContext(nc) as tc:
        _emit(
            tc,
            nc,
            (tokens, embed, wih_t, wih_s, whh_t, biases, biases_mm, bias1,
             onehot3, ones, identity, out_xh),
            n_leaves_core,
        )
    nc.compile()
    _PROG_CACHE[n_leaves_core] = nc
    return nc


def _retile_weights(w):
    # w: [1152, 384] -> lhsT tiles [128(k_part), 3(k), 9(m), 128(m_col)] bf16
    wt = np.ascontiguousarray(w.T)  # [384, 1152]
    wt = wt.reshape(J, P, 9, P).transpose(1, 0, 2, 3)
    return np.ascontiguousarray(wt).astype(BF16)


def _prep_bias(b_ih, b_hh):
    biases = np.zeros((P, 12), np.float32)
    comb = (b_ih + b_hh).reshape(9, P)
    biases[:, 0:6] = comb[0:6].T
    biases[:, 6:9] = b_hh.reshape(9, P)[6:9].T
    biases[:, 9:12] = b_ih.reshape(9, P)[6:9].T
    return biases


def _prep_bias_mm(b_ih, b_hh):
    # lhsT[k, ro, q] = bias[q, 3*ro + k]: the K=3 bias matmul against the
    # one-hot rhs yields out[q, (j, n)] = bias[q, 3*ro + j].
    b = _prep_bias(b_ih, b_hh)  # [128, 12] cols: r0..2 z0..2 hn0..2 in0..2
    out = b.T.reshape(4, 3, P).transpose(1, 0, 2)
    return np.ascontiguousarray(out).astype(BF16)


def _prep_bias1(b_ih, b_hh):
    # K=1 lhsT for the gi-precompute bias: out[col, :] += bias1[0, m, col].
    # r/z rows carry the combined input+hidden bias; n rows carry b_in only.
    out = np.zeros((1, 9, P), np.float32)
    comb = (b_ih + b_hh).reshape(9, P)
    out[0, 0:6] = comb[0:6]
    out[0, 6:9] = b_ih.reshape(9, P)[6:9]
    return out.astype(BF16)


def _wrap_tokens(tok):
    # dma_gather idx layout: idx i of a 512-row call lives at partition i%16,
    # column i//16, replicated across the 8 Q7 core partition-groups. Each
    # call's 512 indices are pre-permuted child-major (position c*64+q is
    # leaf 8q+c), so the transposing gather writes the x tile directly in
    # [P, J, 8 child, 64 parent] order.
    arr = np.zeros((P, len(tok) // 16), np.int16)
    for b in range(len(tok) // 512):
        blk = tok[b * 512 : (b + 1) * 512].astype(np.int16)
        cm = np.ascontiguousarray(blk.reshape(64, 8).T).reshape(-1)
        m = cm.reshape(32, 16).T  # [16, 32]
        arr[:, b * 32 : (b + 1) * 32] = np.tile(m, (8, 1))
    return arr


def _prep_onehot3():
    out = np.zeros((3, 3, 512), np.float32)
    for k in range(3):
        out[k, k, :] = 1.0
    return out.astype(BF16)


def _gru_gates(x_t, h, w_ih, w_hh, b_ih, b_hh):
    gi = x_t @ w_ih.T + b_ih
    gh = h @ w_hh.T + b_hh
    i_r, i_z, i_n = np.split(gi, 3, axis=-1)
    h_r, h_z, h_n = np.split(gh, 3, axis=-1)
    r = 1.0 / (1.0 + np.exp(-(i_r + h_r)))
    z = 1.0 / (1.0 + np.exp(-(i_z + h_z)))
    n = np.tanh(i_n + r * h_n)
    return (1.0 - z) * n + z * h


def _root_gru(x_children, h0, w_ih, w_hh, b_ih, b_hh):
    h = h0.astype(np.float64)
    acc = np.zeros_like(h)
    for t in range(ARITY):
        x_t = x_children[ARITY - 1 - t].astype(np.float64)
        h = _gru_gates(x_t, h, w_ih.astype(np.float64), w_hh.astype(np.float64),
                       b_ih.astype(np.float64), b_hh.astype(np.float64))
        acc += h
    return (acc / ARITY).astype(np.float32)


def kernel(leaf_tokens, embed_table, w_ih, w_hh, b_ih, b_hh):
    from concourse.bass_utils import run_bass_kernel_spmd

    leaf_tokens = np.asarray(leaf_tokens, np.int32)
    embed_table = np.asarray(embed_table, np.float32)
    w_ih = np.asarray(w_ih, np.float32)
    w_hh = np.asarray(w_hh, np.float32)
    b_ih = np.asarray(b_ih, np.float32)
    b_hh = np.asarray(b_hh, np.float32)

    nc = _build_program(LEAVES_CORE)

    embed_bf = embed_table.astype(BF16)
    wih_t = _retile_weights(w_ih)
    wih_s = _retile_weights(w_ih / ARITY)
    whh_t = _retile_weights(w_hh)
    biases = _prep_bias(b_ih, b_hh)
    biases_mm = _prep_bias_mm(b_ih, b_hh)
    bias1 = _prep_bias1(b_ih, b_hh)
    ones = np.ones((1, 512), np.float32).astype(BF16)
    in_maps = []
    for core in range(NCORES):
        in_maps.append(
            {
                "tokens": _wrap_tokens(
                    leaf_tokens[core * LEAVES_CORE : (core + 1) * LEAVES_CORE]
                ),
                "embed": embed_bf,
                "wih_t": wih_t,
                "wih_s": wih_s,
                "whh_t": whh_t,
                "biases": biases,
                "biases_mm": biases_mm,
                "bias1": bias1,
                "onehot3": _prep_onehot3(),
                "ones": ones,
                "identity": np.eye(P, dtype=np.float32).astype(BF16),
            }
        )
    res = run_bass_kernel_spmd(nc, in_maps, core_ids=list(range(NCORES)))

    xs = np.zeros((NCORES, DIM), np.float32)
    h8 = np.zeros((NCORES, DIM), np.float32)
    for core in range(NCORES):
        out = res.results[core]["out_xh"]  # [P, 2, J]
        xs[core] = out[:, 0].T.reshape(-1)
        h8[core] = out[:, 1].T.reshape(-1)

    h0 = h8.mean(axis=0)
    out = _root_gru(xs, h0, w_ih, w_hh, b_ih, b_hh)
    return out.reshape(1, 1, DIM)


# revision 35
# speedup vs baseline: 1.2388x; 1.0516x over previous
"""Tree-GRU (arity-8, depth-5) over embedded leaves on 8 TRN2 NeuronCores.

Sharding: data-parallel over subtrees. Each core takes 4096 contiguous leaves
and runs levels 5..2 of the tree locally (512 -> 64 -> 8 -> 1 parents). The
root (level 1, 8 children = the 8 cores' level-2 outputs) is a trivial
16-matvec GRU done on host after gathering the per-core [384] outputs.

Device layout is feature-transposed: tensors live as [128 part, 3 ktile, ...]
with feature f = 128*k + p, so the GRU matmuls contract the partition dim.
Node storage is flat leaf-order (child-fastest), so all elementwise state
updates and the level-boundary x_next writes are contiguous; only the matmul
rhs / gi reads use stride-8 child slices. Weights are host-pre-transposed
into lhsT tiles; matmul dtype bf16 with fp32 PSUM accumulation.

Leaf level: embedding gather (indirect DMA, bf16 table, 4 SWDGE queues)
feeds PE transposes; step 0 (h=0, gi-only) runs in 4 sub-chunks of 128
parents that track gather completion; steps 1-7 ping-pong 2 chunks of 256.

Small levels (64/8/1 parents): the input transform gi for the whole level
(all 8 children x all parents) is precomputed in one batched matmul pass
(biases folded in via a K=1 ones-column matmul) and stored in SBUF, so the
sequential per-step work is only the recurrent matmul + gate chain. Per step
the r/z gi rows are injected into PSUM via an identity matmul (one start=True
covering MM per bank), the hn bias via a K=3 one-hot matmul, and the h update
uses h' = zc*n + z*h with zc = sigmoid(-pre_z) so z*h is computed off-path
(gpsimd) while tanh runs. The 1/8 output-mean scale is folded into a
pre-scaled copy of W_ih used by the gi passes; per-step output sums are
accumulated (hacc) and the final step writes the next level's input directly
as a fused raw-sum add.
"""

import numpy as np
import ml_dtypes

ARITY = 8
DIM = 384
VOCAB = 32000
NCORES = 8
P = 128
J = 3  # DIM // 128 feature tiles
N_LEAVES = 32768
LEAVES_CORE = N_LEAVES // NCORES  # 4096

BF16 = ml_dtypes.bfloat16

_PROG_CACHE = {}


def _levels_for(n_leaves_core):
    levels = []
    p = n_leaves_core // ARITY
    while p >= 1:
        levels.append(p)
        p //= ARITY
    assert levels[-1] == 1
    return levels


def _emit(tc, nc, aps, n_leaves_core):
    import concourse.mybir as mybir
    import concourse.bass as bass

    f32 = mybir.dt.float32
    bf16 = mybir.dt.bfloat16
    Sig = mybir.ActivationFunctionType.Sigmoid
    Tanh = mybir.ActivationFunctionType.Tanh
    Add = mybir.AluOpType.add
    Sub = mybir.AluOpType.subtract
    Mult = mybir.AluOpType.mult

    (tokens, embed, wih_t, wih_s, whh_t, biases, biases_mm, bias1, onehot3,
     ones, identity, out_xh) = aps
    levels = _levels_for(n_leaves_core)
    P5 = levels[0]
    n_gtiles = n_leaves_core // P  # 32

    from contextlib import ExitStack

    with ExitStack() as ctx:
        const = ctx.enter_context(tc.tile_pool(name="const", bufs=1))
        xpool = ctx.enter_context(tc.tile_pool(name="xpool", bufs=1))
        state = ctx.enter_context(tc.tile_pool(name="state", bufs=1))
        gates = ctx.enter_context(tc.tile_pool(name="gates", bufs=6))
        gpool = ctx.enter_context(tc.tile_pool(name="gpool", bufs=1))
        pspool = ctx.enter_context(tc.tile_pool(name="pspool", bufs=8, space="PSUM"))

        # ---- tokens first, then kick off all gathers (dma_gather ucode,
        # int16 idx lists, 512 rows per call; 2 SWDGE queues). The gathers
        # are paced by gpsimd descriptor generation, so nothing else may sit
        # ahead of them in the gpsimd FIFO -- the identity build comes after.
        from concourse.library_config import mlp as _mlp_lib

        GB = 512  # rows per dma_gather call (64 parents, child-major)
        n_gcalls = n_leaves_core // GB  # 8
        QB = GB // ARITY  # 64 parents per call tile
        tok_sb = const.tile([P, n_gcalls * (GB // 16)], mybir.dt.int16)
        nc.sync.dma_start(tok_sb[:], tokens[:])
        nc.gpsimd.load_library(_mlp_lib)

        # transpose=True lands each embedding row across partitions with
        # feature f = 128*j + p -- exactly the matmul lhsT layout -- and the
        # host orders each call's 512 indices child-major, so the tile is
        # directly the [P, J, 8, 64] x input. No PE transposes, no copies.
        xts = []
        for b in range(n_gcalls):
            xt = gpool.tile([P, J, GB], bf16, name="xt", tag="xt", bufs=n_gcalls)
            nc.gpsimd.dma_gather(
                xt[:],
                embed[:],
                tok_sb[:, b * (GB // 16) : (b + 1) * (GB // 16)],
                GB,
                GB,
                DIM,
                transpose=True,
                queue_num=(b % 2),
            )
            xts.append(xt)

        def xtv(b):
            # child-sliced view of call b: [P, J, 8, QB]
            return xts[b][:].rearrange("p j (c q) -> p j c q", q=QB)

        # identity comes in via DMA so no gpsimd work sits ahead of the
        # gather descriptor generation
        ident = const.tile([P, P], bf16)
        nc.sync.dma_start(ident[:], identity[:])

        # ---- constants / weights ----
        wih_sb = const.tile([P, J, 9, P], bf16)
        wih_s_sb = const.tile([P, J, 9, P], bf16)
        whh_sb = const.tile([P, J, 9, P], bf16)
        bias_sb = const.tile([P, 12], f32)
        bias3_sb = const.tile([3, 4, P], bf16)
        bias1_sb = const.tile([1, 9, P], bf16)
        onehot3_sb = const.tile([3, 3, 512], bf16)
        ones_sb = const.tile([1, 512], bf16)
        nc.sync.dma_start(wih_sb[:], wih_t[:])
        nc.sync.dma_start(wih_s_sb[:], wih_s[:])
        nc.sync.dma_start(whh_sb[:], whh_t[:])
        nc.sync.dma_start(bias_sb[:], biases[:])
        nc.sync.dma_start(bias3_sb[:], biases_mm[:])
        nc.sync.dma_start(bias1_sb[:], bias1[:])
        nc.sync.dma_start(onehot3_sb[:], onehot3[:])
        nc.sync.dma_start(ones_sb[:], ones[:])

        # child-major x per level: [P, J, 8, Pl] (contiguous matmul rhs)
        x_in = {}
        for Pl in levels[1:]:
            x_in[Pl] = xpool.tile([P, J, ARITY, Pl], bf16, name=f"x{Pl}", tag=f"x{Pl}")

        # gi stored child-major [P, 9, 8, Pl]: the gi pass rhs is the
        # child-major x (contiguous), so PSUM comes out (c, q)-ordered
        gi_tiles = {}
        for Pl in levels[1:]:
            gi_tiles[Pl] = xpool.tile([P, 9, ARITY, Pl], bf16, name=f"gi{Pl}",
                                      tag=f"gi{Pl}")

        def psum_tile():
            return pspool.tile([P, 512], f32, name="ps", tag="ps")

        def warm(n):
            # PE warm-keeper: HAM re-throttles the PE clock to 1.2 GHz after
            # ~3.4us of idle; during the latency-bound small levels, issue
            # dependency-free matmuls so the gate-chain windows don't cool
            # the PE and the next real matmul burst runs at 2.4 GHz.
            for _ in range(n):
                wp = pspool.tile([P, 512], f32, name="warm", tag="ps")
                nc.tensor.matmul(
                    wp[:, :512], ident[:], wih_sb[:, 0, 0:4, :], start=True, stop=True
                )

        def new_state(name, dtype, Pl):
            return state.tile([P, J, Pl], dtype, name=f"{name}{Pl}", tag=f"{name}{Pl}")

        # =====================  LEAF LEVEL (Pl = P5)  =====================
        h = new_state("h", bf16, P5)
        # permuted (child-major) accumulator so the level-end x_next add is a
        # contiguous write into the next level's child-major x
        hacc = state.tile([P, J, ARITY, P5 // ARITY], f32, name="hacc5", tag="hacc5")

        level_csum = [None]

        NSC = P5 // 4  # 128 parents per t0 sub-chunk
        c0 = ARITY - 1  # first GRU input is the last child

        def emit_t0_sub(sc):
            # step 0: h=0, gi only. Biases are injected into PSUM via the
            # K=3 one-hot matmul (the single start=True per bank), so the
            # activations span all 3 m-tiles in one op.
            with nc.named_scope(f"leaf_t0s{sc}"):
                sl = slice(sc * NSC, (sc + 1) * NSC)
                N3 = 3 * NSC
                ps_r, ps_z, ps_in = psum_tile(), psum_tile(), psum_tile()

                def view3s(pst):
                    return pst[:, :N3].rearrange("p (j n) -> p j n", j=3)

                for pst, ro, moff in ((ps_r, 0, 0), (ps_z, 1, 3), (ps_in, 3, 6)):
                    nc.tensor.matmul(
                        pst[:, :N3], bias3_sb[:, ro, :], onehot3_sb[:, :, :NSC],
                        start=True, stop=False,
                    )
                    for m in range(3):
                        for k in range(J):
                            for b2 in range(2):
                                nc.tensor.matmul(
                                    pst[:, m * NSC + b2 * QB : m * NSC + (b2 + 1) * QB],
                                    wih_sb[:, k, moff + m, :],
                                    xtv(2 * sc + b2)[:, k, c0, :],
                                    start=False,
                                    stop=(m == 2 and k == 2 and b2 == 1),
                                )
                r_sb = gates.tile([P, J, NSC], bf16, name="r0", tag="r0")
                z_sb = gates.tile([P, J, NSC], bf16, name="z0", tag="z0")
                n_sb = gates.tile([P, J, NSC], bf16, name="n0", tag="n0")
                rhn = gates.tile([P, J, NSC], f32, name="rhn0", tag="rhn0")
                t1 = gates.tile([P, J, NSC], bf16, name="t10", tag="t10")
                nc.scalar.activation(r_sb[:], view3s(ps_r), Sig)
                nc.scalar.activation(z_sb[:], view3s(ps_z), Sig)
                # n = tanh(i_n + b_in + r*b_hn): gh_n of the zero state is
                # just b_hn, folded in per m via the scalar port
                for m in range(3):
                    nc.vector.scalar_tensor_tensor(
                        out=rhn[:, m],
                        in0=r_sb[:, m],
                        scalar=bias_sb[:, 6 + m : 7 + m],
                        in1=view3s(ps_in)[:, m],
                        op0=Mult,
                        op1=Add,
                    )
                nc.scalar.activation(n_sb[:], rhn[:], Tanh)
                # h0=0: h' = n - z*n
                hsl = h[:, :, sl]
                nc.vector.tensor_tensor(out=t1[:], in0=z_sb[:], in1=n_sb[:], op=Mult)
                nc.vector.tensor_tensor(out=hsl, in0=n_sb[:], in1=t1[:], op=Sub)
                nc.gpsimd.tensor_copy(
                    out=hacc[:, :, :, sc * (NSC // ARITY) : (sc + 1) * (NSC // ARITY)],
                    in_=hsl.rearrange("p j (q c) -> p j c q", c=ARITY),
                )

        NCH = 256

        def emit_step(t, ch):
            c = ARITY - 1 - t
            with nc.named_scope(f"leaf_t{t}c{ch}"):
                sl = slice(ch * NCH, (ch + 1) * NCH)
                ps_r = [psum_tile()[:, :NCH] for _ in range(3)]
                ps_z = [psum_tile()[:, :NCH] for _ in range(3)]
                ps_in = [psum_tile()[:, :NCH] for _ in range(3)]
                ps_hn = [psum_tile()[:, :NCH] for _ in range(3)]
                for ps, moff in ((ps_r, 0), (ps_z, 3), (ps_in, 6)):
                    for m in range(3):
                        for k in range(J):
                            for b4 in range(4):
                                nc.tensor.matmul(
                                    ps[m][:, b4 * QB : (b4 + 1) * QB],
                                    wih_sb[:, k, moff + m, :],
                                    xtv(4 * ch + b4)[:, k, c, :],
                                    start=(k == 0 and b4 == 0),
                                    stop=(k == 2 and moff == 6 and b4 == 3),
                                )
                for ps, moff in ((ps_r, 0), (ps_z, 3), (ps_hn, 6)):
                    for m in range(3):
                        for k in range(J):
                            nc.tensor.matmul(
                                ps[m],
                                whh_sb[:, k, moff + m, :],
                                h[:, k, sl],
                                start=(k == 0 and moff == 6),
                                stop=(k == 2),
                            )

                r_sb = gates.tile([P, J, NCH], bf16, name="r_sb", tag="r_sb")
                z_sb = gates.tile([P, J, NCH], bf16, name="z_sb", tag="z_sb")
                n_sb = gates.tile([P, J, NCH], bf16, name="n_sb", tag="n_sb")
                rhn = gates.tile([P, J, NCH], f32, name="rhn", tag="rhn")
                t1 = gates.tile([P, J, NCH], bf16, name="t1", tag="t1")

                for m in range(3):
                    nc.scalar.activation(
                        r_sb[:, m], ps_r[m], Sig, bias=bias_sb[:, m : m + 1]
                    )
                for m in range(3):
                    nc.scalar.activation(
                        z_sb[:, m], ps_z[m], Sig, bias=bias_sb[:, 3 + m : 4 + m]
                    )
                for m in range(3):
                    nc.vector.scalar_tensor_tensor(
                        out=rhn[:, m],
                        in0=ps_hn[m],
                        scalar=bias_sb[:, 6 + m : 7 + m],
                        in1=r_sb[:, m],
                        op0=Add,
                        op1=Mult,
                    )
                for m in range(3):
                    nc.vector.tensor_tensor(
                        out=rhn[:, m], in0=rhn[:, m], in1=ps_in[m], op=Add
                    )
                for m in range(3):
                    nc.scalar.activation(
                        n_sb[:, m], rhn[:, m], Tanh, bias=bias_sb[:, 9 + m : 10 + m]
                    )

                # h' = n + z*(h - n)
                hsl = h[:, :, sl]
                nc.vector.tensor_tensor(out=t1[:], in0=hsl, in1=n_sb[:], op=Sub)
                nc.vector.tensor_tensor(out=t1[:], in0=z_sb[:], in1=t1[:], op=Mult)
                nc.vector.tensor_tensor(out=hsl, in0=n_sb[:], in1=t1[:], op=Add)
                hperm = hsl.rearrange("p j (q c) -> p j c q", c=ARITY)
                qsl = slice(ch * NCH // ARITY, (ch + 1) * NCH // ARITY)
                if t == ARITY - 1:
                    if ch == 0:
                        csum = state.tile(
                            [P, J, P5 // ARITY], f32, name="csum5", tag="csum5"
                        )
                        level_csum[0] = csum
                    nc.vector.tensor_reduce(
                        out=level_csum[0][:, :, qsl],
                        in_=hsl.rearrange("p j (q c) -> p j q c", c=ARITY),
                        axis=mybir.AxisListType.X,
                        op=Add,
                    )
                    xn = x_in[P5 // ARITY]
                    for j in range(J):
                        eng = nc.gpsimd if j == 2 else nc.vector
                        eng.tensor_tensor(
                            out=xn[:, j, :, qsl],
                            in0=hacc[:, j, :, qsl],
                            in1=hperm[:, j],
                            op=Add,
                        )
                    # bridge the level-end tail so the PE stays warm into
                    # the gi_64 pass
                    warm(8)
                else:
                    nc.gpsimd.tensor_tensor(
                        out=hacc[:, :, :, qsl],
                        in0=hacc[:, :, :, qsl],
                        in1=hperm,
                        op=Add,
                    )

        def emit_gi64_half(half):
            # gi pass for half the level-64 parents, overlapped with the
            # other leaf chunk's trailing steps
            Pl = P5 // ARITY
            with nc.named_scope(f"gi64h{half}"):
                gi_sb = gi_tiles[Pl]
                hQ = Pl // 2
                for m in (0, 1, 2, 3, 4, 5, 6, 7, 8):
                    ps = psum_tile()[:, : ARITY * hQ]
                    nc.tensor.matmul(
                        ps, bias1_sb[:, m, :], ones_sb[:, : ARITY * hQ],
                        start=True, stop=False,
                    )
                    for k in range(J):
                        nc.tensor.matmul(
                            ps,
                            wih_s_sb[:, k, m, :],
                            x_in[Pl][:, k, :, half * hQ : (half + 1) * hQ],
                            start=False,
                            stop=(k == 2),
                        )
                    eng = nc.vector if m % 2 == 0 else nc.scalar
                    psv = ps.rearrange("p (c q) -> p c q", q=hQ)
                    if m % 2 == 0:
                        nc.vector.tensor_copy(
                            out=gi_sb[:, m, :, half * hQ : (half + 1) * hQ], in_=psv
                        )
                    else:
                        nc.scalar.copy(
                            out=gi_sb[:, m, :, half * hQ : (half + 1) * hQ], in_=psv
                        )

        # Interleaved emission: engine FIFOs are in-order, so each piece is
        # queued roughly at its data-arrival time -- t0 sub-chunks track the
        # gathers, chunk B's steps slot between chunk A's from t=1 on.
        emit_t0_sub(0)
        emit_t0_sub(1)
        emit_t0_sub(2)
        emit_step(1, 0)
        emit_t0_sub(3)
        emit_step(2, 0)
        for t in range(3, ARITY):
            emit_step(t - 2, 1)
            emit_step(t, 0)
        emit_gi64_half(0)
        emit_step(ARITY - 2, 1)
        emit_step(ARITY - 1, 1)
        emit_gi64_half(1)

        # =====================  SMALL LEVELS (64, 8, 1)  ==================
        for Pl in levels[1:]:
            NC8 = ARITY * Pl  # children count = gi batch size
            gi_sb = gi_tiles[Pl]
            if Pl != P5 // ARITY:
              with nc.named_scope(f"gi_{Pl}"):
                # m-order: r (0,1,2) first so step 0's r-inject unblocks
                # early, then z (3,4,5), then n (6,7,8)
                for mi, m in enumerate((0, 1, 2, 3, 4, 5, 6, 7, 8)):
                    ps = psum_tile()[:, :NC8]
                    nc.tensor.matmul(
                        ps, bias1_sb[:, m, :], ones_sb[:, :NC8],
                        start=True, stop=False,
                    )
                    for k in range(J):
                        nc.tensor.matmul(
                            ps,
                            wih_s_sb[:, k, m, :],
                            x_in[Pl][:, k, :, :],
                            start=False,
                            stop=(k == 2),
                        )
                    # alternate copy engine so the PSUM->SBUF drain keeps up
                    # with the matmul waves
                    if mi % 2 == 0:
                        nc.vector.tensor_copy(
                            out=gi_sb[:, m].rearrange("p c q -> p (c q)"), in_=ps
                        )
                    else:
                        nc.scalar.copy(
                            out=gi_sb[:, m].rearrange("p c q -> p (c q)"), in_=ps
                        )

            csum = level_csum[0]
            h = new_state("h", bf16, Pl)
            hacc = new_state("hacc", f32, Pl)
            nc.scalar.mul(h[:], csum[:], 1.0 / ARITY)

            for t in range(ARITY):
                c = ARITY - 1 - t
                with nc.named_scope(f"lv{Pl}_t{t}"):
                    N3 = 3 * Pl
                    ps_z, ps_r, ps_hn = psum_tile(), psum_tile(), psum_tile()

                    def view3(pst):
                        return pst[:, :N3].rearrange("p (j n) -> p j n", j=3)

                    def msl(pst, m):
                        return pst[:, m * Pl : (m + 1) * Pl]

                    # r first: sigma(r) heads the serial chain, so its PSUM
                    # group must close first and nothing may sit ahead of
                    # sigma(r) in the scalar FIFO
                    nc.tensor.matmul(
                        ps_r[:, :N3], ident[:], gi_sb[:, 0:3, c, :],
                        start=True, stop=False,
                    )
                    for m in range(3):
                        for k in range(J):
                            nc.tensor.matmul(
                                msl(ps_r, m), whh_sb[:, k, m, :], h[:, k, :],
                                start=False, stop=(m == 2 and k == 2),
                            )
                    # hn second so its accumulation closes before z's: the
                    # serial path is sigma(r) -> rhn = ps_hn*r -> tanh
                    nc.tensor.matmul(
                        ps_hn[:, :N3], bias3_sb[:, 2, :], onehot3_sb[:, :, :Pl],
                        start=True, stop=False,
                    )
                    for m in range(3):
                        for k in range(J):
                            nc.tensor.matmul(
                                msl(ps_hn, m), whh_sb[:, k, 6 + m, :], h[:, k, :],
                                start=False, stop=(m == 2 and k == 2),
                            )
                    # z last (sigma(z)/zc/t2 have slack until the h update)
                    nc.tensor.matmul(
                        ps_z[:, :N3], ident[:], gi_sb[:, 3:6, c, :],
                        start=True, stop=False,
                    )
                    for m in range(3):
                        for k in range(J):
                            nc.tensor.matmul(
                                msl(ps_z, m), whh_sb[:, k, 3 + m, :], h[:, k, :],
                                start=False, stop=(m == 2 and k == 2),
                            )
                    if Pl == 64:
                        warm(6)

                    z_sb = gates.tile([P, J, Pl], bf16, name="z_sb", tag="z_sb")
                    zc_sb = gates.tile([P, J, Pl], bf16, name="zc_sb", tag="zc_sb")
                    r_sb = gates.tile([P, J, Pl], bf16, name="r_sb", tag="r_sb")
                    n_sb = gates.tile([P, J, Pl], bf16, name="n_sb", tag="n_sb")
                    rhn = gates.tile([P, J, Pl], f32, name="rhn", tag="rhn")
                    t1 = gates.tile([P, J, Pl], f32, name="t1", tag="t1")
                    t2 = gates.tile([P, J, Pl], f32, name="t2", tag="t2")

                    nc.scalar.activation(r_sb[:], view3(ps_r), Sig)
                    nc.scalar.activation(z_sb[:], view3(ps_z), Sig)
                    nc.scalar.activation(zc_sb[:], view3(ps_z), Sig, scale=-1.0)
                    # t2 = z*h off-path while r/n compute
                    nc.gpsimd.tensor_tensor(out=t2[:], in0=z_sb[:], in1=h[:], op=Mult)
                    nc.vector.tensor_tensor(
                        out=rhn[:], in0=view3(ps_hn), in1=r_sb[:], op=Mult
                    )
                    nc.vector.tensor_tensor(
                        out=rhn[:], in0=rhn[:], in1=gi_sb[:, 6:9, c, :], op=Add
                    )
                    nc.scalar.activation(n_sb[:], rhn[:], Tanh)
                    # h' = zc*n + z*h
                    nc.vector.tensor_tensor(out=t1[:], in0=zc_sb[:], in1=n_sb[:], op=Mult)
                    nc.vector.tensor_tensor(out=h[:], in0=t1[:], in1=t2[:], op=Add)

                    if t == 0:
                        nc.gpsimd.tensor_copy(out=hacc[:], in_=h[:])
                    elif t == ARITY - 1 and Pl > 1:
                        csum = state.tile(
                            [P, J, Pl // ARITY], f32, name=f"csum{Pl}", tag=f"csum{Pl}"
                        )
                        level_csum[0] = csum
                        nc.vector.tensor_reduce(
                            out=csum[:],
                            in_=h[:].rearrange("p j (q c) -> p j q c", c=ARITY),
                            axis=mybir.AxisListType.X,
                            op=Add,
                        )
                        xn = x_in[Pl // ARITY]
                        nc.vector.tensor_tensor(
                            out=xn[:],
                            in0=hacc[:].rearrange("p j (q c) -> p j c q", c=ARITY),
                            in1=h[:].rearrange("p j (q c) -> p j c q", c=ARITY),
                            op=Add,
                        )
                    else:
                        nc.gpsimd.tensor_tensor(
                            out=hacc[:], in0=hacc[:], in1=h[:], op=Add
                        )

        # ---- outputs: [P, 2, J] = (x_root, h_root) ----
        out_t = state.tile([P, 2, J], f32, name="out_t", tag="out_t")
        nc.scalar.mul(out_t[:, 0], hacc[:, :, 0], 1.0 / ARITY)
        nc.vector.tensor_copy(out=out_t[:, 1], in_=h[:, :, 0])
        nc.sync.dma_start(out_xh[:], out_t[:])


def _build_program(n_leaves_core):
    if n_leaves_core in _PROG_CACHE:
        return _PROG_CACHE[n_leaves_core]
    import concourse.bacc as bacc
    import concourse.mybir as mybir
    import concourse.tile as tile

    f32 = mybir.dt.float32
    bf16 = mybir.dt.bfloat16

    nc = bacc.Bacc(
        "TRN2",
        target_bir_lowering=False,
        debug=False,
        enable_asserts=False,
        num_devices=NCORES,
        num_swdge_queues=4,
    )
    tokens = nc.dram_tensor(
        "tokens", [P, n_leaves_core // 16], mybir.dt.int16, kind="ExternalInput"
    ).ap()
    embed = nc.dram_tensor("embed", [VOCAB, DIM], bf16, kind="ExternalInput").ap()
    wih_t = nc.dram_tensor("wih_t", [P, J, 9, P], bf16, kind="ExternalInput").ap()
    wih_s = nc.dram_tensor("wih_s", [P, J, 9, P], bf16, kind="ExternalInput").ap()
    whh_t = nc.dram_tensor("whh_t", [P, J, 9, P], bf16, kind="ExternalInput").ap()
    biases = nc.dram_tensor("biases", [P, 12], f32, kind="ExternalInput").ap()
    biases_mm = nc.dram_tensor("biases_mm", [3, 4, P], bf16, kind="ExternalInput").ap()
    bias1 = nc.dram_tensor("bias1", [1, 9, P], bf16, kind="ExternalInput").ap()
    onehot3 = nc.dram_tensor("onehot3", [3, 3, 512], bf16, kind="ExternalInput").ap()
    ones = nc.dram_tensor("ones", [1, 512], bf16, kind="ExternalInput").ap()
    identity = nc.dram_tensor("identity", [P, P], bf16, kind="ExternalInput").ap()
    out_xh = nc.dram_tensor("out_xh", [P, 2, J], f32, kind="ExternalOutput").ap()

    with tile.TileContext(nc) as tc:
        _emit(
            tc,
            nc,
            (tokens, embed, wih_t, wih_s, whh_t, biases, biases_mm, bias1,
             onehot3, ones, identity, out_xh),
            n_leaves_core,
        )
    nc.compile()
    _PROG_CACHE[n_leaves_core] = nc
    return nc


def _retile_weights(w):
    # w: [1152, 384] -> lhsT tiles [128(k_part), 3(k), 9(m), 128(m_col)] bf16
    wt = np.ascontiguousarray(w.T)  # [384, 1152]
    wt = wt.reshape(J, P, 9, P).transpose(1, 0, 2, 3)
    return np.ascontiguousarray(wt).astype(BF16)


def _prep_bias(b_ih, b_hh):
    biases = np.zeros((P, 12), np.float32)
    comb = (b_ih + b_hh).reshape(9, P)
    biases[:, 0:6] = comb[0:6].T
    biases[:, 6:9] = b_hh.reshape(9, P)[6:9].T
    biases[:, 9:12] = b_ih.reshape(9, P)[6:9].T
    return biases


def _prep_bias_mm(b_ih, b_hh):
    # lhsT[k, ro, q] = bias[q, 3*ro + k]: the K=3 bias matmul against the
    # one-hot rhs yields out[q, (j, n)] = bias[q, 3*ro + j].
    b = _prep_bias(b_ih, b_hh)  # [128, 12] cols: r0..2 z0..2 hn0..2 in0..2
    out = b.T.reshape(4, 3, P).transpose(1, 0, 2)
    return np.ascontiguousarray(out).astype(BF16)


def _prep_bias1(b_ih, b_hh):
    # K=1 lhsT for the gi-precompute bias: out[col, :] += bias1[0, m, col].
    # r/z rows carry the combined input+hidden bias; n rows carry b_in only.
    out = np.zeros((1, 9, P), np.float32)
    comb = (b_ih + b_hh).reshape(9, P)
    out[0, 0:6] = comb[0:6]
    out[0, 6:9] = b_ih.reshape(9, P)[6:9]
    return out.astype(BF16)


def _wrap_tokens(tok):
    # dma_gather idx layout: idx i of a 512-row call lives at partition i%16,
    # column i//16, replicated across the 8 Q7 core partition-groups. Each
    # call's 512 indices are pre-permuted child-major (position c*64+q is
    # leaf 8q+c), so the transposing gather writes the x tile directly in
    # [P, J, 8 child, 64 parent] order.
    arr = np.zeros((P, len(tok) // 16), np.int16)
    for b in range(len(tok) // 512):
        blk = tok[b * 512 : (b + 1) * 512].astype(np.int16)
        cm = np.ascontiguousarray(blk.reshape(64, 8).T).reshape(-1)
        m = cm.reshape(32, 16).T  # [16, 32]
        arr[:, b * 32 : (b + 1) * 32] = np.tile(m, (8, 1))
    return arr


def _prep_onehot3():
    out = np.zeros((3, 3, 512), np.float32)
    for k in range(3):
        out[k, k, :] = 1.0
    return out.astype(BF16)


def _gru_gates(x_t, h, w_ih, w_hh, b_ih, b_hh):
    gi = x_t @ w_ih.T + b_ih
    gh = h @ w_hh.T + b_hh
    i_r, i_z, i_n = np.split(gi, 3, axis=-1)
    h_r, h_z, h_n = np.split(gh, 3, axis=-1)
    r = 1.0 / (1.0 + np.exp(-(i_r + h_r)))
    z = 1.0 / (1.0 + np.exp(-(i_z + h_z)))
    n = np.tanh(i_n + r * h_n)
    return (1.0 - z) * n + z * h


def _root_gru(x_children, h0, w_ih, w_hh, b_ih, b_hh):
    h = h0.astype(np.float64)
    acc = np.zeros_like(h)
    for t in range(ARITY):
        x_t = x_children[ARITY - 1 - t].astype(np.float64)
        h = _gru_gates(x_t, h, w_ih.astype(np.float64), w_hh.astype(np.float64),
                       b_ih.astype(np.float64), b_hh.astype(np.float64))
        acc += h
    return (acc / ARITY).astype(np.float32)


def kernel(leaf_tokens, embed_table, w_ih, w_hh, b_ih, b_hh):
    from concourse.bass_utils import run_bass_kernel_spmd

    leaf_tokens = np.asarray(leaf_tokens, np.int32)
    embed_table = np.asarray(embed_table, np.float32)
    w_ih = np.asarray(w_ih, np.float32)
    w_hh = np.asarray(w_hh, np.float32)
    b_ih = np.asarray(b_ih, np.float32)
    b_hh = np.asarray(b_hh, np.float32)

    nc = _build_program(LEAVES_CORE)

    embed_bf = embed_table.astype(BF16)
    wih_t = _retile_weights(w_ih)
    wih_s = _retile_weights(w_ih / ARITY)
    whh_t = _retile_weights(w_hh)
    biases = _prep_bias(b_ih, b_hh)
    biases_mm = _prep_bias_mm(b_ih, b_hh)
    bias1 = _prep_bias1(b_ih, b_hh)
    ones = np.ones((1, 512), np.float32).astype(BF16)
    in_maps = []
    for core in range(NCORES):
        in_maps.append(
            {
                "tokens": _wrap_tokens(
                    leaf_tokens[core * LEAVES_CORE : (core + 1) * LEAVES_CORE]
                ),
                "embed": embed_bf,
                "wih_t": wih_t,
                "wih_s": wih_s,
                "whh_t": whh_t,
                "biases": biases,
                "biases_mm": biases_mm,
                "bias1": bias1,
                "onehot3": _prep_onehot3(),
                "ones": ones,
                "identity": np.eye(P, dtype=np.float32).astype(BF16),
            }
        )
    res = run_bass_kernel_spmd(nc, in_maps, core_ids=list(range(NCORES)))

    xs = np.zeros((NCORES, DIM), np.float32)
    h8 = np.zeros((NCORES, DIM), np.float32)
    for core in range(NCORES):
        out = res.results[core]["out_xh"]  # [P, 2, J]
        xs[core] = out[:, 0].T.reshape(-1)
        h8[core] = out[:, 1].T.reshape(-1)

    h0 = h8.mean(axis=0)
    out = _root_gru(xs, h0, w_ih, w_hh, b_ih, b_hh)
    return out.reshape(1, 1, DIM)


# revision 37
# speedup vs baseline: 1.2607x; 1.0177x over previous
"""Tree-GRU (arity-8, depth-5) over embedded leaves on 8 TRN2 NeuronCores.

Sharding: data-parallel over subtrees. Each core takes 4096 contiguous leaves
and runs levels 5..2 of the tree locally (512 -> 64 -> 8 -> 1 parents). The
root (level 1, 8 children = the 8 cores' level-2 outputs) is a trivial
16-matvec GRU done on host after gathering the per-core [384] outputs.

Device layout is feature-transposed: tensors live as [128 part, 3 ktile, ...]
with feature f = 128*k + p, so the GRU matmuls contract the partition dim.
Node storage is child-major, so matmul rhs slices are contiguous. Weights
are host-pre-transposed into lhsT tiles; matmul dtype bf16 with fp32 PSUM
accumulation.

Leaf level: the embedding gather uses the transposing dma_gather ucode
(int16 index lists, 512 rows per call, 2 SWDGE queues) with the indices
pre-permuted child-major on host, so each call lands directly as a
feature-transposed [P, J, 8 child, 64 parent] x tile -- no PE transposes or
PSUM->SBUF copies. Step 0 (h=0, gi-only) runs in 4 sub-chunks that track
gather completion; steps 1-7 ping-pong 2 chunks of 256 parents with the
emission order interleaved so the in-order engine FIFOs receive each piece
near its data-arrival time.

Small levels (64/8/1 parents): the input transform gi for the whole level
(all 8 children x all parents) is precomputed in one batched matmul pass
(biases folded in via a K=1 ones-column matmul) and stored in SBUF, so the
sequential per-step work is only the recurrent matmul + gate chain. Per step
the r/z gi rows are injected into PSUM via an identity matmul (one start=True
covering MM per bank), the hn bias via a K=3 one-hot matmul, and the h update
uses h' = zc*n + z*h with zc = sigmoid(-pre_z) so z*h is computed off-path
(gpsimd) while tanh runs. The 1/8 output-mean scale is folded into a
pre-scaled copy of W_ih used by the gi passes; per-step output sums are
accumulated (hacc) and the final step writes the next level's input directly
as a fused raw-sum add.
"""

import numpy as np
import ml_dtypes

ARITY = 8
DIM = 384
VOCAB = 32000
NCORES = 8
P = 128
J = 3  # DIM // 128 feature tiles
N_LEAVES = 32768
LEAVES_CORE = N_LEAVES // NCORES  # 4096

BF16 = ml_dtypes.bfloat16

_PROG_CACHE = {}


def _levels_for(n_leaves_core):
    levels = []
    p = n_leaves_core // ARITY
    while p >= 1:
        levels.append(p)
        p //= ARITY
    assert levels[-1] == 1
    return levels


def _emit(tc, nc, aps, n_leaves_core):
    import concourse.mybir as mybir
    import concourse.bass as bass

    f32 = mybir.dt.float32
    bf16 = mybir.dt.bfloat16
    Sig = mybir.ActivationFunctionType.Sigmoid
    Tanh = mybir.ActivationFunctionType.Tanh
    Add = mybir.AluOpType.add
    Sub = mybir.AluOpType.subtract
    Mult = mybir.AluOpType.mult

    (tokens, embed, wih_t, wih_s, whh_t, biases, biases_mm, bias1, onehot3,
     ones, identity, out_xh) = aps
    levels = _levels_for(n_leaves_core)
    P5 = levels[0]
    n_gtiles = n_leaves_core // P  # 32

    from contextlib import ExitStack

    with ExitStack() as ctx:
        const = ctx.enter_context(tc.tile_pool(name="const", bufs=1))
        xpool = ctx.enter_context(tc.tile_pool(name="xpool", bufs=1))
        state = ctx.enter_context(tc.tile_pool(name="state", bufs=1))
        gates = ctx.enter_context(tc.tile_pool(name="gates", bufs=6))
        gpool = ctx.enter_context(tc.tile_pool(name="gpool", bufs=1))
        pspool = ctx.enter_context(tc.tile_pool(name="pspool", bufs=8, space="PSUM"))

        # ---- tokens first, then kick off all gathers (dma_gather ucode,
        # int16 idx lists, 512 rows per call; 2 SWDGE queues). The gathers
        # are paced by gpsimd descriptor generation, so nothing else may sit
        # ahead of them in the gpsimd FIFO -- the identity build comes after.
        from concourse.library_config import mlp as _mlp_lib

        GB = 512  # rows per dma_gather call (64 parents, child-major)
        n_gcalls = n_leaves_core // GB  # 8
        QB = GB // ARITY  # 64 parents per call tile
        tok_sb = const.tile([P, n_gcalls * (GB // 16)], mybir.dt.int16)
        nc.sync.dma_start(tok_sb[:], tokens[:])
        nc.gpsimd.load_library(_mlp_lib)

        # transpose=True lands each embedding row across partitions with
        # feature f = 128*j + p -- exactly the matmul lhsT layout -- and the
        # host orders each call's 512 indices child-major, so the tile is
        # directly the [P, J, 8, 64] x input. No PE transposes, no copies.
        xts = []
        for b in range(n_gcalls):
            xt = gpool.tile([P, J, GB], bf16, name="xt", tag="xt", bufs=n_gcalls)
            nc.gpsimd.dma_gather(
                xt[:],
                embed[:],
                tok_sb[:, b * (GB // 16) : (b + 1) * (GB // 16)],
                GB,
                GB,
                DIM,
                transpose=True,
                queue_num=(b % 2),
            )
            xts.append(xt)

        def xtv(b):
            # child-sliced view of call b: [P, J, 8, QB]
            return xts[b][:].rearrange("p j (c q) -> p j c q", q=QB)

        # identity comes in via DMA so no gpsimd work sits ahead of the
        # gather descriptor generation
        ident = const.tile([P, P], bf16)
        nc.sync.dma_start(ident[:], identity[:])

        # ---- constants / weights ----
        wih_sb = const.tile([P, J, 9, P], bf16)
        wih_s_sb = const.tile([P, J, 9, P], bf16)
        whh_sb = const.tile([P, J, 9, P], bf16)
        bias_sb = const.tile([P, 12], f32)
        bias3_sb = const.tile([3, 4, P], bf16)
        bias1_sb = const.tile([1, 9, P], bf16)
        onehot3_sb = const.tile([3, 3, 512], bf16)
        ones_sb = const.tile([1, 512], bf16)
        nc.sync.dma_start(wih_sb[:], wih_t[:])
        nc.sync.dma_start(wih_s_sb[:], wih_s[:])
        nc.sync.dma_start(whh_sb[:], whh_t[:])
        nc.sync.dma_start(bias_sb[:], biases[:])
        nc.sync.dma_start(bias3_sb[:], biases_mm[:])
        nc.sync.dma_start(bias1_sb[:], bias1[:])
        nc.sync.dma_start(onehot3_sb[:], onehot3[:])
        nc.sync.dma_start(ones_sb[:], ones[:])

        # child-major x per level: [P, J, 8, Pl] (contiguous matmul rhs)
        x_in = {}
        for Pl in levels[1:]:
            x_in[Pl] = xpool.tile([P, J, ARITY, Pl], bf16, name=f"x{Pl}", tag=f"x{Pl}")

        # gi stored child-major [P, 9, 8, Pl]: the gi pass rhs is the
        # child-major x (contiguous), so PSUM comes out (c, q)-ordered
        gi_tiles = {}
        for Pl in levels[1:]:
            gi_tiles[Pl] = xpool.tile([P, 9, ARITY, Pl], bf16, name=f"gi{Pl}",
                                      tag=f"gi{Pl}")

        def psum_tile():
            return pspool.tile([P, 512], f32, name="ps", tag="ps")

        def warm(n):
            # PE warm-keeper: HAM re-throttles the PE clock to 1.2 GHz after
            # ~3.4us of idle; during the latency-bound small levels, issue
            # dependency-free matmuls so the gate-chain windows don't cool
            # the PE and the next real matmul burst runs at 2.4 GHz.
            for _ in range(n):
                wp = pspool.tile([P, 512], f32, name="warm", tag="ps")
                nc.tensor.matmul(
                    wp[:, :512], ident[:], wih_sb[:, 0, 0:4, :], start=True, stop=True
                )

        def new_state(name, dtype, Pl):
            return state.tile([P, J, Pl], dtype, name=f"{name}{Pl}", tag=f"{name}{Pl}")

        # =====================  LEAF LEVEL (Pl = P5)  =====================
        h = new_state("h", bf16, P5)
        # permuted (child-major) accumulator so the level-end x_next add is a
        # contiguous write into the next level's child-major x
        hacc = state.tile([P, J, ARITY, P5 // ARITY], f32, name="hacc5", tag="hacc5")

        level_csum = [None]

        NSC = P5 // 4  # 128 parents per t0 sub-chunk
        c0 = ARITY - 1  # first GRU input is the last child

        def emit_t0_sub(sc):
            # step 0: h=0, gi only. Biases are injected into PSUM via the
            # K=3 one-hot matmul (the single start=True per bank), so the
            # activations span all 3 m-tiles in one op.
            with nc.named_scope(f"leaf_t0s{sc}"):
                sl = slice(sc * NSC, (sc + 1) * NSC)
                N3 = 3 * NSC
                ps_r, ps_z, ps_in = psum_tile(), psum_tile(), psum_tile()

                def view3s(pst):
                    return pst[:, :N3].rearrange("p (j n) -> p j n", j=3)

                for pst, ro, moff in ((ps_r, 0, 0), (ps_z, 1, 3), (ps_in, 3, 6)):
                    nc.tensor.matmul(
                        pst[:, :N3], bias3_sb[:, ro, :], onehot3_sb[:, :, :NSC],
                        start=True, stop=False,
                    )
                    for m in range(3):
                        for k in range(J):
                            for b2 in range(2):
                                nc.tensor.matmul(
                                    pst[:, m * NSC + b2 * QB : m * NSC + (b2 + 1) * QB],
                                    wih_sb[:, k, moff + m, :],
                                    xtv(2 * sc + b2)[:, k, c0, :],
                                    start=False,
                                    stop=(m == 2 and k == 2 and b2 == 1),
                                )
                r_sb = gates.tile([P, J, NSC], bf16, name="r0", tag="r0")
                z_sb = gates.tile([P, J, NSC], bf16, name="z0", tag="z0")
                n_sb = gates.tile([P, J, NSC], bf16, name="n0", tag="n0")
                rhn = gates.tile([P, J, NSC], f32, name="rhn0", tag="rhn0")
                t1 = gates.tile([P, J, NSC], bf16, name="t10", tag="t10")
                nc.scalar.activation(r_sb[:], view3s(ps_r), Sig)
                nc.scalar.activation(z_sb[:], view3s(ps_z), Sig)
                # n = tanh(i_n + b_in + r*b_hn): gh_n of the zero state is
                # just b_hn, folded in per m via the scalar port
                for m in range(3):
                    nc.vector.scalar_tensor_tensor(
                        out=rhn[:, m],
                        in0=r_sb[:, m],
                        scalar=bias_sb[:, 6 + m : 7 + m],
                        in1=view3s(ps_in)[:, m],
                        op0=Mult,
                        op1=Add,
                    )
                nc.scalar.activation(n_sb[:], rhn[:], Tanh)
                # h0=0: h' = n - z*n
                hsl = h[:, :, sl]
                nc.vector.tensor_tensor(out=t1[:], in0=z_sb[:], in1=n_sb[:], op=Mult)
                nc.vector.tensor_tensor(out=hsl, in0=n_sb[:], in1=t1[:], op=Sub)
                nc.gpsimd.tensor_copy(
                    out=hacc[:, :, :, sc * (NSC // ARITY) : (sc + 1) * (NSC // ARITY)],
                    in_=hsl.rearrange("p j (q c) -> p j c q", c=ARITY),
                )

        NCH = 256

        def emit_step(t, ch):
            c = ARITY - 1 - t
            with nc.named_scope(f"leaf_t{t}c{ch}"):
                sl = slice(ch * NCH, (ch + 1) * NCH)
                ps_r = [psum_tile()[:, :NCH] for _ in range(3)]
                ps_z = [psum_tile()[:, :NCH] for _ in range(3)]
                ps_in = [psum_tile()[:, :NCH] for _ in range(3)]
                ps_hn = [psum_tile()[:, :NCH] for _ in range(3)]
                for ps, moff in ((ps_r, 0), (ps_z, 3), (ps_in, 6)):
                    for m in range(3):
                        for k in range(J):
                            for b4 in range(4):
                                nc.tensor.matmul(
                                    ps[m][:, b4 * QB : (b4 + 1) * QB],
                                    wih_sb[:, k, moff + m, :],
                                    xtv(4 * ch + b4)[:, k, c, :],
                                    start=(k == 0 and b4 == 0),
                                    stop=(k == 2 and moff == 6 and b4 == 3),
                                )
                for ps, moff in ((ps_r, 0), (ps_z, 3), (ps_hn, 6)):
                    for m in range(3):
                        for k in range(J):
                            nc.tensor.matmul(
                                ps[m],
                                whh_sb[:, k, moff + m, :],
                                h[:, k, sl],
                                start=(k == 0 and moff == 6),
                                stop=(k == 2),
                            )

                r_sb = gates.tile([P, J, NCH], bf16, name="r_sb", tag="r_sb")
                z_sb = gates.tile([P, J, NCH], bf16, name="z_sb", tag="z_sb")
                n_sb = gates.tile([P, J, NCH], bf16, name="n_sb", tag="n_sb")
                rhn = gates.tile([P, J, NCH], f32, name="rhn", tag="rhn")
                t1 = gates.tile([P, J, NCH], bf16, name="t1", tag="t1")

                for m in range(3):
                    nc.scalar.activation(
                        r_sb[:, m], ps_r[m], Sig, bias=bias_sb[:, m : m + 1]
                    )
                for m in range(3):
                    nc.scalar.activation(
                        z_sb[:, m], ps_z[m], Sig, bias=bias_sb[:, 3 + m : 4 + m]
                    )
                for m in range(3):
                    nc.vector.scalar_tensor_tensor(
                        out=rhn[:, m],
                        in0=ps_hn[m],
                        scalar=bias_sb[:, 6 + m : 7 + m],
                        in1=r_sb[:, m],
                        op0=Add,
                        op1=Mult,
                    )
                for m in range(3):
                    nc.vector.tensor_tensor(
                        out=rhn[:, m], in0=rhn[:, m], in1=ps_in[m], op=Add
                    )
                for m in range(3):
                    nc.scalar.activation(
                        n_sb[:, m], rhn[:, m], Tanh, bias=bias_sb[:, 9 + m : 10 + m]
                    )

                # h' = n + z*(h - n)
                hsl = h[:, :, sl]
                nc.vector.tensor_tensor(out=t1[:], in0=hsl, in1=n_sb[:], op=Sub)
                nc.vector.tensor_tensor(out=t1[:], in0=z_sb[:], in1=t1[:], op=Mult)
                nc.vector.tensor_tensor(out=hsl, in0=n_sb[:], in1=t1[:], op=Add)
                hperm = hsl.rearrange("p j (q c) -> p j c q", c=ARITY)
                qsl = slice(ch * NCH // ARITY, (ch + 1) * NCH // ARITY)
                if t == ARITY - 1:
                    if ch == 0:
                        csum = state.tile(
                            [P, J, P5 // ARITY], f32, name="csum5", tag="csum5"
                        )
                        level_csum[0] = csum
                    nc.vector.tensor_reduce(
                        out=level_csum[0][:, :, qsl],
                        in_=hsl.rearrange("p j (q c) -> p j q c", c=ARITY),
                        axis=mybir.AxisListType.X,
                        op=Add,
                    )
                    xn = x_in[P5 // ARITY]
                    for j in range(J):
                        eng = nc.gpsimd if j == 2 else nc.vector
                        eng.tensor_tensor(
                            out=xn[:, j, :, qsl],
                            in0=hacc[:, j, :, qsl],
                            in1=hperm[:, j],
                            op=Add,
                        )
                    # bridge the level-end tail so the PE stays warm into
                    # the gi_64 pass
                    warm(8)
                else:
                    nc.gpsimd.tensor_tensor(
                        out=hacc[:, :, :, qsl],
                        in0=hacc[:, :, :, qsl],
                        in1=hperm,
                        op=Add,
                    )

        def emit_gi64_half(half):
            # gi pass for half the level-64 parents, overlapped with the
            # other leaf chunk's trailing steps
            Pl = P5 // ARITY
            with nc.named_scope(f"gi64h{half}"):
                gi_sb = gi_tiles[Pl]
                hQ = Pl // 2
                for m in (0, 1, 2, 3, 4, 5, 6, 7, 8):
                    ps = psum_tile()[:, : ARITY * hQ]
                    nc.tensor.matmul(
                        ps, bias1_sb[:, m, :], ones_sb[:, : ARITY * hQ],
                        start=True, stop=False,
                    )
                    for k in range(J):
                        nc.tensor.matmul(
                            ps,
                            wih_s_sb[:, k, m, :],
                            x_in[Pl][:, k, :, half * hQ : (half + 1) * hQ],
                            start=False,
                            stop=(k == 2),
                        )
                    eng = nc.vector if m % 2 == 0 else nc.scalar
                    psv = ps.rearrange("p (c q) -> p c q", q=hQ)
                    if m % 2 == 0:
                        nc.vector.tensor_copy(
                            out=gi_sb[:, m, :, half * hQ : (half + 1) * hQ], in_=psv
                        )
                    else:
                        nc.scalar.copy(
                            out=gi_sb[:, m, :, half * hQ : (half + 1) * hQ], in_=psv
                        )

        # Interleaved emission: engine FIFOs are in-order, so each piece is
        # queued roughly at its data-arrival time -- t0 sub-chunks track the
        # gathers, chunk B's steps slot between chunk A's from t=1 on.
        emit_t0_sub(0)
        emit_t0_sub(1)
        emit_t0_sub(2)
        emit_step(1, 0)
        emit_t0_sub(3)
        emit_step(2, 0)
        for t in range(3, ARITY):
            emit_step(t - 2, 1)
            emit_step(t, 0)
        emit_step(ARITY - 2, 1)
        emit_gi64_half(0)
        emit_step(ARITY - 1, 1)
        emit_gi64_half(1)

        # =====================  SMALL LEVELS (64, 8, 1)  ==================
        for Pl in levels[1:]:
            NC8 = ARITY * Pl  # children count = gi batch size
            gi_sb = gi_tiles[Pl]
            if Pl != P5 // ARITY:
              with nc.named_scope(f"gi_{Pl}"):
                # m-order: r (0,1,2) first so step 0's r-inject unblocks
                # early, then z (3,4,5), then n (6,7,8)
                for mi, m in enumerate((0, 1, 2, 3, 4, 5, 6, 7, 8)):
                    ps = psum_tile()[:, :NC8]
                    nc.tensor.matmul(
                        ps, bias1_sb[:, m, :], ones_sb[:, :NC8],
                        start=True, stop=False,
                    )
                    for k in range(J):
                        nc.tensor.matmul(
                            ps,
                            wih_s_sb[:, k, m, :],
                            x_in[Pl][:, k, :, :],
                            start=False,
                            stop=(k == 2),
                        )
                    # alternate copy engine so the PSUM->SBUF drain keeps up
                    # with the matmul waves
                    if mi % 2 == 0:
                        nc.vector.tensor_copy(
                            out=gi_sb[:, m].rearrange("p c q -> p (c q)"), in_=ps
                        )
                    else:
                        nc.scalar.copy(
                            out=gi_sb[:, m].rearrange("p c q -> p (c q)"), in_=ps
                        )

            csum = level_csum[0]
            h = new_state("h", bf16, Pl)
            hacc = new_state("hacc", f32, Pl)
            nc.scalar.mul(h[:], csum[:], 1.0 / ARITY)

            for t in range(ARITY):
                c = ARITY - 1 - t
                with nc.named_scope(f"lv{Pl}_t{t}"):
                    N3 = 3 * Pl
                    ps_z, ps_r, ps_hn = psum_tile(), psum_tile(), psum_tile()

                    def view3(pst):
                        return pst[:, :N3].rearrange("p (j n) -> p j n", j=3)

                    def msl(pst, m):
                        return pst[:, m * Pl : (m + 1) * Pl]

                    # r first: sigma(r) heads the serial chain, so its PSUM
                    # group must close first and nothing may sit ahead of
                    # sigma(r) in the scalar FIFO
                    nc.tensor.matmul(
                        ps_r[:, :N3], ident[:], gi_sb[:, 0:3, c, :],
                        start=True, stop=False,
                    )
                    for m in range(3):
                        for k in range(J):
                            nc.tensor.matmul(
                                msl(ps_r, m), whh_sb[:, k, m, :], h[:, k, :],
                                start=False, stop=(m == 2 and k == 2),
                            )
                    # hn second so its accumulation closes before z's: the
                    # serial path is sigma(r) -> rhn = ps_hn*r -> tanh
                    nc.tensor.matmul(
                        ps_hn[:, :N3], bias3_sb[:, 2, :], onehot3_sb[:, :, :Pl],
                        start=True, stop=False,
                    )
                    for m in range(3):
                        for k in range(J):
                            nc.tensor.matmul(
                                msl(ps_hn, m), whh_sb[:, k, 6 + m, :], h[:, k, :],
                                start=False, stop=(m == 2 and k == 2),
                            )
                    # z last (sigma(z)/zc/t2 have slack until the h update)
                    nc.tensor.matmul(
                        ps_z[:, :N3], ident[:], gi_sb[:, 3:6, c, :],
                        start=True, stop=False,
                    )
                    for m in range(3):
                        for k in range(J):
                            nc.tensor.matmul(
                                msl(ps_z, m), whh_sb[:, k, 3 + m, :], h[:, k, :],
                                start=False, stop=(m == 2 and k == 2),
                            )
                    if Pl == 64:
                        warm(6)

                    z_sb = gates.tile([P, J, Pl], bf16, name="z_sb", tag="z_sb")
                    zc_sb = gates.tile([P, J, Pl], bf16, name="zc_sb", tag="zc_sb")
                    r_sb = gates.tile([P, J, Pl], bf16, name="r_sb", tag="r_sb")
                    n_sb = gates.tile([P, J, Pl], bf16, name="n_sb", tag="n_sb")
                    rhn = gates.tile([P, J, Pl], f32, name="rhn", tag="rhn")
                    t1 = gates.tile([P, J, Pl], f32, name="t1", tag="t1")
                    t2 = gates.tile([P, J, Pl], f32, name="t2", tag="t2")

                    nc.scalar.activation(r_sb[:], view3(ps_r), Sig)
                    nc.scalar.activation(z_sb[:], view3(ps_z), Sig)
                    nc.scalar.activation(zc_sb[:], view3(ps_z), Sig, scale=-1.0)
                    # t2 = z*h off-path while r/n compute
                    nc.gpsimd.tensor_tensor(out=t2[:], in0=z_sb[:], in1=h[:], op=Mult)
                    nc.vector.tensor_tensor(
                        out=rhn[:], in0=view3(ps_hn), in1=r_sb[:], op=Mult
                    )
                    nc.vector.tensor_tensor(
                        out=rhn[:], in0=rhn[:], in1=gi_sb[:, 6:9, c, :], op=Add
                    )
                    nc.scalar.activation(n_sb[:], rhn[:], Tanh)
                    # h' = zc*n + z*h
                    nc.vector.tensor_tensor(out=t1[:], in0=zc_sb[:], in1=n_sb[:], op=Mult)
                    nc.vector.tensor_tensor(out=h[:], in0=t1[:], in1=t2[:], op=Add)

                    if t == 0:
                        nc.gpsimd.tensor_copy(out=hacc[:], in_=h[:])
                    elif t == ARITY - 1 and Pl > 1:
                        csum = state.tile(
                            [P, J, Pl // ARITY], f32, name=f"csum{Pl}", tag=f"csum{Pl}"
                        )
                        level_csum[0] = csum
                        nc.vector.tensor_reduce(
                            out=csum[:],
                            in_=h[:].rearrange("p j (q c) -> p j q c", c=ARITY),
                            axis=mybir.AxisListType.X,
                            op=Add,
                        )
                        xn = x_in[Pl // ARITY]
                        nc.vector.tensor_tensor(
                            out=xn[:],
                            in0=hacc[:].rearrange("p j (q c) -> p j c q", c=ARITY),
                            in1=h[:].rearrange("p j (q c) -> p j c q", c=ARITY),
                            op=Add,
                        )
                    else:
                        nc.gpsimd.tensor_tensor(
                            out=hacc[:], in0=hacc[:], in1=h[:], op=Add
                        )

        # ---- outputs: [P, 2, J] = (x_root, h_root) ----
        out_t = state.tile([P, 2, J], f32, name="out_t", tag="out_t")
        nc.scalar.mul(out_t[:, 0], hacc[:, :, 0], 1.0 / ARITY)
        nc.vector.tensor_copy(out=out_t[:, 1], in_=h[:, :, 0])
        nc.sync.dma_start(out_xh[:], out_t[:])


def _build_program(n_leaves_core):
    if n_leaves_core in _PROG_CACHE:
        return _PROG_CACHE[n_leaves_core]
    import concourse.bacc as bacc
    import concourse.mybir as mybir
    import concourse.tile as tile

    f32 = mybir.dt.float32
    bf16 = mybir.dt.bfloat16

    nc = bacc.Bacc(
        "TRN2",
        target_bir_lowering=False,
        debug=False,
        enable_asserts=False,
        num_devices=NCORES,
        num_swdge_queues=4,
    )
    tokens = nc.dram_tensor(
        "tokens", [P, n_leaves_core // 16], mybir.dt.int16, kind="ExternalInput"
    ).ap()
    embed = nc.dram_tensor("embed", [VOCAB, DIM], bf16, kind="ExternalInput").ap()
    wih_t = nc.dram_tensor("wih_t", [P, J, 9, P], bf16, kind="ExternalInput").ap()
    wih_s = nc.dram_tensor("wih_s", [P, J, 9, P], bf16, kind="ExternalInput").ap()
    whh_t = nc.dram_tensor("whh_t", [P, J, 9, P], bf16, kind="ExternalInput").ap()
    biases = nc.dram_tensor("biases", [P, 12], f32, kind="ExternalInput").ap()
    biases_mm = nc.dram_tensor("biases_mm", [3, 4, P], bf16, kind="ExternalInput").ap()
    bias1 = nc.dram_tensor("bias1", [1, 9, P], bf16, kind="ExternalInput").ap()
    onehot3 = nc.dram_tensor("onehot3", [3, 3, 512], bf16, kind="ExternalInput").ap()
    ones = nc.dram_tensor("ones", [1, 512], bf16, kind="ExternalInput").ap()
    identity = nc.dram_tensor("identity", [P, P], bf16, kind="ExternalInput").ap()
    out_xh = nc.dram_tensor("out_xh", [P, 2, J], f32, kind="ExternalOutput").ap()

    with tile.TileContext(nc) as tc:
        _emit(
            tc,
            nc,
            (tokens, embed, wih_t, wih_s, whh_t, biases, biases_mm, bias1,
             onehot3, ones, identity, out_xh),
            n_leaves_core,
        )
    nc.compile()
    _PROG_CACHE[n_leaves_core] = nc
    return nc


def _retile_weights(w):
    # w: [1152, 384] -> lhsT tiles [128(k_part), 3(k), 9(m), 128(m_col)] bf16
    wt = np.ascontiguousarray(w.T)  # [384, 1152]
    wt = wt.reshape(J, P, 9, P).transpose(1, 0, 2, 3)
    return np.ascontiguousarray(wt).astype(BF16)


def _prep_bias(b_ih, b_hh):
    biases = np.zeros((P, 12), np.float32)
    comb = (b_ih + b_hh).reshape(9, P)
    biases[:, 0:6] = comb[0:6].T
    biases[:, 6:9] = b_hh.reshape(9, P)[6:9].T
    biases[:, 9:12] = b_ih.reshape(9, P)[6:9].T
    return biases


def _prep_bias_mm(b_ih, b_hh):
    # lhsT[k, ro, q] = bias[q, 3*ro + k]: the K=3 bias matmul against the
    # one-hot rhs yields out[q, (j, n)] = bias[q, 3*ro + j].
    b = _prep_bias(b_ih, b_hh)  # [128, 12] cols: r0..2 z0..2 hn0..2 in0..2
    out = b.T.reshape(4, 3, P).transpose(1, 0, 2)
    return np.ascontiguousarray(out).astype(BF16)


def _prep_bias1(b_ih, b_hh):
    # K=1 lhsT for the gi-precompute bias: out[col, :] += bias1[0, m, col].
    # r/z rows carry the combined input+hidden bias; n rows carry b_in only.
    out = np.zeros((1, 9, P), np.float32)
    comb = (b_ih + b_hh).reshape(9, P)
    out[0, 0:6] = comb[0:6]
    out[0, 6:9] = b_ih.reshape(9, P)[6:9]
    return out.astype(BF16)


def _wrap_tokens(tok):
    # dma_gather idx layout: idx i of a 512-row call lives at partition i%16,
    # column i//16, replicated across the 8 Q7 core partition-groups. Each
    # call's 512 indices are pre-permuted child-major (position c*64+q is
    # leaf 8q+c), so the transposing gather writes the x tile directly in
    # [P, J, 8 child, 64 parent] order.
    arr = np.zeros((P, len(tok) // 16), np.int16)
    for b in range(len(tok) // 512):
        blk = tok[b * 512 : (b + 1) * 512].astype(np.int16)
        cm = np.ascontiguousarray(blk.reshape(64, 8).T).reshape(-1)
        m = cm.reshape(32, 16).T  # [16, 32]
        arr[:, b * 32 : (b + 1) * 32] = np.tile(m, (8, 1))
    return arr


def _prep_onehot3():
    out = np.zeros((3, 3, 512), np.float32)
    for k in range(3):
        out[k, k, :] = 1.0
    return out.astype(BF16)


def _gru_gates(x_t, h, w_ih, w_hh, b_ih, b_hh):
    gi = x_t @ w_ih.T + b_ih
    gh = h @ w_hh.T + b_hh
    i_r, i_z, i_n = np.split(gi, 3, axis=-1)
    h_r, h_z, h_n = np.split(gh, 3, axis=-1)
    r = 1.0 / (1.0 + np.exp(-(i_r + h_r)))
    z = 1.0 / (1.0 + np.exp(-(i_z + h_z)))
    n = np.tanh(i_n + r * h_n)
    return (1.0 - z) * n + z * h


def _root_gru(x_children, h0, w_ih, w_hh, b_ih, b_hh):
    h = h0.astype(np.float64)
    acc = np.zeros_like(h)
    for t in range(ARITY):
        x_t = x_children[ARITY - 1 - t].astype(np.float64)
        h = _gru_gates(x_t, h, w_ih.astype(np.float64), w_hh.astype(np.float64),
                       b_ih.astype(np.float64), b_hh.astype(np.float64))
        acc += h
    return (acc / ARITY).astype(np.float32)


def kernel(leaf_tokens, embed_table, w_ih, w_hh, b_ih, b_hh):
    from concourse.bass_utils import run_bass_kernel_spmd

    leaf_tokens = np.asarray(leaf_tokens, np.int32)
    embed_table = np.asarray(embed_table, np.float32)
    w_ih = np.asarray(w_ih, np.float32)
    w_hh = np.asarray(w_hh, np.float32)
    b_ih = np.asarray(b_ih, np.float32)
    b_hh = np.asarray(b_hh, np.float32)

    nc = _build_program(LEAVES_CORE)

    embed_bf = embed_table.astype(BF16)
    wih_t = _retile_weights(w_ih)
    wih_s = _retile_weights(w_ih / ARITY)
    whh_t = _retile_weights(w_hh)
    biases = _prep_bias(b_ih, b_hh)
    biases_mm = _prep_bias_mm(b_ih, b_hh)
    bias1 = _prep_bias1(b_ih, b_hh)
    ones = np.ones((1, 512), np.float32).astype(BF16)
    in_maps = []
    for core in range(NCORES):
        in_maps.append(
            {
                "tokens": _wrap_tokens(
                    leaf_tokens[core * LEAVES_CORE : (core + 1) * LEAVES_CORE]
                ),
                "embed": embed_bf,
                "wih_t": wih_t,
                "wih_s": wih_s,
                "whh_t": whh_t,
                "biases": biases,
                "biases_mm": biases_mm,
                "bias1": bias1,
                "onehot3": _prep_onehot3(),
                "ones": ones,
                "identity": np.eye(P, dtype=np.float32).astype(BF16),
            }
        )
    res = run_bass_kernel_spmd(nc, in_maps, core_ids=list(range(NCORES)))

    xs = np.zeros((NCORES, DIM), np.float32)
    h8 = np.zeros((NCORES, DIM), np.float32)
    for core in range(NCORES):
        out = res.results[core]["out_xh"]  # [P, 2, J]
        xs[core] = out[:, 0].T.reshape(-1)
        h8[core] = out[:, 1].T.reshape(-1)

    h0 = h8.mean(axis=0)
    out = _root_gru(xs, h0, w_ih, w_hh, b_ih, b_hh)
    return out.reshape(1, 1, DIM)


# revision 38
# speedup vs baseline: 1.2783x; 1.0139x over previous
"""Tree-GRU (arity-8, depth-5) over embedded leaves on 8 TRN2 NeuronCores.

Sharding: data-parallel over subtrees. Each core takes 4096 contiguous leaves
and runs levels 5..2 of the tree locally (512 -> 64 -> 8 -> 1 parents). The
root (level 1, 8 children = the 8 cores' level-2 outputs) is a trivial
16-matvec GRU done on host after gathering the per-core [384] outputs.

Device layout is feature-transposed: tensors live as [128 part, 3 ktile, ...]
with feature f = 128*k + p, so the GRU matmuls contract the partition dim.
Node storage is child-major, so matmul rhs slices are contiguous. Weights
are host-pre-transposed into lhsT tiles; matmul dtype bf16 with fp32 PSUM
accumulation.

Leaf level: the embedding gather uses the transposing dma_gather ucode
(int16 index lists, 512 rows per call, 2 SWDGE queues) with the indices
pre-permuted child-major on host, so each call lands directly as a
feature-transposed [P, J, 8 child, 64 parent] x tile -- no PE transposes or
PSUM->SBUF copies. Step 0 (h=0, gi-only) runs in 4 sub-chunks that track
gather completion; steps 1-7 ping-pong 2 chunks of 256 parents with the
emission order interleaved so the in-order engine FIFOs receive each piece
near its data-arrival time.

Small levels (64/8/1 parents): the input transform gi for the whole level
(all 8 children x all parents) is precomputed in one batched matmul pass
(biases folded in via a K=1 ones-column matmul) and stored in SBUF, so the
sequential per-step work is only the recurrent matmul + gate chain. Per step
the r/z gi rows are injected into PSUM via an identity matmul (one start=True
covering MM per bank), the hn bias via a K=3 one-hot matmul, and the h update
uses h' = zc*n + z*h with zc = sigmoid(-pre_z) so z*h is computed off-path
(gpsimd) while tanh runs. The 1/8 output-mean scale is folded into a
pre-scaled copy of W_ih used by the gi passes; per-step output sums are
accumulated (hacc) and the final step writes the next level's input directly
as a fused raw-sum add.
"""

import numpy as np
import ml_dtypes

ARITY = 8
DIM = 384
VOCAB = 32000
NCORES = 8
P = 128
J = 3  # DIM // 128 feature tiles
N_LEAVES = 32768
LEAVES_CORE = N_LEAVES // NCORES  # 4096

BF16 = ml_dtypes.bfloat16

_PROG_CACHE = {}


def _levels_for(n_leaves_core):
    levels = []
    p = n_leaves_core // ARITY
    while p >= 1:
        levels.append(p)
        p //= ARITY
    assert levels[-1] == 1
    return levels


def _emit(tc, nc, aps, n_leaves_core):
    import concourse.mybir as mybir
    import concourse.bass as bass

    f32 = mybir.dt.float32
    bf16 = mybir.dt.bfloat16
    Sig = mybir.ActivationFunctionType.Sigmoid
    Tanh = mybir.ActivationFunctionType.Tanh
    Add = mybir.AluOpType.add
    Sub = mybir.AluOpType.subtract
    Mult = mybir.AluOpType.mult

    (tokens, embed, wih_t, wih_s, whh_t, biases, biases_mm, bias1, onehot3,
     ones, identity, out_xh) = aps
    levels = _levels_for(n_leaves_core)
    P5 = levels[0]
    n_gtiles = n_leaves_core // P  # 32

    from contextlib import ExitStack

    with ExitStack() as ctx:
        const = ctx.enter_context(tc.tile_pool(name="const", bufs=1))
        xpool = ctx.enter_context(tc.tile_pool(name="xpool", bufs=1))
        state = ctx.enter_context(tc.tile_pool(name="state", bufs=1))
        gates = ctx.enter_context(tc.tile_pool(name="gates", bufs=8))
        gpool = ctx.enter_context(tc.tile_pool(name="gpool", bufs=1))
        pspool = ctx.enter_context(tc.tile_pool(name="pspool", bufs=8, space="PSUM"))

        # ---- tokens first, then kick off all gathers (dma_gather ucode,
        # int16 idx lists, 512 rows per call; 2 SWDGE queues). The gathers
        # are paced by gpsimd descriptor generation, so nothing else may sit
        # ahead of them in the gpsimd FIFO -- the identity build comes after.
        from concourse.library_config import mlp as _mlp_lib

        GB = 512  # rows per dma_gather call (64 parents, child-major)
        n_gcalls = n_leaves_core // GB  # 8
        QB = GB // ARITY  # 64 parents per call tile
        tok_sb = const.tile([P, n_gcalls * (GB // 16)], mybir.dt.int16)
        nc.sync.dma_start(tok_sb[:], tokens[:])
        nc.gpsimd.load_library(_mlp_lib)

        # transpose=True lands each embedding row across partitions with
        # feature f = 128*j + p -- exactly the matmul lhsT layout -- and the
        # host orders each call's 512 indices child-major, so the tile is
        # directly the [P, J, 8, 64] x input. No PE transposes, no copies.
        xts = []
        for b in range(n_gcalls):
            xt = gpool.tile([P, J, GB], bf16, name="xt", tag="xt", bufs=n_gcalls)
            nc.gpsimd.dma_gather(
                xt[:],
                embed[:],
                tok_sb[:, b * (GB // 16) : (b + 1) * (GB // 16)],
                GB,
                GB,
                DIM,
                transpose=True,
                queue_num=(b % 2),
            )
            xts.append(xt)

        def xtv(b):
            # child-sliced view of call b: [P, J, 8, QB]
            return xts[b][:].rearrange("p j (c q) -> p j c q", q=QB)

        # identity comes in via DMA so no gpsimd work sits ahead of the
        # gather descriptor generation
        ident = const.tile([P, P], bf16)
        nc.sync.dma_start(ident[:], identity[:])

        # ---- constants / weights ----
        wih_sb = const.tile([P, J, 9, P], bf16)
        wih_s_sb = const.tile([P, J, 9, P], bf16)
        whh_sb = const.tile([P, J, 9, P], bf16)
        bias_sb = const.tile([P, 12], f32)
        bias3_sb = const.tile([3, 4, P], bf16)
        bias1_sb = const.tile([1, 9, P], bf16)
        onehot3_sb = const.tile([3, 3, 512], bf16)
        ones_sb = const.tile([1, 512], bf16)
        nc.sync.dma_start(wih_sb[:], wih_t[:])
        nc.sync.dma_start(wih_s_sb[:], wih_s[:])
        nc.sync.dma_start(whh_sb[:], whh_t[:])
        nc.sync.dma_start(bias_sb[:], biases[:])
        nc.sync.dma_start(bias3_sb[:], biases_mm[:])
        nc.sync.dma_start(bias1_sb[:], bias1[:])
        nc.sync.dma_start(onehot3_sb[:], onehot3[:])
        nc.sync.dma_start(ones_sb[:], ones[:])

        # child-major x per level: [P, J, 8, Pl] (contiguous matmul rhs)
        x_in = {}
        for Pl in levels[1:]:
            x_in[Pl] = xpool.tile([P, J, ARITY, Pl], bf16, name=f"x{Pl}", tag=f"x{Pl}")

        # gi stored child-major [P, 9, 8, Pl]: the gi pass rhs is the
        # child-major x (contiguous), so PSUM comes out (c, q)-ordered
        gi_tiles = {}
        for Pl in levels[1:]:
            gi_tiles[Pl] = xpool.tile([P, 9, ARITY, Pl], bf16, name=f"gi{Pl}",
                                      tag=f"gi{Pl}")

        def psum_tile():
            return pspool.tile([P, 512], f32, name="ps", tag="ps")

        def warm(n):
            # PE warm-keeper: HAM re-throttles the PE clock to 1.2 GHz after
            # ~3.4us of idle; during the latency-bound small levels, issue
            # dependency-free matmuls so the gate-chain windows don't cool
            # the PE and the next real matmul burst runs at 2.4 GHz.
            for _ in range(n):
                wp = pspool.tile([P, 512], f32, name="warm", tag="ps")
                nc.tensor.matmul(
                    wp[:, :512], ident[:], wih_sb[:, 0, 0:4, :], start=True, stop=True
                )

        def new_state(name, dtype, Pl):
            return state.tile([P, J, Pl], dtype, name=f"{name}{Pl}", tag=f"{name}{Pl}")

        # =====================  LEAF LEVEL (Pl = P5)  =====================
        h = new_state("h", bf16, P5)
        # permuted (child-major) accumulator so the level-end x_next add is a
        # contiguous write into the next level's child-major x
        hacc = state.tile([P, J, ARITY, P5 // ARITY], f32, name="hacc5", tag="hacc5")

        level_csum = [None]

        NSC = P5 // 4  # 128 parents per t0 sub-chunk
        c0 = ARITY - 1  # first GRU input is the last child

        def emit_t0_sub(sc):
            # step 0: h=0, gi only. Biases are injected into PSUM via the
            # K=3 one-hot matmul (the single start=True per bank), so the
            # activations span all 3 m-tiles in one op.
            with nc.named_scope(f"leaf_t0s{sc}"):
                sl = slice(sc * NSC, (sc + 1) * NSC)
                N3 = 3 * NSC
                ps_r, ps_z, ps_in = psum_tile(), psum_tile(), psum_tile()

                def view3s(pst):
                    return pst[:, :N3].rearrange("p (j n) -> p j n", j=3)

                for pst, ro, moff in ((ps_r, 0, 0), (ps_z, 1, 3), (ps_in, 3, 6)):
                    nc.tensor.matmul(
                        pst[:, :N3], bias3_sb[:, ro, :], onehot3_sb[:, :, :NSC],
                        start=True, stop=False,
                    )
                    for m in range(3):
                        for k in range(J):
                            for b2 in range(2):
                                nc.tensor.matmul(
                                    pst[:, m * NSC + b2 * QB : m * NSC + (b2 + 1) * QB],
                                    wih_sb[:, k, moff + m, :],
                                    xtv(2 * sc + b2)[:, k, c0, :],
                                    start=False,
                                    stop=(m == 2 and k == 2 and b2 == 1),
                                )
                r_sb = gates.tile([P, J, NSC], bf16, name="r0", tag="r0")
                z_sb = gates.tile([P, J, NSC], bf16, name="z0", tag="z0")
                n_sb = gates.tile([P, J, NSC], bf16, name="n0", tag="n0")
                rhn = gates.tile([P, J, NSC], f32, name="rhn0", tag="rhn0")
                t1 = gates.tile([P, J, NSC], bf16, name="t10", tag="t10")
                nc.scalar.activation(r_sb[:], view3s(ps_r), Sig)
                nc.scalar.activation(z_sb[:], view3s(ps_z), Sig)
                # n = tanh(i_n + b_in + r*b_hn): gh_n of the zero state is
                # just b_hn, folded in per m via the scalar port
                for m in range(3):
                    nc.vector.scalar_tensor_tensor(
                        out=rhn[:, m],
                        in0=r_sb[:, m],
                        scalar=bias_sb[:, 6 + m : 7 + m],
                        in1=view3s(ps_in)[:, m],
                        op0=Mult,
                        op1=Add,
                    )
                nc.scalar.activation(n_sb[:], rhn[:], Tanh)
                # h0=0: h' = n - z*n
                hsl = h[:, :, sl]
                nc.vector.tensor_tensor(out=t1[:], in0=z_sb[:], in1=n_sb[:], op=Mult)
                nc.vector.tensor_tensor(out=hsl, in0=n_sb[:], in1=t1[:], op=Sub)
                nc.gpsimd.tensor_copy(
                    out=hacc[:, :, :, sc * (NSC // ARITY) : (sc + 1) * (NSC // ARITY)],
                    in_=hsl.rearrange("p j (q c) -> p j c q", c=ARITY),
                )

        NCH = 256

        def emit_step(t, ch):
            c = ARITY - 1 - t
            with nc.named_scope(f"leaf_t{t}c{ch}"):
                sl = slice(ch * NCH, (ch + 1) * NCH)
                ps_r = [psum_tile()[:, :NCH] for _ in range(3)]
                ps_z = [psum_tile()[:, :NCH] for _ in range(3)]
                ps_in = [psum_tile()[:, :NCH] for _ in range(3)]
                ps_hn = [psum_tile()[:, :NCH] for _ in range(3)]
                for ps, moff in ((ps_r, 0), (ps_z, 3), (ps_in, 6)):
                    for m in range(3):
                        for k in range(J):
                            for b4 in range(4):
                                nc.tensor.matmul(
                                    ps[m][:, b4 * QB : (b4 + 1) * QB],
                                    wih_sb[:, k, moff + m, :],
                                    xtv(4 * ch + b4)[:, k, c, :],
                                    start=(k == 0 and b4 == 0),
                                    stop=(k == 2 and moff == 6 and b4 == 3),
                                )
                for ps, moff in ((ps_r, 0), (ps_z, 3), (ps_hn, 6)):
                    for m in range(3):
                        for k in range(J):
                            nc.tensor.matmul(
                                ps[m],
                                whh_sb[:, k, moff + m, :],
                                h[:, k, sl],
                                start=(k == 0 and moff == 6),
                                stop=(k == 2),
                            )

                r_sb = gates.tile([P, J, NCH], bf16, name="r_sb", tag="r_sb")
                z_sb = gates.tile([P, J, NCH], bf16, name="z_sb", tag="z_sb")
                n_sb = gates.tile([P, J, NCH], bf16, name="n_sb", tag="n_sb")
                rhn = gates.tile([P, J, NCH], f32, name="rhn", tag="rhn")
                t1 = gates.tile([P, J, NCH], bf16, name="t1", tag="t1")

                for m in range(3):
                    nc.scalar.activation(
                        r_sb[:, m], ps_r[m], Sig, bias=bias_sb[:, m : m + 1]
                    )
                for m in range(3):
                    nc.scalar.activation(
                        z_sb[:, m], ps_z[m], Sig, bias=bias_sb[:, 3 + m : 4 + m]
                    )
                for m in range(3):
                    nc.vector.scalar_tensor_tensor(
                        out=rhn[:, m],
                        in0=ps_hn[m],
                        scalar=bias_sb[:, 6 + m : 7 + m],
                        in1=r_sb[:, m],
                        op0=Add,
                        op1=Mult,
                    )
                for m in range(3):
                    nc.vector.tensor_tensor(
                        out=rhn[:, m], in0=rhn[:, m], in1=ps_in[m], op=Add
                    )
                for m in range(3):
                    nc.scalar.activation(
                        n_sb[:, m], rhn[:, m], Tanh, bias=bias_sb[:, 9 + m : 10 + m]
                    )

                # h' = n + z*(h - n)
                hsl = h[:, :, sl]
                nc.vector.tensor_tensor(out=t1[:], in0=hsl, in1=n_sb[:], op=Sub)
                nc.vector.tensor_tensor(out=t1[:], in0=z_sb[:], in1=t1[:], op=Mult)
                nc.vector.tensor_tensor(out=hsl, in0=n_sb[:], in1=t1[:], op=Add)
                hperm = hsl.rearrange("p j (q c) -> p j c q", c=ARITY)
                qsl = slice(ch * NCH // ARITY, (ch + 1) * NCH // ARITY)
                if t == ARITY - 1:
                    if ch == 0:
                        csum = state.tile(
                            [P, J, P5 // ARITY], f32, name="csum5", tag="csum5"
                        )
                        level_csum[0] = csum
                    nc.vector.tensor_reduce(
                        out=level_csum[0][:, :, qsl],
                        in_=hsl.rearrange("p j (q c) -> p j q c", c=ARITY),
                        axis=mybir.AxisListType.X,
                        op=Add,
                    )
                    xn = x_in[P5 // ARITY]
                    for j in range(J):
                        eng = nc.gpsimd if j == 2 else nc.vector
                        eng.tensor_tensor(
                            out=xn[:, j, :, qsl],
                            in0=hacc[:, j, :, qsl],
                            in1=hperm[:, j],
                            op=Add,
                        )
                    # bridge the level-end tail so the PE stays warm into
                    # the gi_64 pass
                    warm(8)
                else:
                    nc.gpsimd.tensor_tensor(
                        out=hacc[:, :, :, qsl],
                        in0=hacc[:, :, :, qsl],
                        in1=hperm,
                        op=Add,
                    )

        def emit_gi64_half(half):
            # gi pass for half the level-64 parents, overlapped with the
            # other leaf chunk's trailing steps
            Pl = P5 // ARITY
            with nc.named_scope(f"gi64h{half}"):
                gi_sb = gi_tiles[Pl]
                hQ = Pl // 2
                for m in (0, 1, 2, 3, 4, 5, 6, 7, 8):
                    ps = psum_tile()[:, : ARITY * hQ]
                    nc.tensor.matmul(
                        ps, bias1_sb[:, m, :], ones_sb[:, : ARITY * hQ],
                        start=True, stop=False,
                    )
                    for k in range(J):
                        nc.tensor.matmul(
                            ps,
                            wih_s_sb[:, k, m, :],
                            x_in[Pl][:, k, :, half * hQ : (half + 1) * hQ],
                            start=False,
                            stop=(k == 2),
                        )
                    eng = nc.vector if m % 2 == 0 else nc.scalar
                    psv = ps.rearrange("p (c q) -> p c q", q=hQ)
                    if m % 2 == 0:
                        nc.vector.tensor_copy(
                            out=gi_sb[:, m, :, half * hQ : (half + 1) * hQ], in_=psv
                        )
                    else:
                        nc.scalar.copy(
                            out=gi_sb[:, m, :, half * hQ : (half + 1) * hQ], in_=psv
                        )

        # Interleaved emission: engine FIFOs are in-order, so each piece is
        # queued roughly at its data-arrival time -- t0 sub-chunks track the
        # gathers, chunk B's steps slot between chunk A's from t=1 on.
        emit_t0_sub(0)
        emit_t0_sub(1)
        emit_t0_sub(2)
        emit_step(1, 0)
        emit_t0_sub(3)
        emit_step(2, 0)
        for t in range(3, ARITY):
            emit_step(t - 2, 1)
            emit_step(t, 0)
        emit_step(ARITY - 2, 1)
        emit_gi64_half(0)
        emit_step(ARITY - 1, 1)
        emit_gi64_half(1)

        # =====================  SMALL LEVELS (64, 8, 1)  ==================
        for Pl in levels[1:]:
            NC8 = ARITY * Pl  # children count = gi batch size
            gi_sb = gi_tiles[Pl]
            if Pl != P5 // ARITY:
              with nc.named_scope(f"gi_{Pl}"):
                # m-order: r (0,1,2) first so step 0's r-inject unblocks
                # early, then z (3,4,5), then n (6,7,8)
                for mi, m in enumerate((0, 1, 2, 3, 4, 5, 6, 7, 8)):
                    ps = psum_tile()[:, :NC8]
                    nc.tensor.matmul(
                        ps, bias1_sb[:, m, :], ones_sb[:, :NC8],
                        start=True, stop=False,
                    )
                    for k in range(J):
                        nc.tensor.matmul(
                            ps,
                            wih_s_sb[:, k, m, :],
                            x_in[Pl][:, k, :, :],
                            start=False,
                            stop=(k == 2),
                        )
                    # alternate copy engine so the PSUM->SBUF drain keeps up
                    # with the matmul waves
                    if mi % 2 == 0:
                        nc.vector.tensor_copy(
                            out=gi_sb[:, m].rearrange("p c q -> p (c q)"), in_=ps
                        )
                    else:
                        nc.scalar.copy(
                            out=gi_sb[:, m].rearrange("p c q -> p (c q)"), in_=ps
                        )

            csum = level_csum[0]
            h = new_state("h", bf16, Pl)
            hacc = new_state("hacc", f32, Pl)
            nc.scalar.mul(h[:], csum[:], 1.0 / ARITY)

            for t in range(ARITY):
                c = ARITY - 1 - t
                with nc.named_scope(f"lv{Pl}_t{t}"):
                    N3 = 3 * Pl
                    ps_z, ps_r, ps_hn = psum_tile(), psum_tile(), psum_tile()

                    def view3(pst):
                        return pst[:, :N3].rearrange("p (j n) -> p j n", j=3)

                    def msl(pst, m):
                        return pst[:, m * Pl : (m + 1) * Pl]

                    # r first: sigma(r) heads the serial chain, so its PSUM
                    # group must close first and nothing may sit ahead of
                    # sigma(r) in the scalar FIFO
                    nc.tensor.matmul(
                        ps_r[:, :N3], ident[:], gi_sb[:, 0:3, c, :],
                        start=True, stop=False,
                    )
                    for m in range(3):
                        for k in range(J):
                            nc.tensor.matmul(
                                msl(ps_r, m), whh_sb[:, k, m, :], h[:, k, :],
                                start=False, stop=(m == 2 and k == 2),
                            )
                    # hn second so its accumulation closes before z's: the
                    # serial path is sigma(r) -> rhn = ps_hn*r -> tanh
                    nc.tensor.matmul(
                        ps_hn[:, :N3], bias3_sb[:, 2, :], onehot3_sb[:, :, :Pl],
                        start=True, stop=False,
                    )
                    for m in range(3):
                        for k in range(J):
                            nc.tensor.matmul(
                                msl(ps_hn, m), whh_sb[:, k, 6 + m, :], h[:, k, :],
                                start=False, stop=(m == 2 and k == 2),
                            )
                    # z last (sigma(z)/zc/t2 have slack until the h update)
                    nc.tensor.matmul(
                        ps_z[:, :N3], ident[:], gi_sb[:, 3:6, c, :],
                        start=True, stop=False,
                    )
                    for m in range(3):
                        for k in range(J):
                            nc.tensor.matmul(
                                msl(ps_z, m), whh_sb[:, k, 3 + m, :], h[:, k, :],
                                start=False, stop=(m == 2 and k == 2),
                            )
                    if Pl == 64:
                        warm(6)

                    z_sb = gates.tile([P, J, Pl], bf16, name="z_sb", tag="z_sb")
                    zc_sb = gates.tile([P, J, Pl], bf16, name="zc_sb", tag="zc_sb")
                    r_sb = gates.tile([P, J, Pl], bf16, name="r_sb", tag="r_sb")
                    n_sb = gates.tile([P, J, Pl], bf16, name="n_sb", tag="n_sb")
                    rhn = gates.tile([P, J, Pl], f32, name="rhn", tag="rhn")
                    t1 = gates.tile([P, J, Pl], f32, name="t1", tag="t1")
                    t2 = gates.tile([P, J, Pl], f32, name="t2", tag="t2")

                    nc.scalar.activation(r_sb[:], view3(ps_r), Sig)
                    nc.scalar.activation(z_sb[:], view3(ps_z), Sig)
                    nc.scalar.activation(zc_sb[:], view3(ps_z), Sig, scale=-1.0)
                    # t2 = z*h off-path while r/n compute
                    nc.gpsimd.tensor_tensor(out=t2[:], in0=z_sb[:], in1=h[:], op=Mult)
                    nc.vector.tensor_tensor(
                        out=rhn[:], in0=view3(ps_hn), in1=r_sb[:], op=Mult
                    )
                    nc.vector.tensor_tensor(
                        out=rhn[:], in0=rhn[:], in1=gi_sb[:, 6:9, c, :], op=Add
                    )
                    nc.scalar.activation(n_sb[:], rhn[:], Tanh)
                    # h' = zc*n + z*h
                    nc.vector.tensor_tensor(out=t1[:], in0=zc_sb[:], in1=n_sb[:], op=Mult)
                    nc.vector.tensor_tensor(out=h[:], in0=t1[:], in1=t2[:], op=Add)

                    if t == 0:
                        nc.gpsimd.tensor_copy(out=hacc[:], in_=h[:])
                    elif t == ARITY - 1 and Pl > 1:
                        csum = state.tile(
                            [P, J, Pl // ARITY], f32, name=f"csum{Pl}", tag=f"csum{Pl}"
                        )
                        level_csum[0] = csum
                        nc.vector.tensor_reduce(
                            out=csum[:],
                            in_=h[:].rearrange("p j (q c) -> p j q c", c=ARITY),
                            axis=mybir.AxisListType.X,
                            op=Add,
                        )
                        xn = x_in[Pl // ARITY]
                        nc.vector.tensor_tensor(
                            out=xn[:],
                            in0=hacc[:].rearrange("p j (q c) -> p j c q", c=ARITY),
                            in1=h[:].rearrange("p j (q c) -> p j c q", c=ARITY),
                            op=Add,
                        )
                    else:
                        nc.gpsimd.tensor_tensor(
                            out=hacc[:], in0=hacc[:], in1=h[:], op=Add
                        )

        # ---- outputs: [P, 2, J] = (x_root, h_root) ----
        out_t = state.tile([P, 2, J], f32, name="out_t", tag="out_t")
        nc.scalar.mul(out_t[:, 0], hacc[:, :, 0], 1.0 / ARITY)
        nc.vector.tensor_copy(out=out_t[:, 1], in_=h[:, :, 0])
        nc.sync.dma_start(out_xh[:], out_t[:])


def _build_program(n_leaves_core):
    if n_leaves_core in _PROG_CACHE:
        return _PROG_CACHE[n_leaves_core]
    import concourse.bacc as bacc
    import concourse.mybir as mybir
    import concourse.tile as tile

    f32 = mybir.dt.float32
    bf16 = mybir.dt.bfloat16

    nc = bacc.Bacc(
        "TRN2",
        target_bir_lowering=False,
        debug=False,
        enable_asserts=False,
        num_devices=NCORES,
        num_swdge_queues=4,
    )
    tokens = nc.dram_tensor(
        "tokens", [P, n_leaves_core // 16], mybir.dt.int16, kind="ExternalInput"
    ).ap()
    embed = nc.dram_tensor("embed", [VOCAB, DIM], bf16, kind="ExternalInput").ap()
    wih_t = nc.dram_tensor("wih_t", [P, J, 9, P], bf16, kind="ExternalInput").ap()
    wih_s = nc.dram_tensor("wih_s", [P, J, 9, P], bf16, kind="ExternalInput").ap()
    whh_t = nc.dram_tensor("whh_t", [P, J, 9, P], bf16, kind="ExternalInput").ap()
    biases = nc.dram_tensor("biases", [P, 12], f32, kind="ExternalInput").ap()
    biases_mm = nc.dram_tensor("biases_mm", [3, 4, P], bf16, kind="ExternalInput").ap()
    bias1 = nc.dram_tensor("bias1", [1, 9, P], bf16, kind="ExternalInput").ap()
    onehot3 = nc.dram_tensor("onehot3", [3, 3, 512], bf16, kind="ExternalInput").ap()
    ones = nc.dram_tensor("ones", [1, 512], bf16, kind="ExternalInput").ap()
    identity = nc.dram_tensor("identity", [P, P], bf16, kind="ExternalInput").ap()
    out_xh = nc.dram_tensor("out_xh", [P, 2, J], f32, kind="ExternalOutput").ap()

    with tile.TileContext(nc) as tc:
        _emit(
            tc,
            nc,
            (tokens, embed, wih_t, wih_s, whh_t, biases, biases_mm, bias1,
             onehot3, ones, identity, out_xh),
            n_leaves_core,
        )
    nc.compile()
    _PROG_CACHE[n_leaves_core] = nc
    return nc


def _retile_weights(w):
    # w: [1152, 384] -> lhsT tiles [128(k_part), 3(k), 9(m), 128(m_col)] bf16
    wt = np.ascontiguousarray(w.T)  # [384, 1152]
    wt = wt.reshape(J, P, 9, P).transpose(1, 0, 2, 3)
    return np.ascontiguousarray(wt).astype(BF16)


def _prep_bias(b_ih, b_hh):
    biases = np.zeros((P, 12), np.float32)
    comb = (b_ih + b_hh).reshape(9, P)
    biases[:, 0:6] = comb[0:6].T
    biases[:, 6:9] = b_hh.reshape(9, P)[6:9].T
    biases[:, 9:12] = b_ih.reshape(9, P)[6:9].T
    return biases


def _prep_bias_mm(b_ih, b_hh):
    # lhsT[k, ro, q] = bias[q, 3*ro + k]: the K=3 bias matmul against the
    # one-hot rhs yields out[q, (j, n)] = bias[q, 3*ro + j].
    b = _prep_bias(b_ih, b_hh)  # [128, 12] cols: r0..2 z0..2 hn0..2 in0..2
    out = b.T.reshape(4, 3, P).transpose(1, 0, 2)
    return np.ascontiguousarray(out).astype(BF16)


def _prep_bias1(b_ih, b_hh):
    # K=1 lhsT for the gi-precompute bias: out[col, :] += bias1[0, m, col].
    # r/z rows carry the combined input+hidden bias; n rows carry b_in only.
    out = np.zeros((1, 9, P), np.float32)
    comb = (b_ih + b_hh).reshape(9, P)
    out[0, 0:6] = comb[0:6]
    out[0, 6:9] = b_ih.reshape(9, P)[6:9]
    return out.astype(BF16)


def _wrap_tokens(tok):
    # dma_gather idx layout: idx i of a 512-row call lives at partition i%16,
    # column i//16, replicated across the 8 Q7 core partition-groups. Each
    # call's 512 indices are pre-permuted child-major (position c*64+q is
    # leaf 8q+c), so the transposing gather writes the x tile directly in
    # [P, J, 8 child, 64 parent] order.
    arr = np.zeros((P, len(tok) // 16), np.int16)
    for b in range(len(tok) // 512):
        blk = tok[b * 512 : (b + 1) * 512].astype(np.int16)
        cm = np.ascontiguousarray(blk.reshape(64, 8).T).reshape(-1)
        m = cm.reshape(32, 16).T  # [16, 32]
        arr[:, b * 32 : (b + 1) * 32] = np.tile(m, (8, 1))
    return arr


def _prep_onehot3():
    out = np.zeros((3, 3, 512), np.float32)
    for k in range(3):
        out[k, k, :] = 1.0
    return out.astype(BF16)


def _gru_gates(x_t, h, w_ih, w_hh, b_ih, b_hh):
    gi = x_t @ w_ih.T + b_ih
    gh = h @ w_hh.T + b_hh
    i_r, i_z, i_n = np.split(gi, 3, axis=-1)
    h_r, h_z, h_n = np.split(gh, 3, axis=-1)
    r = 1.0 / (1.0 + np.exp(-(i_r + h_r)))
    z = 1.0 / (1.0 + np.exp(-(i_z + h_z)))
    n = np.tanh(i_n + r * h_n)
    return (1.0 - z) * n + z * h


def _root_gru(x_children, h0, w_ih, w_hh, b_ih, b_hh):
    h = h0.astype(np.float64)
    acc = np.zeros_like(h)
    for t in range(ARITY):
        x_t = x_children[ARITY - 1 - t].astype(np.float64)
        h = _gru_gates(x_t, h, w_ih.astype(np.float64), w_hh.astype(np.float64),
                       b_ih.astype(np.float64), b_hh.astype(np.float64))
        acc += h
    return (acc / ARITY).astype(np.float32)


def kernel(leaf_tokens, embed_table, w_ih, w_hh, b_ih, b_hh):
    from concourse.bass_utils import run_bass_kernel_spmd

    leaf_tokens = np.asarray(leaf_tokens, np.int32)
    embed_table = np.asarray(embed_table, np.float32)
    w_ih = np.asarray(w_ih, np.float32)
    w_hh = np.asarray(w_hh, np.float32)
    b_ih = np.asarray(b_ih, np.float32)
    b_hh = np.asarray(b_hh, np.float32)

    nc = _build_program(LEAVES_CORE)

    embed_bf = embed_table.astype(BF16)
    wih_t = _retile_weights(w_ih)
    wih_s = _retile_weights(w_ih / ARITY)
    whh_t = _retile_weights(w_hh)
    biases = _prep_bias(b_ih, b_hh)
    biases_mm = _prep_bias_mm(b_ih, b_hh)
    bias1 = _prep_bias1(b_ih, b_hh)
    ones = np.ones((1, 512), np.float32).astype(BF16)
    in_maps = []
    for core in range(NCORES):
        in_maps.append(
            {
                "tokens": _wrap_tokens(
                    leaf_tokens[core * LEAVES_CORE : (core + 1) * LEAVES_CORE]
                ),
                "embed": embed_bf,
                "wih_t": wih_t,
                "wih_s": wih_s,
                "whh_t": whh_t,
                "biases": biases,
                "biases_mm": biases_mm,
                "bias1": bias1,
                "onehot3": _prep_onehot3(),
                "ones": ones,
                "identity": np.eye(P, dtype=np.float32).astype(BF16),
            }
        )
    res = run_bass_kernel_spmd(nc, in_maps, core_ids=list(range(NCORES)))

    xs = np.zeros((NCORES, DIM), np.float32)
    h8 = np.zeros((NCORES, DIM), np.float32)
    for core in range(NCORES):
        out = res.results[core]["out_xh"]  # [P, 2, J]
        xs[core] = out[:, 0].T.reshape(-1)
        h8[core] = out[:, 1].T.reshape(-1)

    h0 = h8.mean(axis=0)
    out = _root_gru(xs, h0, w_ih, w_hh, b_ih, b_hh)
    return out.reshape(1, 1, DIM)
